# revision 21
# baseline (speedup 1.0000x reference)
"""Trainium2 8-core tensor-parallel transformer layer — v10.

On top of v9 (MLP 4hh row-parallel + per-chunk ReduceScatter):
- Dense (attention output) projection also row-parallel + per-chunk
  ReduceScatter: each core contracts its own 4 heads' ctx (straight from
  SBUF, no ctx AllGather / DRAM bounce) into a [H, TC] partial, RS'd
  down to the core's 256 resident features.
- Phase D (LN3 -> AR2 -> LN2 -> x2 AllGather) for token-half 0 is
  emitted *inside* the attention loop (split into LN3-part and
  LN2-part) so the x2 AG completes while attention for batch 3 is
  still on the tensor engine; half 1 is emitted right after the first
  h4h chunk of the MLP.
- Softmax mask-multiplies moved to GpSimd so early-emitted phase-D
  vector work cannot stall the attention pipeline.
- LN4 stat AllReduces quartered; LN4 applies interleaved into the MLP
  loop to shrink the tail.
"""

import os
import sys

sys.path.insert(0, "/opt/trn_rl_repo")
os.environ.setdefault("MYCRO_LOCAL_CACHE", "1")
os.environ.setdefault("JAX_PLATFORMS", "cpu,axon")

import numpy as np
import ml_dtypes

import concourse.bass as bass
import concourse.mybir as mybir
import concourse.tile as tile
from concourse import bacc
from concourse.bass_utils import run_bass_kernel_spmd

F32 = mybir.dt.float32
BF16 = mybir.dt.bfloat16
AF = mybir.ActivationFunctionType
ALU = mybir.AluOpType

P = 128
B, S, H, NH = 4, 1024, 2048, 32
HD = H // NH
T = B * S
NC = 8
HPC = NH // NC                 # 4 heads/core
DC = H // NC                   # 256
FC = 4 * H // NC               # 1024
F4 = 4 * H                     # 8192
TC = 512
NTC = T // TC                  # 8
NFC = H // P                   # 16
EPS = 1e-5
RG = [list(range(NC))]

bf16 = ml_dtypes.bfloat16


def _causal_block_status(mask2d):
    mt = mask2d.T
    status = {}
    patterns = {}   # fingerprint -> unique slot
    slot_of = {}    # (kt, qc) -> (unique slot, src block)
    for kt in range(S // P):
        for qc in range(S // TC):
            blk = mt[kt * P:(kt + 1) * P, qc * TC:(qc + 1) * TC]
            if np.all(blk == 0):
                status[(kt, qc)] = "skip"
            elif np.all(blk == 1):
                status[(kt, qc)] = "full"
            else:
                status[(kt, qc)] = "masked"
                fp = blk.astype(np.float32).tobytes()
                if fp not in patterns:
                    patterns[fp] = (len(patterns), (kt, qc))
                slot_of[(kt, qc)] = patterns[fp][0]
    uniq = [src for _, src in sorted(patterns.values())]
    return status, slot_of, uniq


def build_program(blockinfo, zero_bv=True, zero_bias=True):
    block_status, mask_slot, mask_uniq = blockinfo
    nc = bacc.Bacc("TRN2", target_bir_lowering=False, debug=False,
                   num_devices=NC)

    def register_const_ap(dtype, value):
        t = nc.alloc_sbuf_tensor(f"const-{dtype.name}-{value}", [128, 1], dtype)
        nc.gpsimd.memset(t.ap(), value)
        nc.const_aps.aps[(dtype, value)] = t.ap()

    register_const_ap(F32, EPS)
    register_const_ap(F32, float(1.0 / np.sqrt(HD)))
    nc.all_engine_barrier()

    # ---------------- DRAM I/O ----------------
    h_ln1 = nc.dram_tensor("h_ln1", [H, TC], F32, kind="ExternalInput")
    h_res = nc.dram_tensor("h_res", [DC, T], F32, kind="ExternalInput")
    ln1_w = nc.dram_tensor("ln1_w", [H, 1], F32, kind="ExternalInput")
    ln1_b = nc.dram_tensor("ln1_b", [H, 1], F32, kind="ExternalInput")
    ln2_w = nc.dram_tensor("ln2_w", [DC, 1], F32, kind="ExternalInput")
    ln2_b = nc.dram_tensor("ln2_b", [DC, 1], F32, kind="ExternalInput")
    ln3_w = nc.dram_tensor("ln3_w", [DC, 1], F32, kind="ExternalInput")
    ln3_b = nc.dram_tensor("ln3_b", [DC, 1], F32, kind="ExternalInput")
    ln4_w = nc.dram_tensor("ln4_w", [DC, 1], F32, kind="ExternalInput")
    ln4_b = nc.dram_tensor("ln4_b", [DC, 1], F32, kind="ExternalInput")
    w_qkv = nc.dram_tensor("w_qkv", [H, 3 * DC], BF16, kind="ExternalInput")
    b_qk = nc.dram_tensor("b_qk", [2 * DC, 1], F32, kind="ExternalInput")
    b_v = nc.dram_tensor("b_v", [1, DC], F32, kind="ExternalInput")
    w_dense = nc.dram_tensor("w_dense", [DC, H], BF16, kind="ExternalInput")
    b_dense = nc.dram_tensor("b_dense", [DC, 1], F32, kind="ExternalInput")
    w_h4h = nc.dram_tensor("w_h4h", [H, FC], BF16, kind="ExternalInput")
    b_h4h = nc.dram_tensor("b_h4h", [FC, 1], F32, kind="ExternalInput")
    w_4hh = nc.dram_tensor("w_4hh", [FC, H], BF16, kind="ExternalInput")
    b_4hh = nc.dram_tensor("b_4hh", [DC, 1], F32, kind="ExternalInput")
    maskT = nc.dram_tensor("maskT", [S, S], BF16, kind="ExternalInput")
    out_ext = nc.dram_tensor("out", [DC, T], F32, kind="ExternalOutput")


    with tile.TileContext(nc) as tc:
        with tc.tile_pool(name="const", bufs=1) as const, \
             tc.tile_pool(name="resid", bufs=1) as resid, \
             tc.tile_pool(name="dram", bufs=1, space="DRAM") as dram:

            # ---------- constants ----------
            ones_f = const.tile([P, 1], F32)
            nc.vector.memset(ones_f[:, :], 1.0)
            ones_bf = const.tile([P, 1], BF16)
            nc.vector.memset(ones_bf[:, :], 1.0)
            ones_rows_bf = const.tile([P, P], BF16)
            nc.vector.memset(ones_rows_bf[:, :], 1.0)

            ln1w_sb = const.tile([P, NFC], F32)
            ln1b_sb = const.tile([P, NFC], F32)
            for fc in range(NFC):
                nc.sync.dma_start(out=ln1w_sb[:, fc:fc + 1],
                                  in_=ln1_w[fc * P:(fc + 1) * P, 0:1])
                nc.sync.dma_start(out=ln1b_sb[:, fc:fc + 1],
                                  in_=ln1_b[fc * P:(fc + 1) * P, 0:1])

            cpack = const.tile([P, 28], F32)
            _cofs = [0]

            def load_cols(t, ncols=2):
                base = _cofs[0]
                _cofs[0] += ncols
                for m in range(ncols):
                    nc.sync.dma_start(out=cpack[:, base + m:base + m + 1],
                                      in_=t[m * P:(m + 1) * P, 0:1])
                return cpack[:, base:base + ncols]

            ln2w_sb = load_cols(ln2_w)
            ln2b_sb = load_cols(ln2_b)
            ln3w_sb = load_cols(ln3_w)
            ln3b_sb = load_cols(ln3_b)
            ln4w_sb = load_cols(ln4_w)
            ln4b_sb = load_cols(ln4_b)
            bdense_sb = load_cols(b_dense)
            b4hh_sb = load_cols(b_4hh)
            bqk_sb = load_cols(b_qk, 4)
            bh4h_sb = load_cols(b_h4h, 8)

            if not zero_bv:
                bv_row = const.tile([1, DC], F32)
                nc.sync.dma_start(out=bv_row[:, :], in_=b_v[0:1, :])
                bv_b = const.tile([P, DC], F32)
                nc.gpsimd.partition_broadcast(bv_b[:, :], bv_row[:, :])

            if mask_uniq:
                mask_sb = const.tile([P, len(mask_uniq) * TC], BF16)
                for i, (kt, qc) in enumerate(mask_uniq):
                    nc.sync.dma_start(
                        out=mask_sb[:, i * TC:(i + 1) * TC],
                        in_=maskT[kt * P:(kt + 1) * P, qc * TC:(qc + 1) * TC])

            # ---------- residents ----------
            ln_in = [resid.tile([P, T], BF16, name=f"ln_in{m}")
                     for m in range(2)]
            attn_sb = [resid.tile([P, T], BF16, tag="colsA", bufs=2,
                                  name=f"attn_sb{m}") for m in range(2)]

            # ---------- DRAM bounces ----------
            ag_x1_in = [dram.tile([P, 4 * TC], BF16, name=f"agx1i{h}")
                        for h in range(4)]
            ag_x1_out = [dram.tile([NC * P, 4 * TC], BF16,
                                   addr_space="Shared", name=f"agx1o{h}")
                         for h in range(4)]
            # dense partials: per-t8 ReduceScatter bounces
            rs_d_in = [dram.tile([H, TC], BF16, name=f"rsdi{k}")
                       for k in range(NTC)]
            rs_d_out = [dram.tile([DC, TC], BF16, name=f"rsdo{k}")
                        for k in range(NTC)]
            # x2: 4 quarters, free = t8r*1024 + m*512
            ag_x2_in = [dram.tile([P, 2 * 1024], BF16, name=f"agx2i{h}")
                        for h in range(4)]
            ag_x2_out = [dram.tile([NC * P, 2 * 1024], BF16,
                                   addr_space="Shared", name=f"agx2o{h}")
                         for h in range(4)]
            # mlp 4hh partials: per-t8 ReduceScatter bounces
            rs_mlp_in = [dram.tile([H, TC], BF16, name=f"rsmi{k}")
                         for k in range(NTC)]
            rs_mlp_out = [dram.tile([DC, TC], BF16, name=f"rsmo{k}")
                          for k in range(NTC)]
            ar3q_in = [dram.tile([2, 2 * TC], F32, name=f"ar3i{i}")
                       for i in range(4)]
            ar3q_out = [dram.tile([2, 2 * TC], F32, addr_space="Shared",
                                  name=f"ar3o{i}") for i in range(4)]
            ar2q_in = [dram.tile([2, 2 * TC], F32, name=f"ar2i{i}")
                       for i in range(4)]
            ar2q_out = [dram.tile([2, 2 * TC], F32, addr_space="Shared",
                                  name=f"ar2o{i}") for i in range(4)]
            ar4q_in = [dram.tile([2, 2 * TC], F32, name=f"ar4i{i}")
                       for i in range(4)]
            ar4q_out = [dram.tile([2, 2 * TC], F32, addr_space="Shared",
                                  name=f"ar4o{i}") for i in range(4)]

            warm_in = dram.tile([1, 64], BF16, name="warm_in")
            warm_out = dram.tile([NC, 64], BF16, addr_space="Shared",
                                 name="warm_out")
            warm_sb = const.tile([1, 64], BF16)
            nc.vector.memset(warm_sb[:, :], 0.0)
            nc.sync.dma_start(out=warm_in[:, :], in_=warm_sb[:, :])
            nc.gpsimd.collective_compute(
                "AllGather", ALU.bypass, replica_groups=RG,
                ins=[warm_in[:, :].opt()], outs=[warm_out[:, :].opt()])

            # =========================================================
            # Phase A: LN1 -> x1 (bf16) -> 2 half AllGathers
            # =========================================================
            with tc.tile_pool(name="ph_a", bufs=1) as pha, \
                 tc.tile_pool(name="ph_a_ps", bufs=2, space="PSUM") as phaps:
                h1 = [pha.tile([P, TC], F32, name=f"h1_{fc}")
                      for fc in range(NFC)]
                for fc in range(NFC):
                    nc.sync.dma_start(out=h1[fc][:, :],
                                      in_=h_ln1[fc * P:(fc + 1) * P, :])
                ps_s = phaps.tile([1, TC], F32, name="ps_s")
                ps_q = phaps.tile([1, TC], F32, name="ps_q")
                for fc in range(NFC):
                    h1b = pha.tile([P, TC], BF16, tag="h1b", bufs=3,
                                   name="h1b")
                    nc.vector.tensor_copy(h1b[:, :], h1[fc][:, :])
                    nc.tensor.matmul(ps_s[:, :], ones_bf[:, 0:1], h1b[:, :],
                                     start=(fc == 0), stop=(fc == NFC - 1))
                    sq = pha.tile([P, TC], BF16, tag="sq", bufs=3, name="sq")
                    nc.vector.tensor_mul(sq[:, :], h1b[:, :], h1b[:, :])
                    nc.tensor.matmul(ps_q[:, :], ones_bf[:, 0:1], sq[:, :],
                                     start=(fc == 0), stop=(fc == NFC - 1))
                mu = pha.tile([1, TC], F32)
                m2 = pha.tile([1, TC], F32)
                var = pha.tile([1, TC], F32)
                sd = pha.tile([1, TC], F32)
                a_row = pha.tile([1, TC], F32)
                b2_row = pha.tile([1, TC], F32)
                nc.vector.tensor_scalar_mul(mu[:, :], ps_s[:, :], 1.0 / H)
                nc.vector.tensor_scalar_mul(m2[:, :], ps_q[:, :], 1.0 / H)
                nc.vector.tensor_mul(var[:, :], mu[:, :], mu[:, :])
                nc.vector.tensor_sub(var[:, :], m2[:, :], var[:, :])
                nc.scalar.activation(sd[:, :], var[:, :], AF.Sqrt, bias=EPS)
                nc.vector.reciprocal(a_row[:, :], sd[:, :])
                nc.vector.tensor_mul(b2_row[:, :], mu[:, :], a_row[:, :])
                nc.vector.tensor_scalar_mul(b2_row[:, :], b2_row[:, :], -1.0)
                a_b = pha.tile([P, TC], F32)
                b2_b = pha.tile([P, TC], F32)
                nc.gpsimd.partition_broadcast(a_b[:, :], a_row[:, :])
                nc.gpsimd.partition_broadcast(b2_b[:, :], b2_row[:, :])
                x1h = [pha.tile([P, 4 * TC], BF16, name=f"x1h{h}")
                       for h in range(4)]
                for fc in range(NFC):
                    t1 = pha.tile([P, TC], F32, tag="t1", bufs=3, name="t1")
                    nc.vector.tensor_mul(t1[:, :], h1[fc][:, :], a_b[:, :])
                    nc.vector.tensor_add(t1[:, :], t1[:, :], b2_b[:, :])
                    hh, fr = fc // 4, fc % 4
                    nc.vector.tensor_scalar(
                        x1h[hh][:, fr * TC:(fr + 1) * TC], t1[:, :],
                        ln1w_sb[:, fc:fc + 1], ln1b_sb[:, fc:fc + 1],
                        ALU.mult, ALU.add)
                    if fr == 3:
                        nc.sync.dma_start(out=ag_x1_in[hh][:, :],
                                          in_=x1h[hh][:, :])
                        nc.gpsimd.collective_compute(
                            "AllGather", ALU.bypass, replica_groups=RG,
                            ins=[ag_x1_in[hh][:, :].opt()],
                            outs=[ag_x1_out[hh][:, :].opt()])

            # =========================================================
            # Phase B: QKV (consumes x1 halves as they arrive)
            # =========================================================
            attn_res_cm = tc.tile_pool(name="attn_res", bufs=1)
            attn_res = attn_res_cm.__enter__()
            qT2 = attn_res.tile([P, 2 * T], BF16)
            kT2 = attn_res.tile([P, 2 * T], BF16)
            v_sb = attn_res.tile([P, (T // P) * DC], BF16)
            with tc.tile_pool(name="ph_b_w", bufs=1) as phbw, \
                 tc.tile_pool(name="ph_b", bufs=2) as phb, \
                 tc.tile_pool(name="ph_b_ps", bufs=3, space="PSUM") as phbps:
                wq_all = phbw.tile([P, NFC * 3 * DC], BF16, name="wq_all")
                for fc in range(NFC):
                    nc.sync.dma_start(
                        out=wq_all[:, fc * 3 * DC:(fc + 1) * 3 * DC],
                        in_=w_qkv[fc * P:(fc + 1) * P, :])
                for t8 in range(NTC):
                    x1c = [phb.tile([P, 4 * TC], BF16, tag=f"x1c{q}",
                                    name=f"x1c{q}") for q in range(4)]
                    for q in range(4):
                        nc.sync.dma_start(
                            out=x1c[q][:, :],
                            in_=ag_x1_out[q][t8 * P:(t8 + 1) * P, :])
                    qk_ps = [phbps.tile([P, 2 * TC], F32, tag=f"qkp{i}",
                                        bufs=1, name=f"qk_ps{i}")
                             for i in range(2)]
                    v_ps = [phbps.tile([P, DC], F32, tag=f"vps{i}", bufs=1,
                                       name=f"v_ps{i}") for i in range(4)]
                    for qt in range(4):
                        for m in range(4):
                            for f in range(4):
                                fc = qt * 4 + f
                                nc.tensor.matmul(
                                    qk_ps[m // 2][:, (m % 2) * TC:
                                                  (m % 2 + 1) * TC],
                                    wq_all[:, fc * 3 * DC + m * P:
                                           fc * 3 * DC + (m + 1) * P],
                                    x1c[qt][:, f * TC:(f + 1) * TC],
                                    start=(fc == 0), stop=(fc == NFC - 1))
                        for tt in range(TC // P):
                            for f in range(4):
                                fc = qt * 4 + f
                                nc.tensor.matmul(
                                    v_ps[tt][:, :],
                                    x1c[qt][:, f * TC + tt * P:
                                            f * TC + (tt + 1) * P],
                                    wq_all[:, fc * 3 * DC + 2 * DC:
                                           fc * 3 * DC + 3 * DC],
                                    start=(fc == 0), stop=(fc == NFC - 1))
                    for m in range(4):
                        dst = qT2 if m < 2 else kT2
                        pair = m % 2
                        off = pair * T + t8 * TC
                        src_ap = qk_ps[m // 2][:, pair * TC:(pair + 1) * TC]
                        if zero_bias:
                            nc.scalar.activation(dst[:, off:off + TC],
                                                 src_ap, AF.Copy)
                        else:
                            nc.scalar.activation(dst[:, off:off + TC],
                                                 src_ap, AF.Identity,
                                                 bias=bqk_sb[:, m:m + 1])
                    voff = t8 * 4 * DC
                    for tt in range(TC // P):
                        if zero_bv:
                            nc.scalar.activation(
                                v_sb[:, voff + tt * DC:voff + (tt + 1) * DC],
                                v_ps[tt][:, :], AF.Copy)
                        else:
                            nc.vector.tensor_add(
                                v_sb[:, voff + tt * DC:voff + (tt + 1) * DC],
                                v_ps[tt][:, :], bv_b[:, :])

            # =========================================================
            # Phase C: attention + row-parallel dense partials + early
            # phase-D (LN3/AR2/LN2/x2-AG) for token-half 0.
            # PSUM: s(3) + ctx(2) + den(1) + dn(2) = 8 banks.
            # =========================================================
            ctx_cm = tc.tile_pool(name="ctx_pool", bufs=1)
            ctx_pool = ctx_cm.__enter__()
            ctxF = {}
            phdw_cm = tc.tile_pool(name="ph_d_w", bufs=1)
            phdw = phdw_cm.__enter__()
            wd_all = phdw.tile([P, 2 * H], BF16, name="wd_all")
            for p2 in range(2):
                nc.sync.dma_start(out=wd_all[:, p2 * H:(p2 + 1) * H],
                                  in_=w_dense[p2 * P:(p2 + 1) * P, :])
            # MLP h4h weights: load early (SBUF region free after QKV)
            whp_cm = tc.tile_pool(name="ph_wh", bufs=1, side="right")
            whp = whp_cm.__enter__()
            wh_all = whp.tile([P, NFC * FC], BF16, name="wh_all")
            # long-lived pool for dense evicts + phase-D/LN4 row work
            phD_cm = tc.tile_pool(name="ph_D", bufs=1, side="right")
            phD = phD_cm.__enter__()

            def dense_partial(t8, pspool):
                b_, qc_ = t8 // 2, t8 % 2
                for jj in range(NFC):
                    ps = pspool.tile([P, TC], F32, tag="dn", bufs=2,
                                     name="ps_dn")
                    for p2 in range(2):
                        nc.tensor.matmul(
                            ps[:, :],
                            wd_all[:, p2 * H + jj * P:
                                   p2 * H + (jj + 1) * P],
                            ctxF[b_][:, qc_ * 2 * TC + p2 * TC:
                                     qc_ * 2 * TC + (p2 + 1) * TC],
                            start=(p2 == 0), stop=(p2 == 1))
                    pt = phD.tile([P, TC], BF16, tag="dpt", bufs=3,
                                  name="dpt")
                    nc.scalar.activation(pt[:, :], ps[:, :], AF.Copy)
                    nc.sync.dma_start(
                        out=rs_d_in[t8][jj * P:(jj + 1) * P, :],
                        in_=pt[:, :])
                nc.gpsimd.collective_compute(
                    "ReduceScatter", ALU.add, replica_groups=RG,
                    ins=[rs_d_in[t8][:, :].opt()],
                    outs=[rs_d_out[t8][:, :].opt()])

            def dense_consume(t8, pspool, stag, qtag, sbufs):
                sl = slice(t8 * TC, (t8 + 1) * TC)
                for m in range(2):
                    if zero_bias:
                        nc.sync.dma_start(
                            out=attn_sb[m][:, sl],
                            in_=rs_d_out[t8][m * P:(m + 1) * P, :])
                    else:
                        tmp = phD.tile([P, TC], BF16, tag="rsb", bufs=2,
                                       name="rsb")
                        nc.sync.dma_start(
                            out=tmp[:, :],
                            in_=rs_d_out[t8][m * P:(m + 1) * P, :])
                        nc.vector.tensor_scalar(
                            attn_sb[m][:, sl], tmp[:, :],
                            bdense_sb[:, m:m + 1], 0.0, ALU.add, ALU.add)
                _stats_t8(nc, phD, pspool, attn_sb, t8,
                          ar3q_in[t8 // 2], ones_bf, stag=stag, qtag=qtag,
                          sbufs=sbufs, slot=t8 % 2)
                if t8 % 2 == 1:
                    p = t8 // 2
                    nc.gpsimd.collective_compute(
                        "AllReduce", ALU.add, replica_groups=RG,
                        ins=[ar3q_in[p][:, :].opt()],
                        outs=[ar3q_out[p][:, :].opt()])

            def ln3_pair(p, pspool, stag, qtag, sbufs):
                ab3 = _ln_rows_batch(nc, phD, ar3q_out[p], f"ln3p{p}",
                                     nrows=2)
                for t8 in range(2 * p, 2 * p + 2):
                    a_b, b2_b = _ln_bcast(nc, phD, ab3, t8 % 2)
                    for m in range(2):
                        hres = phD.tile([P, TC], F32, tag="hres",
                                        bufs=2, name="hres")
                        nc.sync.dma_start(
                            out=hres[:, :],
                            in_=h_res[m * P:(m + 1) * P,
                                      t8 * TC:(t8 + 1) * TC])
                        sl = slice(t8 * TC, (t8 + 1) * TC)
                        t1 = phD.tile([P, TC], F32, tag="t1", name="t1")
                        t2 = phD.tile([P, TC], F32, tag="t2", name="t2")
                        nc.vector.tensor_mul(t1[:, :], attn_sb[m][:, sl],
                                             a_b[:, :])
                        nc.vector.tensor_add(t1[:, :], t1[:, :], b2_b[:, :])
                        nc.vector.tensor_scalar(t2[:, :], t1[:, :],
                                                ln3w_sb[:, m:m + 1],
                                                ln3b_sb[:, m:m + 1],
                                                ALU.mult, ALU.add)
                        nc.vector.tensor_add(ln_in[m][:, sl], t2[:, :],
                                             hres[:, :])
                    _stats_t8(nc, phD, pspool, ln_in, t8, ar2q_in[p],
                              ones_bf, stag=stag, qtag=qtag, sbufs=sbufs,
                              slot=t8 % 2)
                nc.gpsimd.collective_compute(
                    "AllReduce", ALU.add, replica_groups=RG,
                    ins=[ar2q_in[p][:, :].opt()],
                    outs=[ar2q_out[p][:, :].opt()])

            def ln2_pair(p):
                ab2 = _ln_rows_batch(nc, phD, ar2q_out[p], f"ln2p{p}",
                                     nrows=2)
                for t8 in range(2 * p, 2 * p + 2):
                    a_b, b2_b = _ln_bcast(nc, phD, ab2, t8 % 2)
                    tq = t8 % 2
                    for m in range(2):
                        sl = slice(t8 * TC, (t8 + 1) * TC)
                        t1 = phD.tile([P, TC], F32, tag="t1", name="t1")
                        nc.vector.tensor_mul(t1[:, :], ln_in[m][:, sl],
                                             a_b[:, :])
                        nc.vector.tensor_add(t1[:, :], t1[:, :], b2_b[:, :])
                        x2q = phD.tile([P, TC], BF16, tag="x2q", bufs=2,
                                       name="x2q")
                        nc.vector.tensor_scalar(
                            x2q[:, :], t1[:, :], ln2w_sb[:, m:m + 1],
                            ln2b_sb[:, m:m + 1], ALU.mult, ALU.add)
                        nc.sync.dma_start(
                            out=ag_x2_in[p][:, tq * 1024 + m * TC:
                                            tq * 1024 + (m + 1) * TC],
                            in_=x2q[:, :])
                nc.gpsimd.collective_compute(
                    "AllGather", ALU.bypass, replica_groups=RG,
                    ins=[ag_x2_in[p][:, :].opt()],
                    outs=[ag_x2_out[p][:, :].opt()])

            with tc.tile_pool(name="ph_c", bufs=1) as phc, \
                 tc.tile_pool(name="ph_c_ps", bufs=1, space="PSUM") as phcps:
                for b in range(B):
                    ctxF[b] = ctx_pool.tile([P, 2 * S], BF16, tag="ctxF",
                                            bufs=2, name=f"ctxF{b}")
                    if b == 1:
                        for fc in range(NFC):
                            nc.sync.dma_start(
                                out=wh_all[:, fc * FC:(fc + 1) * FC],
                                in_=w_h4h[fc * P:(fc + 1) * P, :])
                    for qc in range(S // TC):
                        t8c = 2 * b + qc
                        ctx_ps = [phcps.tile([P, TC], F32, tag=f"ctx{p}",
                                             bufs=1, name=f"ctx_ps{p}")
                                  for p in range(2)]
                        den_ps = phcps.tile([P, TC], F32, tag="den",
                                            bufs=1, name="den_ps")
                        kts = [kt for kt in range(S // P)
                               if block_status[(kt, qc)] != "skip"]
                        nkt = len(kts)

                        def emit_scores(ki):
                            kt = kts[ki]
                            st = block_status[(kt, qc)]
                            es = []
                            for h in range(HPC):
                                pair, rho = h // 2, h % 2
                                ps_s = phcps.tile([P, TC], F32, tag="s",
                                                  bufs=3, name="ps_s")
                                qoff = pair * T + b * S + qc * TC
                                koff = pair * T + b * S + kts[ki] * P
                                nc.tensor.matmul(
                                    ps_s[:, :],
                                    kT2[rho * HD:(rho + 1) * HD,
                                        koff:koff + P],
                                    qT2[rho * HD:(rho + 1) * HD,
                                        qoff:qoff + TC],
                                    start=True, stop=True)
                                e = phc.tile([P, TC], BF16, tag="e", bufs=6,
                                             name="e")
                                nc.scalar.activation(e[:, :], ps_s[:, :],
                                                     AF.Exp,
                                                     scale=1.0 / np.sqrt(HD))
                                if st == "masked":
                                    i = mask_slot[(kt, qc)]
                                    nc.vector.tensor_mul(
                                        e[:, :], e[:, :],
                                        mask_sb[:, i * TC:(i + 1) * TC])
                                es.append(e)
                            return es

                        def emit_ctx(ki, es):
                            kt = kts[ki]
                            ttg = b * (S // P) + kt
                            for h in range(HPC):
                                pair, rho = h // 2, h % 2
                                nc.tensor.matmul(
                                    ctx_ps[pair][rho * HD:(rho + 1) * HD, :],
                                    v_sb[:, ttg * DC + h * HD:
                                         ttg * DC + (h + 1) * HD],
                                    es[h][:, :],
                                    start=(ki == 0), stop=(ki == nkt - 1))
                                nc.tensor.matmul(
                                    den_ps[32 * h:32 * h + 1, :],
                                    ones_bf[:, 0:1], es[h][:, :],
                                    start=(ki == 0), stop=(ki == nkt - 1),
                                    tile_position=(0, 32 * h))

                        prev = emit_scores(0)
                        for ki in range(1, nkt):
                            cur = emit_scores(ki)
                            emit_ctx(ki - 1, prev)
                            prev = cur
                        emit_ctx(nkt - 1, prev)
                        # batched reciprocal of all 4 head denominators
                        rd = phc.tile([P, TC], F32, tag="rd", bufs=1,
                                      name="rd")
                        rd_bf = phc.tile([P, TC], BF16, tag="rd_bf", bufs=1,
                                         name="rd_bf")
                        nc.vector.reciprocal(rd[:, :], den_ps[:, :])
                        nc.vector.tensor_copy(rd_bf[:, :], rd[:, :])
                        for h in range(HPC):
                            pair, rho = h // 2, h % 2
                            r32 = slice(32 * h, 32 * h + 1)
                            ps_b = phcps.tile([P, TC], F32, tag="s", bufs=3,
                                              name="ps_b")
                            nc.tensor.matmul(ps_b[:, :],
                                             ones_rows_bf[r32, :],
                                             rd_bf[r32, :], start=True,
                                             stop=True,
                                             tile_position=(32 * h, 0))
                            rd_b = phc.tile([P, TC], F32, tag="rd_b", bufs=1,
                                            name="rd_b")
                            nc.vector.tensor_copy(rd_b[:, :], ps_b[:, :])
                            off = qc * 2 * TC + pair * TC
                            hs = slice(rho * HD, (rho + 1) * HD)
                            nc.vector.tensor_mul(ctxF[b][hs, off:off + TC],
                                                 ctx_ps[pair][hs, :],
                                                 rd_b[hs, :])
                        if t8c == 6:
                            ln2_pair(0)
                        dense_partial(t8c, phcps)
                        if t8c >= 2:
                            dense_consume(t8c - 2, phcps, "s", "s", 3)
                        if t8c == 5:
                            ln3_pair(0, phcps, "s", "s", 3)
                        if t8c == 7:
                            ln3_pair(1, phcps, "s", "s", 3)
                dense_consume(6, phcps, "s", "s", 3)
            phdw_cm.__exit__(None, None, None)
            ctx_cm.__exit__(None, None, None)
            attn_res_cm.__exit__(None, None, None)

            # =========================================================
            # Phase E+F: h4h+gelu -> 4hh row-parallel partials -> per-t8
            # ReduceScatter; LN4 applies interleaved.
            # PSUM: h(2) + f(2) + st(2) = 6 banks.
            # =========================================================
            with tc.tile_pool(name="ph_e_w", bufs=1) as phew, \
                 tc.tile_pool(name="ph_ef", bufs=1) as phef, \
                 tc.tile_pool(name="ph_ef_ps", bufs=1, space="PSUM") as pheps:
                w4_all = phew.tile([P, (FC // P) * H], BF16, name="w4_all")
                for j in range(FC // P):
                    nc.sync.dma_start(out=w4_all[:, j * H:(j + 1) * H],
                                      in_=w_4hh[j * P:(j + 1) * P, :])
                mlp_sb = [resid.tile([P, T], BF16, tag="colsM", bufs=2,
                                     name=f"mlp_sb{m}") for m in range(2)]

                inter_t = {}

                def h4h_chunk(t8):
                    hh, t8r = t8 // 2, t8 % 2
                    x2c_all = phef.tile([P, NFC * TC], BF16, tag="x2c",
                                        bufs=2, name="x2c_all")
                    for c8 in range(NC):
                        for m2 in range(2):
                            fc = c8 * 2 + m2
                            nc.sync.dma_start(
                                out=x2c_all[:, fc * TC:(fc + 1) * TC],
                                in_=ag_x2_out[hh][c8 * P:(c8 + 1) * P,
                                                  t8r * 1024 + m2 * TC:
                                                  t8r * 1024 + (m2 + 1) * TC])
                    inter = phef.tile([P, (FC // P) * TC], BF16, tag="inter",
                                      bufs=2, name="inter")
                    for g in range(4):
                        ps = [pheps.tile([P, TC], F32, tag=f"h{mi}", bufs=1,
                                         name=f"ps_h{mi}") for mi in range(2)]
                        for fc in range(NFC):
                            for mi in range(2):
                                m = g * 2 + mi
                                nc.tensor.matmul(
                                    ps[mi][:, :],
                                    wh_all[:, fc * FC + m * P:
                                           fc * FC + (m + 1) * P],
                                    x2c_all[:, fc * TC:(fc + 1) * TC],
                                    start=(fc == 0), stop=(fc == NFC - 1))
                        for mi in range(2):
                            m = g * 2 + mi
                            nc.scalar.activation(
                                inter[:, m * TC:(m + 1) * TC], ps[mi][:, :],
                                AF.Gelu_apprx_tanh,
                                bias=bh4h_sb[:, m:m + 1])
                    inter_t[t8] = inter

                def fourhh_partial(t8):
                    inter = inter_t.pop(t8)
                    for jj in range(NFC):
                        ps = pheps.tile([P, TC], F32, tag="f", bufs=2,
                                        name="ps_f")
                        for j in range(FC // P):
                            nc.tensor.matmul(
                                ps[:, :],
                                w4_all[:, j * H + jj * P:
                                       j * H + (jj + 1) * P],
                                inter[:, j * TC:(j + 1) * TC],
                                start=(j == 0), stop=(j == FC // P - 1))
                        pt = phef.tile([P, TC], BF16, tag="pt", bufs=4,
                                       name="pt")
                        if jj % 2 == 0:
                            nc.vector.tensor_copy(pt[:, :], ps[:, :])
                        else:
                            nc.scalar.activation(pt[:, :], ps[:, :], AF.Copy)
                        nc.sync.dma_start(
                            out=rs_mlp_in[t8][jj * P:(jj + 1) * P, :],
                            in_=pt[:, :])
                    nc.gpsimd.collective_compute(
                        "ReduceScatter", ALU.add, replica_groups=RG,
                        ins=[rs_mlp_in[t8][:, :].opt()],
                        outs=[rs_mlp_out[t8][:, :].opt()])

                def mlp_consume(t8):
                    sl = slice(t8 * TC, (t8 + 1) * TC)
                    for m in range(2):
                        if zero_bias:
                            nc.sync.dma_start(
                                out=mlp_sb[m][:, sl],
                                in_=rs_mlp_out[t8][m * P:(m + 1) * P, :])
                        else:
                            tmp = phef.tile([P, TC], BF16, tag="rsb", bufs=2,
                                            name="rsb")
                            nc.sync.dma_start(
                                out=tmp[:, :],
                                in_=rs_mlp_out[t8][m * P:(m + 1) * P, :])
                            nc.vector.tensor_scalar(
                                mlp_sb[m][:, sl], tmp[:, :],
                                b4hh_sb[:, m:m + 1], 0.0, ALU.add, ALU.add)
                    _stats_t8(nc, phD, pheps, mlp_sb, t8,
                              ar4q_in[t8 // 2], ones_bf, slot=t8 % 2)
                    if t8 % 2 == 1:
                        qq = t8 // 2
                        nc.gpsimd.collective_compute(
                            "AllReduce", ALU.add, replica_groups=RG,
                            ins=[ar4q_in[qq][:, :].opt()],
                            outs=[ar4q_out[qq][:, :].opt()])

                def ln4_apply(qq):
                    ab4 = _ln_rows_batch(nc, phD, ar4q_out[qq],
                                         f"ln4q{qq}", nrows=2)
                    for t8 in range(2 * qq, 2 * qq + 2):
                        a_b, b2_b = _ln_bcast(nc, phD, ab4, t8 % 2)
                        for m in range(2):
                            sl = slice(t8 * TC, (t8 + 1) * TC)
                            t1 = phD.tile([P, TC], F32, tag="t1", name="t1")
                            t2 = phD.tile([P, TC], F32, tag="t2", name="t2")
                            nc.vector.tensor_mul(t1[:, :], mlp_sb[m][:, sl],
                                                 a_b[:, :])
                            nc.vector.tensor_add(t1[:, :], t1[:, :],
                                                 b2_b[:, :])
                            nc.vector.tensor_scalar(t2[:, :], t1[:, :],
                                                    ln4w_sb[:, m:m + 1],
                                                    ln4b_sb[:, m:m + 1],
                                                    ALU.mult, ALU.add)
                            ot = phD.tile([P, TC], F32, tag="ot", name="ot")
                            nc.vector.tensor_add(ot[:, :], t2[:, :],
                                                 ln_in[m][:, sl])
                            nc.sync.dma_start(
                                out=out_ext[m * P:(m + 1) * P,
                                            t8 * TC:(t8 + 1) * TC],
                                in_=ot[:, :])

                for t8 in range(NTC):
                    h4h_chunk(t8)
                    if t8 == 0:
                        ln2_pair(1)
                    if t8 == 1:
                        dense_consume(7, pheps, "st_s", "st_q", 1)
                        ln3_pair(2, pheps, "st_s", "st_q", 1)
                    if t8 == 2:
                        ln3_pair(3, pheps, "st_s", "st_q", 1)
                    if t8 == 3:
                        ln2_pair(2)
                    if t8 == 4:
                        ln2_pair(3)
                    if t8 >= 1:
                        fourhh_partial(t8 - 1)
                    if t8 >= 2:
                        mlp_consume(t8 - 2)
                    if t8 == 5:
                        ln4_apply(0)
                    if t8 == 7:
                        ln4_apply(1)
                fourhh_partial(NTC - 1)
                mlp_consume(6)
                ln4_apply(2)
                mlp_consume(7)
                ln4_apply(3)
            phD_cm.__exit__(None, None, None)
            whp_cm.__exit__(None, None, None)

    nc.compile()
    return nc


def _stats_t8(nc, pool, pspool, rows, t8, ar_in, ones_bf,
              stag="st_s", qtag="st_q", sbufs=1, slot=None):
    """Sum & sumsq over the 256 local features of token-chunk t8 (bf16)."""
    if slot is None:
        slot = t8
    ps_s = pspool.tile([1, TC], F32, tag=stag, bufs=sbufs, name="ps_s")
    ps_q = pspool.tile([1, TC], F32, tag=qtag, bufs=sbufs, name="ps_q")
    sl = slice(t8 * TC, (t8 + 1) * TC)
    osl = slice(slot * TC, (slot + 1) * TC)
    for m in range(2):
        nc.tensor.matmul(ps_s[:, :], ones_bf[:, 0:1], rows[m][:, sl],
                         start=(m == 0), stop=(m == 1))
    for m in range(2):
        sq = pool.tile([P, TC], BF16, tag="sq", bufs=2, name="sq")
        nc.vector.tensor_mul(sq[:, :], rows[m][:, sl], rows[m][:, sl])
        nc.tensor.matmul(ps_q[:, :], ones_bf[:, 0:1], sq[:, :],
                         start=(m == 0), stop=(m == 1))
    tmp_s = pool.tile([1, TC], F32, tag="tmp_s", bufs=1, name="tmp_s")
    tmp_q = pool.tile([1, TC], F32, tag="tmp_q", bufs=1, name="tmp_q")
    nc.vector.tensor_copy(tmp_s[:, :], ps_s[:, :])
    nc.vector.tensor_copy(tmp_q[:, :], ps_q[:, :])
    nc.sync.dma_start(out=ar_in[0:1, osl], in_=tmp_s[:, :])
    nc.sync.dma_start(out=ar_in[1:2, osl], in_=tmp_q[:, :])


def _ln_rows_batch(nc, pool, ar_out, name, nrows=8):
    """Batched LN row math on [nrows,TC] tiles, one reciprocal total."""
    s8 = pool.tile([nrows, TC], F32, tag="lnrb_s8", bufs=1, name=f"{name}_s8")
    q8 = pool.tile([nrows, TC], F32, tag="lnrb_q8", bufs=1, name=f"{name}_q8")
    nc.sync.dma_start(out=s8[:, :], in_=ar_out[0:1, :])
    nc.sync.dma_start(out=q8[:, :], in_=ar_out[1:2, :])
    mu = pool.tile([nrows, TC], F32, tag="lnrb_mu", bufs=1, name=f"{name}_mu")
    a8 = pool.tile([nrows, TC], F32, tag="lnrb_a8", bufs=1, name=f"{name}_a8")
    b28 = pool.tile([nrows, TC], F32, tag="lnrb_b28", bufs=1,
                    name=f"{name}_b28")
    nc.vector.tensor_scalar_mul(mu[:, :], s8[:, :], 1.0 / H)
    nc.vector.tensor_scalar_mul(q8[:, :], q8[:, :], 1.0 / H)
    nc.vector.tensor_mul(b28[:, :], mu[:, :], mu[:, :])
    nc.vector.tensor_sub(q8[:, :], q8[:, :], b28[:, :])
    nc.scalar.activation(q8[:, :], q8[:, :], AF.Sqrt, bias=EPS)
    nc.vector.reciprocal(a8[:, :], q8[:, :])
    nc.vector.tensor_mul(b28[:, :], mu[:, :], a8[:, :])
    nc.vector.tensor_scalar_mul(b28[:, :], b28[:, :], -1.0)
    return a8, b28


def _ln_bcast(nc, pool, ab, t8):
    """Extract row t8 from the batched (a8,b28) and broadcast to [P,TC]."""
    a8, b28 = ab
    a_row = pool.tile([1, TC], F32, tag="a_row", name="a_row")
    b2_row = pool.tile([1, TC], F32, tag="b2_row", name="b2_row")
    nc.sync.dma_start(out=a_row[:, :], in_=a8[t8:t8 + 1, :])
    nc.sync.dma_start(out=b2_row[:, :], in_=b28[t8:t8 + 1, :])
    a_b = pool.tile([P, TC], F32, tag="a_b", name="a_b")
    b2_b = pool.tile([P, TC], F32, tag="b2_b", name="b2_b")
    nc.gpsimd.partition_broadcast(a_b[:, :], a_row[:, :])
    nc.gpsimd.partition_broadcast(b2_b[:, :], b2_row[:, :])
    return a_b, b2_b


# ----------------------------------------------------------------------
_cache = {}


def _get_program(mask_np, zero_bv, zero_bias):
    key = (mask_np.tobytes(), zero_bv, zero_bias)
    kh = hash(key)
    if kh not in _cache:
        _cache[kh] = build_program(_causal_block_status(mask_np), zero_bv,
                                   zero_bias)
    return _cache[kh]


def kernel(hidden_states, mask, ln1_w, ln1_b, w_qkv, b_qkv, w_dense, b_dense,
           ln3_w, ln3_b, ln2_w, ln2_b, w_h4h, b_h4h, w_4hh, b_4hh,
           ln4_w, ln4_b):
    hidden_states = np.asarray(hidden_states, np.float32)
    mask2d = np.asarray(mask, np.float32).reshape(S, S)
    w_qkv = np.asarray(w_qkv, np.float32)
    b_qkv = np.asarray(b_qkv, np.float32)
    w_dense = np.asarray(w_dense, np.float32)
    w_h4h = np.asarray(w_h4h, np.float32)
    w_4hh = np.asarray(w_4hh, np.float32)

    zero_bv = bool(np.all(b_qkv[2 * H:] == 0.0))
    zero_bias = bool(np.all(b_qkv[:2 * H] == 0.0)
                     and np.all(np.asarray(b_dense) == 0.0)
                     and np.all(np.asarray(b_4hh) == 0.0))
    prog = _get_program(mask2d, zero_bv, zero_bias)

    hT = np.ascontiguousarray(hidden_states.reshape(T, H).T)
    maskT_bf = np.ascontiguousarray(mask2d.T).astype(bf16)

    in_maps = []
    for c in range(NC):
        qs = slice(c * DC, (c + 1) * DC)
        wq_c = np.concatenate([w_qkv[:, c * DC:(c + 1) * DC],
                               w_qkv[:, H + c * DC:H + (c + 1) * DC],
                               w_qkv[:, 2 * H + c * DC:2 * H + (c + 1) * DC]],
                              axis=1)
        b_qk_c = np.concatenate([b_qkv[c * DC:(c + 1) * DC],
                                 b_qkv[H + c * DC:H + (c + 1) * DC]])
        b_v_c = b_qkv[2 * H + c * DC:2 * H + (c + 1) * DC]
        im = {
            "h_ln1": np.ascontiguousarray(hT[:, c * TC:(c + 1) * TC]),
            "h_res": np.ascontiguousarray(hT[qs, :]),
            "ln1_w": np.asarray(ln1_w, np.float32).reshape(H, 1),
            "ln1_b": np.asarray(ln1_b, np.float32).reshape(H, 1),
            "ln2_w": np.asarray(ln2_w, np.float32)[qs].reshape(DC, 1),
            "ln2_b": np.asarray(ln2_b, np.float32)[qs].reshape(DC, 1),
            "ln3_w": np.asarray(ln3_w, np.float32)[qs].reshape(DC, 1),
            "ln3_b": np.asarray(ln3_b, np.float32)[qs].reshape(DC, 1),
            "ln4_w": np.asarray(ln4_w, np.float32)[qs].reshape(DC, 1),
            "ln4_b": np.asarray(ln4_b, np.float32)[qs].reshape(DC, 1),
            "w_qkv": np.ascontiguousarray(wq_c).astype(bf16),
            "b_qk": np.ascontiguousarray(b_qk_c).reshape(2 * DC, 1),
            "b_v": np.ascontiguousarray(b_v_c).reshape(1, DC),
            "w_dense": np.ascontiguousarray(w_dense[qs, :]).astype(bf16),
            "b_dense": np.asarray(b_dense, np.float32)[qs].reshape(DC, 1),
            "w_h4h": np.ascontiguousarray(
                w_h4h[:, c * FC:(c + 1) * FC]).astype(bf16),
            "b_h4h": np.asarray(b_h4h, np.float32)[
                c * FC:(c + 1) * FC].reshape(FC, 1),
            "w_4hh": np.ascontiguousarray(
                w_4hh[c * FC:(c + 1) * FC, :]).astype(bf16),
            "b_4hh": np.asarray(b_4hh, np.float32)[qs].reshape(DC, 1),
            "maskT": maskT_bf,
        }
        in_maps.append(im)

    res = run_bass_kernel_spmd(prog, in_maps, core_ids=list(range(NC)))
    outT = np.concatenate([res.results[c]["out"] for c in range(NC)], axis=0)
    return np.ascontiguousarray(outT.T).reshape(B, S, H).astype(np.float32)


# revision 22
# speedup vs baseline: 1.0176x; 1.0176x over previous
"""Trainium2 8-core tensor-parallel transformer layer — v10.

On top of v9 (MLP 4hh row-parallel + per-chunk ReduceScatter):
- Dense (attention output) projection also row-parallel + per-chunk
  ReduceScatter: each core contracts its own 4 heads' ctx (straight from
  SBUF, no ctx AllGather / DRAM bounce) into a [H, TC] partial, RS'd
  down to the core's 256 resident features.
- Phase D (LN3 -> AR2 -> LN2 -> x2 AllGather) for token-half 0 is
  emitted *inside* the attention loop (split into LN3-part and
  LN2-part) so the x2 AG completes while attention for batch 3 is
  still on the tensor engine; half 1 is emitted right after the first
  h4h chunk of the MLP.
- Softmax mask-multiplies moved to GpSimd so early-emitted phase-D
  vector work cannot stall the attention pipeline.
- LN4 stat AllReduces quartered; LN4 applies interleaved into the MLP
  loop to shrink the tail.
"""

import os
import sys

sys.path.insert(0, "/opt/trn_rl_repo")
os.environ.setdefault("MYCRO_LOCAL_CACHE", "1")
os.environ.setdefault("JAX_PLATFORMS", "cpu,axon")

import numpy as np
import ml_dtypes

import concourse.bass as bass
import concourse.mybir as mybir
import concourse.tile as tile
from concourse import bacc
from concourse.bass_utils import run_bass_kernel_spmd

F32 = mybir.dt.float32
BF16 = mybir.dt.bfloat16
AF = mybir.ActivationFunctionType
ALU = mybir.AluOpType

P = 128
B, S, H, NH = 4, 1024, 2048, 32
HD = H // NH
T = B * S
NC = 8
HPC = NH // NC                 # 4 heads/core
DC = H // NC                   # 256
FC = 4 * H // NC               # 1024
F4 = 4 * H                     # 8192
TC = 512
NTC = T // TC                  # 8
NFC = H // P                   # 16
EPS = 1e-5
RG = [list(range(NC))]

bf16 = ml_dtypes.bfloat16


def _causal_block_status(mask2d):
    mt = mask2d.T
    status = {}
    patterns = {}   # fingerprint -> unique slot
    slot_of = {}    # (kt, qc) -> (unique slot, src block)
    for kt in range(S // P):
        for qc in range(S // TC):
            blk = mt[kt * P:(kt + 1) * P, qc * TC:(qc + 1) * TC]
            if np.all(blk == 0):
                status[(kt, qc)] = "skip"
            elif np.all(blk == 1):
                status[(kt, qc)] = "full"
            else:
                status[(kt, qc)] = "masked"
                fp = blk.astype(np.float32).tobytes()
                if fp not in patterns:
                    patterns[fp] = (len(patterns), (kt, qc))
                slot_of[(kt, qc)] = patterns[fp][0]
    uniq = [src for _, src in sorted(patterns.values())]
    return status, slot_of, uniq


def build_program(blockinfo, zero_bv=True, zero_bias=True):
    block_status, mask_slot, mask_uniq = blockinfo
    nc = bacc.Bacc("TRN2", target_bir_lowering=False, debug=False,
                   num_devices=NC)

    def register_const_ap(dtype, value):
        t = nc.alloc_sbuf_tensor(f"const-{dtype.name}-{value}", [128, 1], dtype)
        nc.gpsimd.memset(t.ap(), value)
        nc.const_aps.aps[(dtype, value)] = t.ap()

    register_const_ap(F32, EPS)
    register_const_ap(F32, float(1.0 / np.sqrt(HD)))
    nc.all_engine_barrier()

    # ---------------- DRAM I/O ----------------
    h_ln1 = nc.dram_tensor("h_ln1", [H, TC], BF16, kind="ExternalInput")
    h_res = nc.dram_tensor("h_res", [DC, T], F32, kind="ExternalInput")
    ln1_w = nc.dram_tensor("ln1_w", [H, 1], F32, kind="ExternalInput")
    ln1_b = nc.dram_tensor("ln1_b", [H, 1], F32, kind="ExternalInput")
    ln2_w = nc.dram_tensor("ln2_w", [DC, 1], F32, kind="ExternalInput")
    ln2_b = nc.dram_tensor("ln2_b", [DC, 1], F32, kind="ExternalInput")
    ln3_w = nc.dram_tensor("ln3_w", [DC, 1], F32, kind="ExternalInput")
    ln3_b = nc.dram_tensor("ln3_b", [DC, 1], F32, kind="ExternalInput")
    ln4_w = nc.dram_tensor("ln4_w", [DC, 1], F32, kind="ExternalInput")
    ln4_b = nc.dram_tensor("ln4_b", [DC, 1], F32, kind="ExternalInput")
    w_qkv = nc.dram_tensor("w_qkv", [H, 3 * DC], BF16, kind="ExternalInput")
    b_qk = nc.dram_tensor("b_qk", [2 * DC, 1], F32, kind="ExternalInput")
    b_v = nc.dram_tensor("b_v", [1, DC], F32, kind="ExternalInput")
    w_dense = nc.dram_tensor("w_dense", [DC, H], BF16, kind="ExternalInput")
    b_dense = nc.dram_tensor("b_dense", [DC, 1], F32, kind="ExternalInput")
    w_h4h = nc.dram_tensor("w_h4h", [H, FC], BF16, kind="ExternalInput")
    b_h4h = nc.dram_tensor("b_h4h", [FC, 1], F32, kind="ExternalInput")
    w_4hh = nc.dram_tensor("w_4hh", [FC, H], BF16, kind="ExternalInput")
    b_4hh = nc.dram_tensor("b_4hh", [DC, 1], F32, kind="ExternalInput")
    maskT = nc.dram_tensor("maskT", [S, S], BF16, kind="ExternalInput")
    out_ext = nc.dram_tensor("out", [DC, T], F32, kind="ExternalOutput")


    with tile.TileContext(nc) as tc:
        with tc.tile_pool(name="const", bufs=1) as const, \
             tc.tile_pool(name="resid", bufs=1) as resid, \
             tc.tile_pool(name="dram", bufs=1, space="DRAM") as dram:

            # ---------- constants ----------
            ones_f = const.tile([P, 1], F32)
            nc.vector.memset(ones_f[:, :], 1.0)
            ones_bf = const.tile([P, 1], BF16)
            nc.vector.memset(ones_bf[:, :], 1.0)
            ones_rows_bf = const.tile([P, P], BF16)
            nc.vector.memset(ones_rows_bf[:, :], 1.0)

            ln1w_sb = const.tile([P, NFC], F32)
            ln1b_sb = const.tile([P, NFC], F32)
            for fc in range(NFC):
                nc.sync.dma_start(out=ln1w_sb[:, fc:fc + 1],
                                  in_=ln1_w[fc * P:(fc + 1) * P, 0:1])
                nc.sync.dma_start(out=ln1b_sb[:, fc:fc + 1],
                                  in_=ln1_b[fc * P:(fc + 1) * P, 0:1])

            cpack = const.tile([P, 28], F32)
            _cofs = [0]

            def load_cols(t, ncols=2):
                base = _cofs[0]
                _cofs[0] += ncols
                for m in range(ncols):
                    nc.sync.dma_start(out=cpack[:, base + m:base + m + 1],
                                      in_=t[m * P:(m + 1) * P, 0:1])
                return cpack[:, base:base + ncols]

            ln2w_sb = load_cols(ln2_w)
            ln2b_sb = load_cols(ln2_b)
            ln3w_sb = load_cols(ln3_w)
            ln3b_sb = load_cols(ln3_b)
            ln4w_sb = load_cols(ln4_w)
            ln4b_sb = load_cols(ln4_b)
            bdense_sb = load_cols(b_dense)
            b4hh_sb = load_cols(b_4hh)
            bqk_sb = load_cols(b_qk, 4)
            bh4h_sb = load_cols(b_h4h, 8)

            if not zero_bv:
                bv_row = const.tile([1, DC], F32)
                nc.sync.dma_start(out=bv_row[:, :], in_=b_v[0:1, :])
                bv_b = const.tile([P, DC], F32)
                nc.gpsimd.partition_broadcast(bv_b[:, :], bv_row[:, :])

            if mask_uniq:
                mask_sb = const.tile([P, len(mask_uniq) * TC], BF16)
                for i, (kt, qc) in enumerate(mask_uniq):
                    nc.sync.dma_start(
                        out=mask_sb[:, i * TC:(i + 1) * TC],
                        in_=maskT[kt * P:(kt + 1) * P, qc * TC:(qc + 1) * TC])

            # ---------- residents ----------
            ln_in = [resid.tile([P, T], BF16, name=f"ln_in{m}")
                     for m in range(2)]
            attn_sb = [resid.tile([P, T], BF16, tag="colsA", bufs=2,
                                  name=f"attn_sb{m}") for m in range(2)]

            # ---------- DRAM bounces ----------
            ag_x1_in = [dram.tile([P, 4 * TC], BF16, name=f"agx1i{h}")
                        for h in range(4)]
            ag_x1_out = [dram.tile([NC * P, 4 * TC], BF16,
                                   addr_space="Shared", name=f"agx1o{h}")
                         for h in range(4)]
            # dense partials: per-t8 ReduceScatter bounces
            rs_d_in = [dram.tile([H, TC], BF16, name=f"rsdi{k}")
                       for k in range(NTC)]
            rs_d_out = [dram.tile([DC, TC], BF16, name=f"rsdo{k}")
                        for k in range(NTC)]
            # x2: 4 quarters, free = t8r*1024 + m*512
            ag_x2_in = [dram.tile([P, 2 * 1024], BF16, name=f"agx2i{h}")
                        for h in range(4)]
            ag_x2_out = [dram.tile([NC * P, 2 * 1024], BF16,
                                   addr_space="Shared", name=f"agx2o{h}")
                         for h in range(4)]
            # mlp 4hh partials: per-t8 ReduceScatter bounces
            rs_mlp_in = [dram.tile([H, TC], BF16, name=f"rsmi{k}")
                         for k in range(NTC)]
            rs_mlp_out = [dram.tile([DC, TC], BF16, name=f"rsmo{k}")
                          for k in range(NTC)]
            rs_m7_in = [dram.tile([H // 2, TC], BF16, name=f"rsm7i{i}")
                        for i in range(2)]
            rs_m7_out = [dram.tile([P, TC], BF16, name=f"rsm7o{i}")
                         for i in range(2)]
            ar3q_in = [dram.tile([2, 2 * TC], F32, name=f"ar3i{i}")
                       for i in range(4)]
            ar3q_out = [dram.tile([2, 2 * TC], F32, addr_space="Shared",
                                  name=f"ar3o{i}") for i in range(4)]
            ar2q_in = [dram.tile([2, 2 * TC], F32, name=f"ar2i{i}")
                       for i in range(4)]
            ar2q_out = [dram.tile([2, 2 * TC], F32, addr_space="Shared",
                                  name=f"ar2o{i}") for i in range(4)]
            ar4q_in = [dram.tile([2, 2 * TC], F32, name=f"ar4i{i}")
                       for i in range(4)]
            ar4q_out = [dram.tile([2, 2 * TC], F32, addr_space="Shared",
                                  name=f"ar4o{i}") for i in range(4)]

            warm_in = dram.tile([1, 64], BF16, name="warm_in")
            warm_out = dram.tile([NC, 64], BF16, addr_space="Shared",
                                 name="warm_out")
            warm_sb = const.tile([1, 64], BF16)
            nc.vector.memset(warm_sb[:, :], 0.0)
            nc.sync.dma_start(out=warm_in[:, :], in_=warm_sb[:, :])
            nc.gpsimd.collective_compute(
                "AllGather", ALU.bypass, replica_groups=RG,
                ins=[warm_in[:, :].opt()], outs=[warm_out[:, :].opt()])

            # =========================================================
            # Phase A: LN1 -> x1 (bf16) -> 2 half AllGathers
            # =========================================================
            with tc.tile_pool(name="ph_a", bufs=1) as pha, \
                 tc.tile_pool(name="ph_a_ps", bufs=2, space="PSUM") as phaps:
                h1 = [pha.tile([P, TC], BF16, name=f"h1_{fc}")
                      for fc in range(NFC)]
                for fc in range(NFC):
                    nc.sync.dma_start(out=h1[fc][:, :],
                                      in_=h_ln1[fc * P:(fc + 1) * P, :])
                ps_s = phaps.tile([1, TC], F32, name="ps_s")
                ps_q = phaps.tile([1, TC], F32, name="ps_q")
                for fc in range(NFC):
                    nc.tensor.matmul(ps_s[:, :], ones_bf[:, 0:1],
                                     h1[fc][:, :],
                                     start=(fc == 0), stop=(fc == NFC - 1))
                    sq = pha.tile([P, TC], BF16, tag="sq", bufs=3, name="sq")
                    nc.vector.tensor_mul(sq[:, :], h1[fc][:, :],
                                         h1[fc][:, :])
                    nc.tensor.matmul(ps_q[:, :], ones_bf[:, 0:1], sq[:, :],
                                     start=(fc == 0), stop=(fc == NFC - 1))
                mu = pha.tile([1, TC], F32)
                m2 = pha.tile([1, TC], F32)
                var = pha.tile([1, TC], F32)
                sd = pha.tile([1, TC], F32)
                a_row = pha.tile([1, TC], F32)
                b2_row = pha.tile([1, TC], F32)
                nc.vector.tensor_scalar_mul(mu[:, :], ps_s[:, :], 1.0 / H)
                nc.vector.tensor_scalar_mul(m2[:, :], ps_q[:, :], 1.0 / H)
                nc.vector.tensor_mul(var[:, :], mu[:, :], mu[:, :])
                nc.vector.tensor_sub(var[:, :], m2[:, :], var[:, :])
                nc.scalar.activation(sd[:, :], var[:, :], AF.Sqrt, bias=EPS)
                nc.vector.reciprocal(a_row[:, :], sd[:, :])
                nc.vector.tensor_mul(b2_row[:, :], mu[:, :], a_row[:, :])
                nc.vector.tensor_scalar_mul(b2_row[:, :], b2_row[:, :], -1.0)
                a_b = pha.tile([P, TC], F32)
                b2_b = pha.tile([P, TC], F32)
                nc.gpsimd.partition_broadcast(a_b[:, :], a_row[:, :])
                nc.gpsimd.partition_broadcast(b2_b[:, :], b2_row[:, :])
                x1h = [pha.tile([P, 4 * TC], BF16, name=f"x1h{h}")
                       for h in range(4)]
                for fc in range(NFC):
                    t1 = pha.tile([P, TC], F32, tag="t1", bufs=3, name="t1")
                    nc.vector.tensor_mul(t1[:, :], h1[fc][:, :], a_b[:, :])
                    nc.vector.tensor_add(t1[:, :], t1[:, :], b2_b[:, :])
                    hh, fr = fc // 4, fc % 4
                    nc.vector.tensor_scalar(
                        x1h[hh][:, fr * TC:(fr + 1) * TC], t1[:, :],
                        ln1w_sb[:, fc:fc + 1], ln1b_sb[:, fc:fc + 1],
                        ALU.mult, ALU.add)
                    if fr == 3:
                        nc.sync.dma_start(out=ag_x1_in[hh][:, :],
                                          in_=x1h[hh][:, :])
                        nc.gpsimd.collective_compute(
                            "AllGather", ALU.bypass, replica_groups=RG,
                            ins=[ag_x1_in[hh][:, :].opt()],
                            outs=[ag_x1_out[hh][:, :].opt()])

            # =========================================================
            # Phase B: QKV (consumes x1 halves as they arrive)
            # =========================================================
            attn_res_cm = tc.tile_pool(name="attn_res", bufs=1)
            attn_res = attn_res_cm.__enter__()
            qT2 = attn_res.tile([P, 2 * T], BF16)
            kT2 = attn_res.tile([P, 2 * T], BF16)
            v_sb = attn_res.tile([P, (T // P) * DC], BF16)
            with tc.tile_pool(name="ph_b_w", bufs=1) as phbw, \
                 tc.tile_pool(name="ph_b", bufs=2) as phb, \
                 tc.tile_pool(name="ph_b_ps", bufs=3, space="PSUM") as phbps:
                wq_all = phbw.tile([P, NFC * 3 * DC], BF16, name="wq_all")
                for fc in range(NFC):
                    nc.sync.dma_start(
                        out=wq_all[:, fc * 3 * DC:(fc + 1) * 3 * DC],
                        in_=w_qkv[fc * P:(fc + 1) * P, :])
                for t8 in range(NTC):
                    x1c = [phb.tile([P, 4 * TC], BF16, tag=f"x1c{q}",
                                    name=f"x1c{q}") for q in range(4)]
                    for q in range(4):
                        nc.sync.dma_start(
                            out=x1c[q][:, :],
                            in_=ag_x1_out[q][t8 * P:(t8 + 1) * P, :])
                    qk_ps = [phbps.tile([P, 2 * TC], F32, tag=f"qkp{i}",
                                        bufs=1, name=f"qk_ps{i}")
                             for i in range(2)]
                    v_ps = [phbps.tile([P, DC], F32, tag=f"vps{i}", bufs=1,
                                       name=f"v_ps{i}") for i in range(4)]
                    for qt in range(4):
                        for m in range(4):
                            for f in range(4):
                                fc = qt * 4 + f
                                nc.tensor.matmul(
                                    qk_ps[m // 2][:, (m % 2) * TC:
                                                  (m % 2 + 1) * TC],
                                    wq_all[:, fc * 3 * DC + m * P:
                                           fc * 3 * DC + (m + 1) * P],
                                    x1c[qt][:, f * TC:(f + 1) * TC],
                                    start=(fc == 0), stop=(fc == NFC - 1))
                        for tt in range(TC // P):
                            for f in range(4):
                                fc = qt * 4 + f
                                nc.tensor.matmul(
                                    v_ps[tt][:, :],
                                    x1c[qt][:, f * TC + tt * P:
                                            f * TC + (tt + 1) * P],
                                    wq_all[:, fc * 3 * DC + 2 * DC:
                                           fc * 3 * DC + 3 * DC],
                                    start=(fc == 0), stop=(fc == NFC - 1))
                    for m in range(4):
                        dst = qT2 if m < 2 else kT2
                        pair = m % 2
                        off = pair * T + t8 * TC
                        src_ap = qk_ps[m // 2][:, pair * TC:(pair + 1) * TC]
                        if zero_bias:
                            nc.scalar.activation(dst[:, off:off + TC],
                                                 src_ap, AF.Copy)
                        else:
                            nc.scalar.activation(dst[:, off:off + TC],
                                                 src_ap, AF.Identity,
                                                 bias=bqk_sb[:, m:m + 1])
                    voff = t8 * 4 * DC
                    for tt in range(TC // P):
                        if zero_bv:
                            nc.scalar.activation(
                                v_sb[:, voff + tt * DC:voff + (tt + 1) * DC],
                                v_ps[tt][:, :], AF.Copy)
                        else:
                            nc.vector.tensor_add(
                                v_sb[:, voff + tt * DC:voff + (tt + 1) * DC],
                                v_ps[tt][:, :], bv_b[:, :])

            # =========================================================
            # Phase C: attention + row-parallel dense partials + early
            # phase-D (LN3/AR2/LN2/x2-AG) for token-half 0.
            # PSUM: s(3) + ctx(2) + den(1) + dn(2) = 8 banks.
            # =========================================================
            ctx_cm = tc.tile_pool(name="ctx_pool", bufs=1)
            ctx_pool = ctx_cm.__enter__()
            ctxF = {}
            phdw_cm = tc.tile_pool(name="ph_d_w", bufs=1)
            phdw = phdw_cm.__enter__()
            wd_all = phdw.tile([P, 2 * H], BF16, name="wd_all")
            for p2 in range(2):
                nc.sync.dma_start(out=wd_all[:, p2 * H:(p2 + 1) * H],
                                  in_=w_dense[p2 * P:(p2 + 1) * P, :])
            # MLP h4h weights: load early (SBUF region free after QKV)
            whp_cm = tc.tile_pool(name="ph_wh", bufs=1, side="right")
            whp = whp_cm.__enter__()
            wh_all = whp.tile([P, NFC * FC], BF16, name="wh_all")
            # long-lived pool for dense evicts + phase-D/LN4 row work
            phD_cm = tc.tile_pool(name="ph_D", bufs=1, side="right")
            phD = phD_cm.__enter__()

            def dense_partial(t8, pspool):
                b_, qc_ = t8 // 2, t8 % 2
                for jj in range(NFC):
                    ps = pspool.tile([P, TC], F32, tag="dn", bufs=2,
                                     name="ps_dn")
                    for p2 in range(2):
                        nc.tensor.matmul(
                            ps[:, :],
                            wd_all[:, p2 * H + jj * P:
                                   p2 * H + (jj + 1) * P],
                            ctxF[b_][:, qc_ * 2 * TC + p2 * TC:
                                     qc_ * 2 * TC + (p2 + 1) * TC],
                            start=(p2 == 0), stop=(p2 == 1))
                    pt = phD.tile([P, TC], BF16, tag="dpt", bufs=3,
                                  name="dpt")
                    nc.scalar.activation(pt[:, :], ps[:, :], AF.Copy)
                    nc.sync.dma_start(
                        out=rs_d_in[t8][jj * P:(jj + 1) * P, :],
                        in_=pt[:, :])
                nc.gpsimd.collective_compute(
                    "ReduceScatter", ALU.add, replica_groups=RG,
                    ins=[rs_d_in[t8][:, :].opt()],
                    outs=[rs_d_out[t8][:, :].opt()])

            def dense_consume(t8, pspool, stag, qtag, sbufs):
                sl = slice(t8 * TC, (t8 + 1) * TC)
                for m in range(2):
                    if zero_bias:
                        nc.sync.dma_start(
                            out=attn_sb[m][:, sl],
                            in_=rs_d_out[t8][m * P:(m + 1) * P, :])
                    else:
                        tmp = phD.tile([P, TC], BF16, tag="rsb", bufs=2,
                                       name="rsb")
                        nc.sync.dma_start(
                            out=tmp[:, :],
                            in_=rs_d_out[t8][m * P:(m + 1) * P, :])
                        nc.vector.tensor_scalar(
                            attn_sb[m][:, sl], tmp[:, :],
                            bdense_sb[:, m:m + 1], 0.0, ALU.add, ALU.add)
                _stats_t8(nc, phD, pspool, attn_sb, t8,
                          ar3q_in[t8 // 2], ones_bf, stag=stag, qtag=qtag,
                          sbufs=sbufs, slot=t8 % 2)
                if t8 % 2 == 1:
                    p = t8 // 2
                    nc.gpsimd.collective_compute(
                        "AllReduce", ALU.add, replica_groups=RG,
                        ins=[ar3q_in[p][:, :].opt()],
                        outs=[ar3q_out[p][:, :].opt()])

            def ln3_pair(p, pspool, stag, qtag, sbufs):
                ab3 = _ln_rows_batch(nc, phD, ar3q_out[p], f"ln3p{p}",
                                     nrows=2)
                for t8 in range(2 * p, 2 * p + 2):
                    a_b, b2_b = _ln_bcast(nc, phD, ab3, t8 % 2)
                    for m in range(2):
                        hres = phD.tile([P, TC], F32, tag="hres",
                                        bufs=2, name="hres")
                        nc.sync.dma_start(
                            out=hres[:, :],
                            in_=h_res[m * P:(m + 1) * P,
                                      t8 * TC:(t8 + 1) * TC])
                        sl = slice(t8 * TC, (t8 + 1) * TC)
                        t1 = phD.tile([P, TC], F32, tag="t1", name="t1")
                        t2 = phD.tile([P, TC], F32, tag="t2", name="t2")
                        nc.vector.tensor_mul(t1[:, :], attn_sb[m][:, sl],
                                             a_b[:, :])
                        nc.vector.tensor_add(t1[:, :], t1[:, :], b2_b[:, :])
                        nc.vector.tensor_scalar(t2[:, :], t1[:, :],
                                                ln3w_sb[:, m:m + 1],
                                                ln3b_sb[:, m:m + 1],
                                                ALU.mult, ALU.add)
                        nc.vector.tensor_add(ln_in[m][:, sl], t2[:, :],
                                             hres[:, :])
                    _stats_t8(nc, phD, pspool, ln_in, t8, ar2q_in[p],
                              ones_bf, stag=stag, qtag=qtag, sbufs=sbufs,
                              slot=t8 % 2)
                nc.gpsimd.collective_compute(
                    "AllReduce", ALU.add, replica_groups=RG,
                    ins=[ar2q_in[p][:, :].opt()],
                    outs=[ar2q_out[p][:, :].opt()])

            def ln2_pair(p):
                ab2 = _ln_rows_batch(nc, phD, ar2q_out[p], f"ln2p{p}",
                                     nrows=2)
                for t8 in range(2 * p, 2 * p + 2):
                    a_b, b2_b = _ln_bcast(nc, phD, ab2, t8 % 2)
                    tq = t8 % 2
                    for m in range(2):
                        sl = slice(t8 * TC, (t8 + 1) * TC)
                        t1 = phD.tile([P, TC], F32, tag="t1", name="t1")
                        nc.vector.tensor_mul(t1[:, :], ln_in[m][:, sl],
                                             a_b[:, :])
                        nc.vector.tensor_add(t1[:, :], t1[:, :], b2_b[:, :])
                        x2q = phD.tile([P, TC], BF16, tag="x2q", bufs=2,
                                       name="x2q")
                        nc.vector.tensor_scalar(
                            x2q[:, :], t1[:, :], ln2w_sb[:, m:m + 1],
                            ln2b_sb[:, m:m + 1], ALU.mult, ALU.add)
                        nc.sync.dma_start(
                            out=ag_x2_in[p][:, tq * 1024 + m * TC:
                                            tq * 1024 + (m + 1) * TC],
                            in_=x2q[:, :])
                nc.gpsimd.collective_compute(
                    "AllGather", ALU.bypass, replica_groups=RG,
                    ins=[ag_x2_in[p][:, :].opt()],
                    outs=[ag_x2_out[p][:, :].opt()])

            with tc.tile_pool(name="ph_c", bufs=1) as phc, \
                 tc.tile_pool(name="ph_c_ps", bufs=1, space="PSUM") as phcps:
                for b in range(B):
                    ctxF[b] = ctx_pool.tile([P, 2 * S], BF16, tag="ctxF",
                                            bufs=2, name=f"ctxF{b}")
                    if b == 1:
                        for fc in range(NFC):
                            nc.sync.dma_start(
                                out=wh_all[:, fc * FC:(fc + 1) * FC],
                                in_=w_h4h[fc * P:(fc + 1) * P, :])
                    for qc in range(S // TC):
                        t8c = 2 * b + qc
                        ctx_ps = [phcps.tile([P, TC], F32, tag=f"ctx{p}",
                                             bufs=1, name=f"ctx_ps{p}")
                                  for p in range(2)]
                        den_ps = phcps.tile([P, TC], F32, tag="den",
                                            bufs=1, name="den_ps")
                        kts = [kt for kt in range(S // P)
                               if block_status[(kt, qc)] != "skip"]
                        nkt = len(kts)

                        def emit_scores(ki):
                            kt = kts[ki]
                            st = block_status[(kt, qc)]
                            es = []
                            for h in range(HPC):
                                pair, rho = h // 2, h % 2
                                ps_s = phcps.tile([P, TC], F32, tag="s",
                                                  bufs=3, name="ps_s")
                                qoff = pair * T + b * S + qc * TC
                                koff = pair * T + b * S + kts[ki] * P
                                nc.tensor.matmul(
                                    ps_s[:, :],
                                    kT2[rho * HD:(rho + 1) * HD,
                                        koff:koff + P],
                                    qT2[rho * HD:(rho + 1) * HD,
                                        qoff:qoff + TC],
                                    start=True, stop=True)
                                e = phc.tile([P, TC], BF16, tag="e", bufs=6,
                                             name="e")
                                nc.scalar.activation(e[:, :], ps_s[:, :],
                                                     AF.Exp,
                                                     scale=1.0 / np.sqrt(HD))
                                if st == "masked":
                                    i = mask_slot[(kt, qc)]
                                    nc.vector.tensor_mul(
                                        e[:, :], e[:, :],
                                        mask_sb[:, i * TC:(i + 1) * TC])
                                es.append(e)
                            return es

                        def emit_ctx(ki, es):
                            kt = kts[ki]
                            ttg = b * (S // P) + kt
                            for h in range(HPC):
                                pair, rho = h // 2, h % 2
                                nc.tensor.matmul(
                                    ctx_ps[pair][rho * HD:(rho + 1) * HD, :],
                                    v_sb[:, ttg * DC + h * HD:
                                         ttg * DC + (h + 1) * HD],
                                    es[h][:, :],
                                    start=(ki == 0), stop=(ki == nkt - 1))
                                nc.tensor.matmul(
                                    den_ps[32 * h:32 * h + 1, :],
                                    ones_bf[:, 0:1], es[h][:, :],
                                    start=(ki == 0), stop=(ki == nkt - 1),
                                    tile_position=(0, 32 * h))

                        prev = emit_scores(0)
                        for ki in range(1, nkt):
                            cur = emit_scores(ki)
                            emit_ctx(ki - 1, prev)
                            prev = cur
                        emit_ctx(nkt - 1, prev)
                        # batched reciprocal of all 4 head denominators
                        rd = phc.tile([P, TC], F32, tag="rd", bufs=1,
                                      name="rd")
                        rd_bf = phc.tile([P, TC], BF16, tag="rd_bf", bufs=1,
                                         name="rd_bf")
                        nc.vector.reciprocal(rd[:, :], den_ps[:, :])
                        nc.vector.tensor_copy(rd_bf[:, :], rd[:, :])
                        for h in range(HPC):
                            pair, rho = h // 2, h % 2
                            r32 = slice(32 * h, 32 * h + 1)
                            ps_b = phcps.tile([P, TC], F32, tag="s", bufs=3,
                                              name="ps_b")
                            nc.tensor.matmul(ps_b[:, :],
                                             ones_rows_bf[r32, :],
                                             rd_bf[r32, :], start=True,
                                             stop=True,
                                             tile_position=(32 * h, 0))
                            rd_b = phc.tile([P, TC], F32, tag="rd_b", bufs=1,
                                            name="rd_b")
                            nc.vector.tensor_copy(rd_b[:, :], ps_b[:, :])
                            off = qc * 2 * TC + pair * TC
                            hs = slice(rho * HD, (rho + 1) * HD)
                            nc.vector.tensor_mul(ctxF[b][hs, off:off + TC],
                                                 ctx_ps[pair][hs, :],
                                                 rd_b[hs, :])
                        if t8c == 6:
                            ln2_pair(0)
                        dense_partial(t8c, phcps)
                        if t8c >= 2:
                            dense_consume(t8c - 2, phcps, "s", "s", 3)
                        if t8c == 5:
                            ln3_pair(0, phcps, "s", "s", 3)
                        if t8c == 6:
                            ln3_pair(1, phcps, "s", "s", 3)
                        if t8c == 7:
                            ln2_pair(1)
                dense_consume(6, phcps, "s", "s", 3)
            phdw_cm.__exit__(None, None, None)
            ctx_cm.__exit__(None, None, None)
            attn_res_cm.__exit__(None, None, None)

            # =========================================================
            # Phase E+F: h4h+gelu -> 4hh row-parallel partials -> per-t8
            # ReduceScatter; LN4 applies interleaved.
            # PSUM: h(2) + f(2) + st(2) = 6 banks.
            # =========================================================
            with tc.tile_pool(name="ph_e_w", bufs=1) as phew, \
                 tc.tile_pool(name="ph_ef", bufs=1) as phef, \
                 tc.tile_pool(name="ph_ef_ps", bufs=1, space="PSUM") as pheps:
                w4_all = phew.tile([P, (FC // P) * H], BF16, name="w4_all")
                for j in range(FC // P):
                    nc.sync.dma_start(out=w4_all[:, j * H:(j + 1) * H],
                                      in_=w_4hh[j * P:(j + 1) * P, :])
                mlp_sb = [resid.tile([P, T], BF16, tag="colsM", bufs=2,
                                     name=f"mlp_sb{m}") for m in range(2)]

                inter_t = {}

                def h4h_chunk(t8):
                    hh, t8r = t8 // 2, t8 % 2
                    x2c_all = phef.tile([P, NFC * TC], BF16, tag="x2c",
                                        bufs=2, name="x2c_all")
                    for c8 in range(NC):
                        for m2 in range(2):
                            fc = c8 * 2 + m2
                            nc.sync.dma_start(
                                out=x2c_all[:, fc * TC:(fc + 1) * TC],
                                in_=ag_x2_out[hh][c8 * P:(c8 + 1) * P,
                                                  t8r * 1024 + m2 * TC:
                                                  t8r * 1024 + (m2 + 1) * TC])
                    inter = phef.tile([P, (FC // P) * TC], BF16, tag="inter",
                                      bufs=2, name="inter")
                    for g in range(4):
                        ps = [pheps.tile([P, TC], F32, tag=f"h{mi}", bufs=1,
                                         name=f"ps_h{mi}") for mi in range(2)]
                        for fc in range(NFC):
                            for mi in range(2):
                                m = g * 2 + mi
                                nc.tensor.matmul(
                                    ps[mi][:, :],
                                    wh_all[:, fc * FC + m * P:
                                           fc * FC + (m + 1) * P],
                                    x2c_all[:, fc * TC:(fc + 1) * TC],
                                    start=(fc == 0), stop=(fc == NFC - 1))
                        for mi in range(2):
                            m = g * 2 + mi
                            nc.scalar.activation(
                                inter[:, m * TC:(m + 1) * TC], ps[mi][:, :],
                                AF.Gelu_apprx_tanh,
                                bias=bh4h_sb[:, m:m + 1])
                    inter_t[t8] = inter

                def fourhh_partial(t8):
                    inter = inter_t.pop(t8)
                    split = (t8 == NTC - 1)
                    order = ([2 * i for i in range(NFC // 2)]
                             + [2 * i + 1 for i in range(NFC // 2)]
                             if split else range(NFC))
                    for jj in order:
                        ps = pheps.tile([P, TC], F32, tag="f", bufs=2,
                                        name="ps_f")
                        for j in range(FC // P):
                            nc.tensor.matmul(
                                ps[:, :],
                                w4_all[:, j * H + jj * P:
                                       j * H + (jj + 1) * P],
                                inter[:, j * TC:(j + 1) * TC],
                                start=(j == 0), stop=(j == FC // P - 1))
                        pt = phef.tile([P, TC], BF16, tag="pt", bufs=4,
                                       name="pt")
                        if jj % 2 == 0:
                            nc.vector.tensor_copy(pt[:, :], ps[:, :])
                        else:
                            nc.scalar.activation(pt[:, :], ps[:, :], AF.Copy)
                        if split:
                            nc.sync.dma_start(
                                out=rs_m7_in[jj % 2][(jj // 2) * P:
                                                    (jj // 2 + 1) * P, :],
                                in_=pt[:, :])
                            if jj == NFC - 2 or jj == NFC - 1:
                                h = jj % 2
                                nc.gpsimd.collective_compute(
                                    "ReduceScatter", ALU.add,
                                    replica_groups=RG,
                                    ins=[rs_m7_in[h][:, :].opt()],
                                    outs=[rs_m7_out[h][:, :].opt()])
                        else:
                            nc.sync.dma_start(
                                out=rs_mlp_in[t8][jj * P:(jj + 1) * P, :],
                                in_=pt[:, :])
                    if not split:
                        nc.gpsimd.collective_compute(
                            "ReduceScatter", ALU.add, replica_groups=RG,
                            ins=[rs_mlp_in[t8][:, :].opt()],
                            outs=[rs_mlp_out[t8][:, :].opt()])

                def mlp_consume(t8):
                    sl = slice(t8 * TC, (t8 + 1) * TC)
                    for m in range(2):
                        src_ap = (rs_m7_out[m][:, :] if t8 == NTC - 1
                                  else rs_mlp_out[t8][m * P:(m + 1) * P, :])
                        if zero_bias:
                            nc.sync.dma_start(
                                out=mlp_sb[m][:, sl], in_=src_ap)
                        else:
                            tmp = phef.tile([P, TC], BF16, tag="rsb", bufs=2,
                                            name="rsb")
                            nc.sync.dma_start(out=tmp[:, :], in_=src_ap)
                            nc.vector.tensor_scalar(
                                mlp_sb[m][:, sl], tmp[:, :],
                                b4hh_sb[:, m:m + 1], 0.0, ALU.add, ALU.add)
                    _stats_t8(nc, phD, pheps, mlp_sb, t8,
                              ar4q_in[t8 // 2], ones_bf, slot=t8 % 2)
                    if t8 % 2 == 1:
                        qq = t8 // 2
                        nc.gpsimd.collective_compute(
                            "AllReduce", ALU.add, replica_groups=RG,
                            ins=[ar4q_in[qq][:, :].opt()],
                            outs=[ar4q_out[qq][:, :].opt()])

                def ln4_apply(qq):
                    ab4 = _ln_rows_batch(nc, phD, ar4q_out[qq],
                                         f"ln4q{qq}", nrows=2)
                    for t8 in range(2 * qq, 2 * qq + 2):
                        a_b, b2_b = _ln_bcast(nc, phD, ab4, t8 % 2)
                        for m in range(2):
                            sl = slice(t8 * TC, (t8 + 1) * TC)
                            t1 = phD.tile([P, TC], F32, tag="t1", name="t1")
                            t2 = phD.tile([P, TC], F32, tag="t2", name="t2")
                            nc.vector.tensor_mul(t1[:, :], mlp_sb[m][:, sl],
                                                 a_b[:, :])
                            nc.vector.tensor_add(t1[:, :], t1[:, :],
                                                 b2_b[:, :])
                            nc.vector.tensor_scalar(t2[:, :], t1[:, :],
                                                    ln4w_sb[:, m:m + 1],
                                                    ln4b_sb[:, m:m + 1],
                                                    ALU.mult, ALU.add)
                            ot = phD.tile([P, TC], F32, tag="ot", name="ot")
                            nc.vector.tensor_add(ot[:, :], t2[:, :],
                                                 ln_in[m][:, sl])
                            nc.sync.dma_start(
                                out=out_ext[m * P:(m + 1) * P,
                                            t8 * TC:(t8 + 1) * TC],
                                in_=ot[:, :])

                for t8 in range(NTC):
                    h4h_chunk(t8)
                    if t8 == 0:
                        ln3_pair(2, pheps, "st_s", "st_q", 1)
                    if t8 == 1:
                        dense_consume(7, pheps, "st_s", "st_q", 1)
                        ln2_pair(2)
                    if t8 == 2:
                        ln3_pair(3, pheps, "st_s", "st_q", 1)
                    if t8 == 3:
                        ln2_pair(3)
                    if t8 >= 1:
                        fourhh_partial(t8 - 1)
                    if t8 >= 2:
                        mlp_consume(t8 - 2)
                    if t8 == 5:
                        ln4_apply(0)
                    if t8 == 7:
                        ln4_apply(1)
                fourhh_partial(NTC - 1)
                mlp_consume(6)
                ln4_apply(2)
                mlp_consume(7)
                ln4_apply(3)
            phD_cm.__exit__(None, None, None)
            whp_cm.__exit__(None, None, None)

    nc.compile()
    return nc


def _stats_t8(nc, pool, pspool, rows, t8, ar_in, ones_bf,
              stag="st_s", qtag="st_q", sbufs=1, slot=None):
    """Sum & sumsq over the 256 local features of token-chunk t8 (bf16)."""
    if slot is None:
        slot = t8
    ps_s = pspool.tile([1, TC], F32, tag=stag, bufs=sbufs, name="ps_s")
    ps_q = pspool.tile([1, TC], F32, tag=qtag, bufs=sbufs, name="ps_q")
    sl = slice(t8 * TC, (t8 + 1) * TC)
    osl = slice(slot * TC, (slot + 1) * TC)
    for m in range(2):
        nc.tensor.matmul(ps_s[:, :], ones_bf[:, 0:1], rows[m][:, sl],
                         start=(m == 0), stop=(m == 1))
    for m in range(2):
        sq = pool.tile([P, TC], BF16, tag="sq", bufs=2, name="sq")
        nc.vector.tensor_mul(sq[:, :], rows[m][:, sl], rows[m][:, sl])
        nc.tensor.matmul(ps_q[:, :], ones_bf[:, 0:1], sq[:, :],
                         start=(m == 0), stop=(m == 1))
    tmp_s = pool.tile([1, TC], F32, tag="tmp_s", bufs=1, name="tmp_s")
    tmp_q = pool.tile([1, TC], F32, tag="tmp_q", bufs=1, name="tmp_q")
    nc.vector.tensor_copy(tmp_s[:, :], ps_s[:, :])
    nc.vector.tensor_copy(tmp_q[:, :], ps_q[:, :])
    nc.sync.dma_start(out=ar_in[0:1, osl], in_=tmp_s[:, :])
    nc.sync.dma_start(out=ar_in[1:2, osl], in_=tmp_q[:, :])


def _ln_rows_batch(nc, pool, ar_out, name, nrows=8):
    """Batched LN row math on [nrows,TC] tiles, one reciprocal total."""
    s8 = pool.tile([nrows, TC], F32, tag="lnrb_s8", bufs=1, name=f"{name}_s8")
    q8 = pool.tile([nrows, TC], F32, tag="lnrb_q8", bufs=1, name=f"{name}_q8")
    nc.sync.dma_start(out=s8[:, :], in_=ar_out[0:1, :])
    nc.sync.dma_start(out=q8[:, :], in_=ar_out[1:2, :])
    mu = pool.tile([nrows, TC], F32, tag="lnrb_mu", bufs=1, name=f"{name}_mu")
    a8 = pool.tile([nrows, TC], F32, tag="lnrb_a8", bufs=1, name=f"{name}_a8")
    b28 = pool.tile([nrows, TC], F32, tag="lnrb_b28", bufs=1,
                    name=f"{name}_b28")
    nc.vector.tensor_scalar_mul(mu[:, :], s8[:, :], 1.0 / H)
    nc.vector.tensor_scalar_mul(q8[:, :], q8[:, :], 1.0 / H)
    nc.vector.tensor_mul(b28[:, :], mu[:, :], mu[:, :])
    nc.vector.tensor_sub(q8[:, :], q8[:, :], b28[:, :])
    nc.scalar.activation(q8[:, :], q8[:, :], AF.Sqrt, bias=EPS)
    nc.vector.reciprocal(a8[:, :], q8[:, :])
    nc.vector.tensor_mul(b28[:, :], mu[:, :], a8[:, :])
    nc.vector.tensor_scalar_mul(b28[:, :], b28[:, :], -1.0)
    return a8, b28


def _ln_bcast(nc, pool, ab, t8):
    """Extract row t8 from the batched (a8,b28) and broadcast to [P,TC]."""
    a8, b28 = ab
    a_row = pool.tile([1, TC], F32, tag="a_row", name="a_row")
    b2_row = pool.tile([1, TC], F32, tag="b2_row", name="b2_row")
    nc.sync.dma_start(out=a_row[:, :], in_=a8[t8:t8 + 1, :])
    nc.sync.dma_start(out=b2_row[:, :], in_=b28[t8:t8 + 1, :])
    a_b = pool.tile([P, TC], F32, tag="a_b", name="a_b")
    b2_b = pool.tile([P, TC], F32, tag="b2_b", name="b2_b")
    nc.gpsimd.partition_broadcast(a_b[:, :], a_row[:, :])
    nc.gpsimd.partition_broadcast(b2_b[:, :], b2_row[:, :])
    return a_b, b2_b


# ----------------------------------------------------------------------
_cache = {}


def _get_program(mask_np, zero_bv, zero_bias):
    key = (mask_np.tobytes(), zero_bv, zero_bias)
    kh = hash(key)
    if kh not in _cache:
        _cache[kh] = build_program(_causal_block_status(mask_np), zero_bv,
                                   zero_bias)
    return _cache[kh]


def kernel(hidden_states, mask, ln1_w, ln1_b, w_qkv, b_qkv, w_dense, b_dense,
           ln3_w, ln3_b, ln2_w, ln2_b, w_h4h, b_h4h, w_4hh, b_4hh,
           ln4_w, ln4_b):
    hidden_states = np.asarray(hidden_states, np.float32)
    mask2d = np.asarray(mask, np.float32).reshape(S, S)
    w_qkv = np.asarray(w_qkv, np.float32)
    b_qkv = np.asarray(b_qkv, np.float32)
    w_dense = np.asarray(w_dense, np.float32)
    w_h4h = np.asarray(w_h4h, np.float32)
    w_4hh = np.asarray(w_4hh, np.float32)

    zero_bv = bool(np.all(b_qkv[2 * H:] == 0.0))
    zero_bias = bool(np.all(b_qkv[:2 * H] == 0.0)
                     and np.all(np.asarray(b_dense) == 0.0)
                     and np.all(np.asarray(b_4hh) == 0.0))
    prog = _get_program(mask2d, zero_bv, zero_bias)

    hT = np.ascontiguousarray(hidden_states.reshape(T, H).T)
    maskT_bf = np.ascontiguousarray(mask2d.T).astype(bf16)

    in_maps = []
    for c in range(NC):
        qs = slice(c * DC, (c + 1) * DC)
        wq_c = np.concatenate([w_qkv[:, c * DC:(c + 1) * DC],
                               w_qkv[:, H + c * DC:H + (c + 1) * DC],
                               w_qkv[:, 2 * H + c * DC:2 * H + (c + 1) * DC]],
                              axis=1)
        b_qk_c = np.concatenate([b_qkv[c * DC:(c + 1) * DC],
                                 b_qkv[H + c * DC:H + (c + 1) * DC]])
        b_v_c = b_qkv[2 * H + c * DC:2 * H + (c + 1) * DC]
        im = {
            "h_ln1": np.ascontiguousarray(
                hT[:, c * TC:(c + 1) * TC]).astype(bf16),
            "h_res": np.ascontiguousarray(hT[qs, :]),
            "ln1_w": np.asarray(ln1_w, np.float32).reshape(H, 1),
            "ln1_b": np.asarray(ln1_b, np.float32).reshape(H, 1),
            "ln2_w": np.asarray(ln2_w, np.float32)[qs].reshape(DC, 1),
            "ln2_b": np.asarray(ln2_b, np.float32)[qs].reshape(DC, 1),
            "ln3_w": np.asarray(ln3_w, np.float32)[qs].reshape(DC, 1),
            "ln3_b": np.asarray(ln3_b, np.float32)[qs].reshape(DC, 1),
            "ln4_w": np.asarray(ln4_w, np.float32)[qs].reshape(DC, 1),
            "ln4_b": np.asarray(ln4_b, np.float32)[qs].reshape(DC, 1),
            "w_qkv": np.ascontiguousarray(wq_c).astype(bf16),
            "b_qk": np.ascontiguousarray(b_qk_c).reshape(2 * DC, 1),
            "b_v": np.ascontiguousarray(b_v_c).reshape(1, DC),
            "w_dense": np.ascontiguousarray(w_dense[qs, :]).astype(bf16),
            "b_dense": np.asarray(b_dense, np.float32)[qs].reshape(DC, 1),
            "w_h4h": np.ascontiguousarray(
                w_h4h[:, c * FC:(c + 1) * FC]).astype(bf16),
            "b_h4h": np.asarray(b_h4h, np.float32)[
                c * FC:(c + 1) * FC].reshape(FC, 1),
            "w_4hh": np.ascontiguousarray(
                w_4hh[c * FC:(c + 1) * FC, :]).astype(bf16),
            "b_4hh": np.asarray(b_4hh, np.float32)[qs].reshape(DC, 1),
            "maskT": maskT_bf,
        }
        in_maps.append(im)

    res = run_bass_kernel_spmd(prog, in_maps, core_ids=list(range(NC)))
    outT = np.concatenate([res.results[c]["out"] for c in range(NC)], axis=0)
    return np.ascontiguousarray(outT.T).reshape(B, S, H).astype(np.float32)


# revision 23
# speedup vs baseline: 1.0189x; 1.0013x over previous
"""Trainium2 8-core tensor-parallel transformer layer — v10.

On top of v9 (MLP 4hh row-parallel + per-chunk ReduceScatter):
- Dense (attention output) projection also row-parallel + per-chunk
  ReduceScatter: each core contracts its own 4 heads' ctx (straight from
  SBUF, no ctx AllGather / DRAM bounce) into a [H, TC] partial, RS'd
  down to the core's 256 resident features.
- Phase D (LN3 -> AR2 -> LN2 -> x2 AllGather) for token-half 0 is
  emitted *inside* the attention loop (split into LN3-part and
  LN2-part) so the x2 AG completes while attention for batch 3 is
  still on the tensor engine; half 1 is emitted right after the first
  h4h chunk of the MLP.
- Softmax mask-multiplies moved to GpSimd so early-emitted phase-D
  vector work cannot stall the attention pipeline.
- LN4 stat AllReduces quartered; LN4 applies interleaved into the MLP
  loop to shrink the tail.
"""

import os
import sys

sys.path.insert(0, "/opt/trn_rl_repo")
os.environ.setdefault("MYCRO_LOCAL_CACHE", "1")
os.environ.setdefault("JAX_PLATFORMS", "cpu,axon")

import numpy as np
import ml_dtypes

import concourse.bass as bass
import concourse.mybir as mybir
import concourse.tile as tile
from concourse import bacc
from concourse.bass_utils import run_bass_kernel_spmd

F32 = mybir.dt.float32
BF16 = mybir.dt.bfloat16
AF = mybir.ActivationFunctionType
ALU = mybir.AluOpType

P = 128
B, S, H, NH = 4, 1024, 2048, 32
HD = H // NH
T = B * S
NC = 8
HPC = NH // NC                 # 4 heads/core
DC = H // NC                   # 256
FC = 4 * H // NC               # 1024
F4 = 4 * H                     # 8192
TC = 512
NTC = T // TC                  # 8
NFC = H // P                   # 16
EPS = 1e-5
RG = [list(range(NC))]

bf16 = ml_dtypes.bfloat16


def _causal_block_status(mask2d):
    mt = mask2d.T
    status = {}
    patterns = {}   # fingerprint -> unique slot
    slot_of = {}    # (kt, qc) -> (unique slot, src block)
    for kt in range(S // P):
        for qc in range(S // TC):
            blk = mt[kt * P:(kt + 1) * P, qc * TC:(qc + 1) * TC]
            if np.all(blk == 0):
                status[(kt, qc)] = "skip"
            elif np.all(blk == 1):
                status[(kt, qc)] = "full"
            else:
                status[(kt, qc)] = "masked"
                fp = blk.astype(np.float32).tobytes()
                if fp not in patterns:
                    patterns[fp] = (len(patterns), (kt, qc))
                slot_of[(kt, qc)] = patterns[fp][0]
    uniq = [src for _, src in sorted(patterns.values())]
    return status, slot_of, uniq


def build_program(blockinfo, zero_bv=True, zero_bias=True):
    block_status, mask_slot, mask_uniq = blockinfo
    nc = bacc.Bacc("TRN2", target_bir_lowering=False, debug=False,
                   num_devices=NC)

    def register_const_ap(dtype, value):
        t = nc.alloc_sbuf_tensor(f"const-{dtype.name}-{value}", [128, 1], dtype)
        nc.gpsimd.memset(t.ap(), value)
        nc.const_aps.aps[(dtype, value)] = t.ap()

    register_const_ap(F32, EPS)
    register_const_ap(F32, float(1.0 / np.sqrt(HD)))
    nc.all_engine_barrier()

    # ---------------- DRAM I/O ----------------
    h_ln1 = nc.dram_tensor("h_ln1", [H, TC], BF16, kind="ExternalInput")
    h_res = nc.dram_tensor("h_res", [DC, T], F32, kind="ExternalInput")
    ln1_w = nc.dram_tensor("ln1_w", [H, 1], F32, kind="ExternalInput")
    ln1_b = nc.dram_tensor("ln1_b", [H, 1], F32, kind="ExternalInput")
    ln2_w = nc.dram_tensor("ln2_w", [DC, 1], F32, kind="ExternalInput")
    ln2_b = nc.dram_tensor("ln2_b", [DC, 1], F32, kind="ExternalInput")
    ln3_w = nc.dram_tensor("ln3_w", [DC, 1], F32, kind="ExternalInput")
    ln3_b = nc.dram_tensor("ln3_b", [DC, 1], F32, kind="ExternalInput")
    ln4_w = nc.dram_tensor("ln4_w", [DC, 1], F32, kind="ExternalInput")
    ln4_b = nc.dram_tensor("ln4_b", [DC, 1], F32, kind="ExternalInput")
    w_qkv = nc.dram_tensor("w_qkv", [H, 3 * DC], BF16, kind="ExternalInput")
    b_qk = nc.dram_tensor("b_qk", [2 * DC, 1], F32, kind="ExternalInput")
    b_v = nc.dram_tensor("b_v", [1, DC], F32, kind="ExternalInput")
    w_dense = nc.dram_tensor("w_dense", [DC, H], BF16, kind="ExternalInput")
    b_dense = nc.dram_tensor("b_dense", [DC, 1], F32, kind="ExternalInput")
    w_h4h = nc.dram_tensor("w_h4h", [H, FC], BF16, kind="ExternalInput")
    b_h4h = nc.dram_tensor("b_h4h", [FC, 1], F32, kind="ExternalInput")
    w_4hh = nc.dram_tensor("w_4hh", [FC, H], BF16, kind="ExternalInput")
    b_4hh = nc.dram_tensor("b_4hh", [DC, 1], F32, kind="ExternalInput")
    maskT = nc.dram_tensor("maskT", [S, S], BF16, kind="ExternalInput")
    out_ext = nc.dram_tensor("out", [DC, T], F32, kind="ExternalOutput")


    with tile.TileContext(nc) as tc:
        with tc.tile_pool(name="const", bufs=1) as const, \
             tc.tile_pool(name="resid", bufs=1) as resid, \
             tc.tile_pool(name="dram", bufs=1, space="DRAM") as dram:

            # ---------- constants ----------
            ones_f = const.tile([P, 1], F32)
            nc.vector.memset(ones_f[:, :], 1.0)
            ones_bf = const.tile([P, 1], BF16)
            nc.vector.memset(ones_bf[:, :], 1.0)
            ones_rows_bf = const.tile([P, P], BF16)
            nc.vector.memset(ones_rows_bf[:, :], 1.0)

            ln1w_sb = const.tile([P, NFC], F32)
            ln1b_sb = const.tile([P, NFC], F32)

            cpack = const.tile([P, 28], F32)
            _cofs = [0]

            def load_cols(t, ncols=2):
                base = _cofs[0]
                _cofs[0] += ncols
                for m in range(ncols):
                    nc.sync.dma_start(out=cpack[:, base + m:base + m + 1],
                                      in_=t[m * P:(m + 1) * P, 0:1])
                return cpack[:, base:base + ncols]

            # ---------- residents ----------
            ln_in = [resid.tile([P, T], BF16, name=f"ln_in{m}")
                     for m in range(2)]
            attn_sb = [resid.tile([P, T], BF16, tag="colsA", bufs=2,
                                  name=f"attn_sb{m}") for m in range(2)]

            # ---------- DRAM bounces ----------
            ag_x1_in = [dram.tile([P, 4 * TC], BF16, name=f"agx1i{h}")
                        for h in range(4)]
            ag_x1_out = [dram.tile([NC * P, 4 * TC], BF16,
                                   addr_space="Shared", name=f"agx1o{h}")
                         for h in range(4)]
            # dense partials: per-t8 ReduceScatter bounces
            rs_d_in = [dram.tile([H, TC], BF16, name=f"rsdi{k}")
                       for k in range(NTC)]
            rs_d_out = [dram.tile([DC, TC], BF16, name=f"rsdo{k}")
                        for k in range(NTC)]
            # x2: 4 quarters, free = t8r*1024 + m*512
            ag_x2_in = [dram.tile([P, 2 * 1024], BF16, name=f"agx2i{h}")
                        for h in range(4)]
            ag_x2_out = [dram.tile([NC * P, 2 * 1024], BF16,
                                   addr_space="Shared", name=f"agx2o{h}")
                         for h in range(4)]
            # mlp 4hh partials: per-t8 ReduceScatter bounces
            rs_mlp_in = [dram.tile([H, TC], BF16, name=f"rsmi{k}")
                         for k in range(NTC)]
            rs_mlp_out = [dram.tile([DC, TC], BF16, name=f"rsmo{k}")
                          for k in range(NTC)]
            rs_m7_in = [dram.tile([H // 2, TC], BF16, name=f"rsm7i{i}")
                        for i in range(2)]
            rs_m7_out = [dram.tile([P, TC], BF16, name=f"rsm7o{i}")
                         for i in range(2)]
            ar3q_in = [dram.tile([2, 2 * TC], F32, name=f"ar3i{i}")
                       for i in range(4)]
            ar3q_out = [dram.tile([2, 2 * TC], F32, addr_space="Shared",
                                  name=f"ar3o{i}") for i in range(4)]
            ar2q_in = [dram.tile([2, 2 * TC], F32, name=f"ar2i{i}")
                       for i in range(4)]
            ar2q_out = [dram.tile([2, 2 * TC], F32, addr_space="Shared",
                                  name=f"ar2o{i}") for i in range(4)]
            ar4q_in = [dram.tile([2, 2 * TC], F32, name=f"ar4i{i}")
                       for i in range(4)]
            ar4q_out = [dram.tile([2, 2 * TC], F32, addr_space="Shared",
                                  name=f"ar4o{i}") for i in range(4)]

            warm_in = dram.tile([1, 64], BF16, name="warm_in")
            warm_out = dram.tile([NC, 64], BF16, addr_space="Shared",
                                 name="warm_out")
            warm_sb = const.tile([1, 64], BF16)
            nc.vector.memset(warm_sb[:, :], 0.0)
            nc.sync.dma_start(out=warm_in[:, :], in_=warm_sb[:, :])
            nc.gpsimd.collective_compute(
                "AllGather", ALU.bypass, replica_groups=RG,
                ins=[warm_in[:, :].opt()], outs=[warm_out[:, :].opt()])

            # =========================================================
            # Phase A: LN1 -> x1 (bf16) -> 2 half AllGathers
            # =========================================================
            with tc.tile_pool(name="ph_a", bufs=1) as pha, \
                 tc.tile_pool(name="ph_a_ps", bufs=2, space="PSUM") as phaps:
                h1 = [pha.tile([P, TC], BF16, name=f"h1_{fc}")
                      for fc in range(NFC)]
                for fc in range(NFC):
                    nc.sync.dma_start(out=h1[fc][:, :],
                                      in_=h_ln1[fc * P:(fc + 1) * P, :])
                for fc in range(NFC):
                    nc.sync.dma_start(out=ln1w_sb[:, fc:fc + 1],
                                      in_=ln1_w[fc * P:(fc + 1) * P, 0:1])
                    nc.sync.dma_start(out=ln1b_sb[:, fc:fc + 1],
                                      in_=ln1_b[fc * P:(fc + 1) * P, 0:1])
                ln2w_sb = load_cols(ln2_w)
                ln2b_sb = load_cols(ln2_b)
                ln3w_sb = load_cols(ln3_w)
                ln3b_sb = load_cols(ln3_b)
                ln4w_sb = load_cols(ln4_w)
                ln4b_sb = load_cols(ln4_b)
                bdense_sb = load_cols(b_dense)
                b4hh_sb = load_cols(b_4hh)
                bqk_sb = load_cols(b_qk, 4)
                bh4h_sb = load_cols(b_h4h, 8)
                if not zero_bv:
                    bv_row = const.tile([1, DC], F32)
                    nc.sync.dma_start(out=bv_row[:, :], in_=b_v[0:1, :])
                    bv_b = const.tile([P, DC], F32)
                    nc.gpsimd.partition_broadcast(bv_b[:, :], bv_row[:, :])
                if mask_uniq:
                    mask_sb = const.tile([P, len(mask_uniq) * TC], BF16)
                    for i, (kt, qc) in enumerate(mask_uniq):
                        nc.sync.dma_start(
                            out=mask_sb[:, i * TC:(i + 1) * TC],
                            in_=maskT[kt * P:(kt + 1) * P,
                                      qc * TC:(qc + 1) * TC])
                ps_s = phaps.tile([1, TC], F32, name="ps_s")
                ps_q = phaps.tile([1, TC], F32, name="ps_q")
                for fc in range(NFC):
                    nc.tensor.matmul(ps_s[:, :], ones_bf[:, 0:1],
                                     h1[fc][:, :],
                                     start=(fc == 0), stop=(fc == NFC - 1))
                    sq = pha.tile([P, TC], BF16, tag="sq", bufs=3, name="sq")
                    nc.vector.tensor_mul(sq[:, :], h1[fc][:, :],
                                         h1[fc][:, :])
                    nc.tensor.matmul(ps_q[:, :], ones_bf[:, 0:1], sq[:, :],
                                     start=(fc == 0), stop=(fc == NFC - 1))
                mu = pha.tile([1, TC], F32)
                m2 = pha.tile([1, TC], F32)
                var = pha.tile([1, TC], F32)
                sd = pha.tile([1, TC], F32)
                a_row = pha.tile([1, TC], F32)
                b2_row = pha.tile([1, TC], F32)
                nc.vector.tensor_scalar_mul(mu[:, :], ps_s[:, :], 1.0 / H)
                nc.vector.tensor_scalar_mul(m2[:, :], ps_q[:, :], 1.0 / H)
                nc.vector.tensor_mul(var[:, :], mu[:, :], mu[:, :])
                nc.vector.tensor_sub(var[:, :], m2[:, :], var[:, :])
                nc.scalar.activation(sd[:, :], var[:, :], AF.Sqrt, bias=EPS)
                nc.vector.reciprocal(a_row[:, :], sd[:, :])
                nc.vector.tensor_mul(b2_row[:, :], mu[:, :], a_row[:, :])
                nc.vector.tensor_scalar_mul(b2_row[:, :], b2_row[:, :], -1.0)
                a_b = pha.tile([P, TC], F32)
                b2_b = pha.tile([P, TC], F32)
                nc.gpsimd.partition_broadcast(a_b[:, :], a_row[:, :])
                nc.gpsimd.partition_broadcast(b2_b[:, :], b2_row[:, :])
                x1h = [pha.tile([P, 4 * TC], BF16, name=f"x1h{h}")
                       for h in range(4)]
                for fc in range(NFC):
                    t1 = pha.tile([P, TC], F32, tag="t1", bufs=3, name="t1")
                    nc.vector.tensor_mul(t1[:, :], h1[fc][:, :], a_b[:, :])
                    nc.vector.tensor_add(t1[:, :], t1[:, :], b2_b[:, :])
                    hh, fr = fc // 4, fc % 4
                    nc.vector.tensor_scalar(
                        x1h[hh][:, fr * TC:(fr + 1) * TC], t1[:, :],
                        ln1w_sb[:, fc:fc + 1], ln1b_sb[:, fc:fc + 1],
                        ALU.mult, ALU.add)
                    if fr == 3:
                        nc.sync.dma_start(out=ag_x1_in[hh][:, :],
                                          in_=x1h[hh][:, :])
                        nc.gpsimd.collective_compute(
                            "AllGather", ALU.bypass, replica_groups=RG,
                            ins=[ag_x1_in[hh][:, :].opt()],
                            outs=[ag_x1_out[hh][:, :].opt()])

            # =========================================================
            # Phase B: QKV (consumes x1 halves as they arrive)
            # =========================================================
            attn_res_cm = tc.tile_pool(name="attn_res", bufs=1)
            attn_res = attn_res_cm.__enter__()
            qT2 = attn_res.tile([P, 2 * T], BF16)
            kT2 = attn_res.tile([P, 2 * T], BF16)
            v_sb = attn_res.tile([P, (T // P) * DC], BF16)
            with tc.tile_pool(name="ph_b_w", bufs=1) as phbw, \
                 tc.tile_pool(name="ph_b", bufs=2) as phb, \
                 tc.tile_pool(name="ph_b_ps", bufs=3, space="PSUM") as phbps:
                wq_all = phbw.tile([P, NFC * 3 * DC], BF16, name="wq_all")
                for fc in range(NFC):
                    nc.sync.dma_start(
                        out=wq_all[:, fc * 3 * DC:(fc + 1) * 3 * DC],
                        in_=w_qkv[fc * P:(fc + 1) * P, :])
                for t8 in range(NTC):
                    x1c = [phb.tile([P, 4 * TC], BF16, tag=f"x1c{q}",
                                    name=f"x1c{q}") for q in range(4)]
                    for q in range(4):
                        nc.sync.dma_start(
                            out=x1c[q][:, :],
                            in_=ag_x1_out[q][t8 * P:(t8 + 1) * P, :])
                    qk_ps = [phbps.tile([P, 2 * TC], F32, tag=f"qkp{i}",
                                        bufs=1, name=f"qk_ps{i}")
                             for i in range(2)]
                    v_ps = [phbps.tile([P, DC], F32, tag=f"vps{i}", bufs=1,
                                       name=f"v_ps{i}") for i in range(4)]
                    for qt in range(4):
                        for m in range(4):
                            for f in range(4):
                                fc = qt * 4 + f
                                nc.tensor.matmul(
                                    qk_ps[m // 2][:, (m % 2) * TC:
                                                  (m % 2 + 1) * TC],
                                    wq_all[:, fc * 3 * DC + m * P:
                                           fc * 3 * DC + (m + 1) * P],
                                    x1c[qt][:, f * TC:(f + 1) * TC],
                                    start=(fc == 0), stop=(fc == NFC - 1))
                        for tt in range(TC // P):
                            for f in range(4):
                                fc = qt * 4 + f
                                nc.tensor.matmul(
                                    v_ps[tt][:, :],
                                    x1c[qt][:, f * TC + tt * P:
                                            f * TC + (tt + 1) * P],
                                    wq_all[:, fc * 3 * DC + 2 * DC:
                                           fc * 3 * DC + 3 * DC],
                                    start=(fc == 0), stop=(fc == NFC - 1))
                    for m in range(4):
                        dst = qT2 if m < 2 else kT2
                        pair = m % 2
                        off = pair * T + t8 * TC
                        src_ap = qk_ps[m // 2][:, pair * TC:(pair + 1) * TC]
                        if zero_bias:
                            nc.scalar.activation(dst[:, off:off + TC],
                                                 src_ap, AF.Copy)
                        else:
                            nc.scalar.activation(dst[:, off:off + TC],
                                                 src_ap, AF.Identity,
                                                 bias=bqk_sb[:, m:m + 1])
                    voff = t8 * 4 * DC
                    for tt in range(TC // P):
                        if zero_bv:
                            nc.scalar.activation(
                                v_sb[:, voff + tt * DC:voff + (tt + 1) * DC],
                                v_ps[tt][:, :], AF.Copy)
                        else:
                            nc.vector.tensor_add(
                                v_sb[:, voff + tt * DC:voff + (tt + 1) * DC],
                                v_ps[tt][:, :], bv_b[:, :])

            # =========================================================
            # Phase C: attention + row-parallel dense partials + early
            # phase-D (LN3/AR2/LN2/x2-AG) for token-half 0.
            # PSUM: s(3) + ctx(2) + den(1) + dn(2) = 8 banks.
            # =========================================================
            ctx_cm = tc.tile_pool(name="ctx_pool", bufs=1)
            ctx_pool = ctx_cm.__enter__()
            ctxF = {}
            phdw_cm = tc.tile_pool(name="ph_d_w", bufs=1)
            phdw = phdw_cm.__enter__()
            wd_all = phdw.tile([P, 2 * H], BF16, name="wd_all")
            for p2 in range(2):
                nc.sync.dma_start(out=wd_all[:, p2 * H:(p2 + 1) * H],
                                  in_=w_dense[p2 * P:(p2 + 1) * P, :])
            # MLP h4h weights: load early (SBUF region free after QKV)
            whp_cm = tc.tile_pool(name="ph_wh", bufs=1, side="right")
            whp = whp_cm.__enter__()
            wh_all = whp.tile([P, NFC * FC], BF16, name="wh_all")
            # long-lived pool for dense evicts + phase-D/LN4 row work
            phD_cm = tc.tile_pool(name="ph_D", bufs=1, side="right")
            phD = phD_cm.__enter__()

            def dense_partial(t8, pspool):
                b_, qc_ = t8 // 2, t8 % 2
                for jj in range(NFC):
                    ps = pspool.tile([P, TC], F32, tag="dn", bufs=2,
                                     name="ps_dn")
                    for p2 in range(2):
                        nc.tensor.matmul(
                            ps[:, :],
                            wd_all[:, p2 * H + jj * P:
                                   p2 * H + (jj + 1) * P],
                            ctxF[b_][:, qc_ * 2 * TC + p2 * TC:
                                     qc_ * 2 * TC + (p2 + 1) * TC],
                            start=(p2 == 0), stop=(p2 == 1))
                    pt = phD.tile([P, TC], BF16, tag="dpt", bufs=3,
                                  name="dpt")
                    nc.scalar.activation(pt[:, :], ps[:, :], AF.Copy)
                    nc.sync.dma_start(
                        out=rs_d_in[t8][jj * P:(jj + 1) * P, :],
                        in_=pt[:, :])
                nc.gpsimd.collective_compute(
                    "ReduceScatter", ALU.add, replica_groups=RG,
                    ins=[rs_d_in[t8][:, :].opt()],
                    outs=[rs_d_out[t8][:, :].opt()])

            def dense_consume(t8, pspool, stag, qtag, sbufs):
                sl = slice(t8 * TC, (t8 + 1) * TC)
                for m in range(2):
                    if zero_bias:
                        nc.sync.dma_start(
                            out=attn_sb[m][:, sl],
                            in_=rs_d_out[t8][m * P:(m + 1) * P, :])
                    else:
                        tmp = phD.tile([P, TC], BF16, tag="rsb", bufs=2,
                                       name="rsb")
                        nc.sync.dma_start(
                            out=tmp[:, :],
                            in_=rs_d_out[t8][m * P:(m + 1) * P, :])
                        nc.vector.tensor_scalar(
                            attn_sb[m][:, sl], tmp[:, :],
                            bdense_sb[:, m:m + 1], 0.0, ALU.add, ALU.add)
                _stats_t8(nc, phD, pspool, attn_sb, t8,
                          ar3q_in[t8 // 2], ones_bf, stag=stag, qtag=qtag,
                          sbufs=sbufs, slot=t8 % 2)
                if t8 % 2 == 1:
                    p = t8 // 2
                    nc.gpsimd.collective_compute(
                        "AllReduce", ALU.add, replica_groups=RG,
                        ins=[ar3q_in[p][:, :].opt()],
                        outs=[ar3q_out[p][:, :].opt()])

            def ln3_pair(p, pspool, stag, qtag, sbufs):
                ab3 = _ln_rows_batch(nc, phD, ar3q_out[p], f"ln3p{p}",
                                     nrows=2)
                for t8 in range(2 * p, 2 * p + 2):
                    a_b, b2_b = _ln_bcast(nc, phD, ab3, t8 % 2)
                    for m in range(2):
                        hres = phD.tile([P, TC], F32, tag="hres",
                                        bufs=2, name="hres")
                        nc.sync.dma_start(
                            out=hres[:, :],
                            in_=h_res[m * P:(m + 1) * P,
                                      t8 * TC:(t8 + 1) * TC])
                        sl = slice(t8 * TC, (t8 + 1) * TC)
                        t1 = phD.tile([P, TC], F32, tag="t1", name="t1")
                        t2 = phD.tile([P, TC], F32, tag="t2", name="t2")
                        nc.vector.tensor_mul(t1[:, :], attn_sb[m][:, sl],
                                             a_b[:, :])
                        nc.vector.tensor_add(t1[:, :], t1[:, :], b2_b[:, :])
                        nc.vector.tensor_scalar(t2[:, :], t1[:, :],
                                                ln3w_sb[:, m:m + 1],
                                                ln3b_sb[:, m:m + 1],
                                                ALU.mult, ALU.add)
                        nc.vector.tensor_add(ln_in[m][:, sl], t2[:, :],
                                             hres[:, :])
                    _stats_t8(nc, phD, pspool, ln_in, t8, ar2q_in[p],
                              ones_bf, stag=stag, qtag=qtag, sbufs=sbufs,
                              slot=t8 % 2)
                nc.gpsimd.collective_compute(
                    "AllReduce", ALU.add, replica_groups=RG,
                    ins=[ar2q_in[p][:, :].opt()],
                    outs=[ar2q_out[p][:, :].opt()])

            def ln2_pair(p):
                ab2 = _ln_rows_batch(nc, phD, ar2q_out[p], f"ln2p{p}",
                                     nrows=2)
                for t8 in range(2 * p, 2 * p + 2):
                    a_b, b2_b = _ln_bcast(nc, phD, ab2, t8 % 2)
                    tq = t8 % 2
                    for m in range(2):
                        sl = slice(t8 * TC, (t8 + 1) * TC)
                        t1 = phD.tile([P, TC], F32, tag="t1", name="t1")
                        nc.vector.tensor_mul(t1[:, :], ln_in[m][:, sl],
                                             a_b[:, :])
                        nc.vector.tensor_add(t1[:, :], t1[:, :], b2_b[:, :])
                        x2q = phD.tile([P, TC], BF16, tag="x2q", bufs=2,
                                       name="x2q")
                        nc.vector.tensor_scalar(
                            x2q[:, :], t1[:, :], ln2w_sb[:, m:m + 1],
                            ln2b_sb[:, m:m + 1], ALU.mult, ALU.add)
                        nc.sync.dma_start(
                            out=ag_x2_in[p][:, tq * 1024 + m * TC:
                                            tq * 1024 + (m + 1) * TC],
                            in_=x2q[:, :])
                nc.gpsimd.collective_compute(
                    "AllGather", ALU.bypass, replica_groups=RG,
                    ins=[ag_x2_in[p][:, :].opt()],
                    outs=[ag_x2_out[p][:, :].opt()])

            with tc.tile_pool(name="ph_c", bufs=1) as phc, \
                 tc.tile_pool(name="ph_c_ps", bufs=1, space="PSUM") as phcps:
                for b in range(B):
                    ctxF[b] = ctx_pool.tile([P, 2 * S], BF16, tag="ctxF",
                                            bufs=2, name=f"ctxF{b}")
                    if b == 1:
                        for fc in range(NFC):
                            nc.sync.dma_start(
                                out=wh_all[:, fc * FC:(fc + 1) * FC],
                                in_=w_h4h[fc * P:(fc + 1) * P, :])
                    for qc in range(S // TC):
                        t8c = 2 * b + qc
                        ctx_ps = [phcps.tile([P, TC], F32, tag=f"ctx{p}",
                                             bufs=1, name=f"ctx_ps{p}")
                                  for p in range(2)]
                        den_ps = phcps.tile([P, TC], F32, tag="den",
                                            bufs=1, name="den_ps")
                        kts = [kt for kt in range(S // P)
                               if block_status[(kt, qc)] != "skip"]
                        nkt = len(kts)

                        def emit_scores(ki):
                            kt = kts[ki]
                            st = block_status[(kt, qc)]
                            es = []
                            for h in range(HPC):
                                pair, rho = h // 2, h % 2
                                ps_s = phcps.tile([P, TC], F32, tag="s",
                                                  bufs=3, name="ps_s")
                                qoff = pair * T + b * S + qc * TC
                                koff = pair * T + b * S + kts[ki] * P
                                nc.tensor.matmul(
                                    ps_s[:, :],
                                    kT2[rho * HD:(rho + 1) * HD,
                                        koff:koff + P],
                                    qT2[rho * HD:(rho + 1) * HD,
                                        qoff:qoff + TC],
                                    start=True, stop=True)
                                e = phc.tile([P, TC], BF16, tag="e", bufs=6,
                                             name="e")
                                nc.scalar.activation(e[:, :], ps_s[:, :],
                                                     AF.Exp,
                                                     scale=1.0 / np.sqrt(HD))
                                if st == "masked":
                                    i = mask_slot[(kt, qc)]
                                    nc.vector.tensor_mul(
                                        e[:, :], e[:, :],
                                        mask_sb[:, i * TC:(i + 1) * TC])
                                es.append(e)
                            return es

                        def emit_ctx(ki, es):
                            kt = kts[ki]
                            ttg = b * (S // P) + kt
                            for h in range(HPC):
                                pair, rho = h // 2, h % 2
                                nc.tensor.matmul(
                                    ctx_ps[pair][rho * HD:(rho + 1) * HD, :],
                                    v_sb[:, ttg * DC + h * HD:
                                         ttg * DC + (h + 1) * HD],
                                    es[h][:, :],
                                    start=(ki == 0), stop=(ki == nkt - 1))
                                nc.tensor.matmul(
                                    den_ps[32 * h:32 * h + 1, :],
                                    ones_bf[:, 0:1], es[h][:, :],
                                    start=(ki == 0), stop=(ki == nkt - 1),
                                    tile_position=(0, 32 * h))

                        prev = emit_scores(0)
                        for ki in range(1, nkt):
                            cur = emit_scores(ki)
                            emit_ctx(ki - 1, prev)
                            prev = cur
                        emit_ctx(nkt - 1, prev)
                        # batched reciprocal of all 4 head denominators
                        rd = phc.tile([P, TC], F32, tag="rd", bufs=1,
                                      name="rd")
                        rd_bf = phc.tile([P, TC], BF16, tag="rd_bf", bufs=1,
                                         name="rd_bf")
                        nc.vector.reciprocal(rd[:, :], den_ps[:, :])
                        nc.vector.tensor_copy(rd_bf[:, :], rd[:, :])
                        for h in range(HPC):
                            pair, rho = h // 2, h % 2
                            r32 = slice(32 * h, 32 * h + 1)
                            ps_b = phcps.tile([P, TC], F32, tag="s", bufs=3,
                                              name="ps_b")
                            nc.tensor.matmul(ps_b[:, :],
                                             ones_rows_bf[r32, :],
                                             rd_bf[r32, :], start=True,
                                             stop=True,
                                             tile_position=(32 * h, 0))
                            rd_b = phc.tile([P, TC], F32, tag="rd_b", bufs=1,
                                            name="rd_b")
                            nc.vector.tensor_copy(rd_b[:, :], ps_b[:, :])
                            off = qc * 2 * TC + pair * TC
                            hs = slice(rho * HD, (rho + 1) * HD)
                            nc.vector.tensor_mul(ctxF[b][hs, off:off + TC],
                                                 ctx_ps[pair][hs, :],
                                                 rd_b[hs, :])
                        if t8c == 5:
                            ln2_pair(0)
                        dense_partial(t8c, phcps)
                        if t8c >= 2:
                            dense_consume(t8c - 2, phcps, "s", "s", 3)
                        if t8c == 4:
                            ln3_pair(0, phcps, "s", "s", 3)
                        if t8c == 6:
                            ln3_pair(1, phcps, "s", "s", 3)
                        if t8c == 7:
                            ln2_pair(1)
                dense_consume(6, phcps, "s", "s", 3)
            phdw_cm.__exit__(None, None, None)
            ctx_cm.__exit__(None, None, None)
            attn_res_cm.__exit__(None, None, None)

            # =========================================================
            # Phase E+F: h4h+gelu -> 4hh row-parallel partials -> per-t8
            # ReduceScatter; LN4 applies interleaved.
            # PSUM: h(2) + f(2) + st(2) = 6 banks.
            # =========================================================
            with tc.tile_pool(name="ph_e_w", bufs=1) as phew, \
                 tc.tile_pool(name="ph_ef", bufs=1) as phef, \
                 tc.tile_pool(name="ph_ef_ps", bufs=1, space="PSUM") as pheps:
                w4_all = phew.tile([P, (FC // P) * H], BF16, name="w4_all")
                for j in range(FC // P):
                    nc.sync.dma_start(out=w4_all[:, j * H:(j + 1) * H],
                                      in_=w_4hh[j * P:(j + 1) * P, :])
                mlp_sb = [resid.tile([P, T], BF16, tag="colsM", bufs=2,
                                     name=f"mlp_sb{m}") for m in range(2)]

                inter_t = {}

                def h4h_chunk(t8):
                    hh, t8r = t8 // 2, t8 % 2
                    x2c_all = phef.tile([P, NFC * TC], BF16, tag="x2c",
                                        bufs=2, name="x2c_all")
                    for c8 in range(NC):
                        for m2 in range(2):
                            fc = c8 * 2 + m2
                            nc.sync.dma_start(
                                out=x2c_all[:, fc * TC:(fc + 1) * TC],
                                in_=ag_x2_out[hh][c8 * P:(c8 + 1) * P,
                                                  t8r * 1024 + m2 * TC:
                                                  t8r * 1024 + (m2 + 1) * TC])
                    inter = phef.tile([P, (FC // P) * TC], BF16, tag="inter",
                                      bufs=2, name="inter")
                    for g in range(4):
                        ps = [pheps.tile([P, TC], F32, tag=f"h{mi}", bufs=1,
                                         name=f"ps_h{mi}") for mi in range(2)]
                        for fc in range(NFC):
                            for mi in range(2):
                                m = g * 2 + mi
                                nc.tensor.matmul(
                                    ps[mi][:, :],
                                    wh_all[:, fc * FC + m * P:
                                           fc * FC + (m + 1) * P],
                                    x2c_all[:, fc * TC:(fc + 1) * TC],
                                    start=(fc == 0), stop=(fc == NFC - 1))
                        for mi in range(2):
                            m = g * 2 + mi
                            nc.scalar.activation(
                                inter[:, m * TC:(m + 1) * TC], ps[mi][:, :],
                                AF.Gelu_apprx_tanh,
                                bias=bh4h_sb[:, m:m + 1])
                    inter_t[t8] = inter

                def fourhh_partial(t8):
                    inter = inter_t.pop(t8)
                    split = (t8 == NTC - 1)
                    order = ([2 * i for i in range(NFC // 2)]
                             + [2 * i + 1 for i in range(NFC // 2)]
                             if split else range(NFC))
                    for jj in order:
                        ps = pheps.tile([P, TC], F32, tag="f", bufs=2,
                                        name="ps_f")
                        for j in range(FC // P):
                            nc.tensor.matmul(
                                ps[:, :],
                                w4_all[:, j * H + jj * P:
                                       j * H + (jj + 1) * P],
                                inter[:, j * TC:(j + 1) * TC],
                                start=(j == 0), stop=(j == FC // P - 1))
                        pt = phef.tile([P, TC], BF16, tag="pt", bufs=4,
                                       name="pt")
                        if jj % 2 == 0:
                            nc.vector.tensor_copy(pt[:, :], ps[:, :])
                        else:
                            nc.scalar.activation(pt[:, :], ps[:, :], AF.Copy)
                        if split:
                            nc.sync.dma_start(
                                out=rs_m7_in[jj % 2][(jj // 2) * P:
                                                    (jj // 2 + 1) * P, :],
                                in_=pt[:, :])
                            if jj == NFC - 2 or jj == NFC - 1:
                                h = jj % 2
                                nc.gpsimd.collective_compute(
                                    "ReduceScatter", ALU.add,
                                    replica_groups=RG,
                                    ins=[rs_m7_in[h][:, :].opt()],
                                    outs=[rs_m7_out[h][:, :].opt()])
                        else:
                            nc.sync.dma_start(
                                out=rs_mlp_in[t8][jj * P:(jj + 1) * P, :],
                                in_=pt[:, :])
                    if not split:
                        nc.gpsimd.collective_compute(
                            "ReduceScatter", ALU.add, replica_groups=RG,
                            ins=[rs_mlp_in[t8][:, :].opt()],
                            outs=[rs_mlp_out[t8][:, :].opt()])

                def mlp_consume(t8):
                    sl = slice(t8 * TC, (t8 + 1) * TC)
                    for m in range(2):
                        src_ap = (rs_m7_out[m][:, :] if t8 == NTC - 1
                                  else rs_mlp_out[t8][m * P:(m + 1) * P, :])
                        if zero_bias:
                            nc.sync.dma_start(
                                out=mlp_sb[m][:, sl], in_=src_ap)
                        else:
                            tmp = phef.tile([P, TC], BF16, tag="rsb", bufs=2,
                                            name="rsb")
                            nc.sync.dma_start(out=tmp[:, :], in_=src_ap)
                            nc.vector.tensor_scalar(
                                mlp_sb[m][:, sl], tmp[:, :],
                                b4hh_sb[:, m:m + 1], 0.0, ALU.add, ALU.add)
                    _stats_t8(nc, phD, pheps, mlp_sb, t8,
                              ar4q_in[t8 // 2], ones_bf, slot=t8 % 2)
                    if t8 % 2 == 1:
                        qq = t8 // 2
                        nc.gpsimd.collective_compute(
                            "AllReduce", ALU.add, replica_groups=RG,
                            ins=[ar4q_in[qq][:, :].opt()],
                            outs=[ar4q_out[qq][:, :].opt()])

                def ln4_apply(qq):
                    ab4 = _ln_rows_batch(nc, phD, ar4q_out[qq],
                                         f"ln4q{qq}", nrows=2)
                    for t8 in range(2 * qq, 2 * qq + 2):
                        a_b, b2_b = _ln_bcast(nc, phD, ab4, t8 % 2)
                        for m in range(2):
                            sl = slice(t8 * TC, (t8 + 1) * TC)
                            t1 = phD.tile([P, TC], F32, tag="t1", name="t1")
                            t2 = phD.tile([P, TC], F32, tag="t2", name="t2")
                            nc.vector.tensor_mul(t1[:, :], mlp_sb[m][:, sl],
                                                 a_b[:, :])
                            nc.vector.tensor_add(t1[:, :], t1[:, :],
                                                 b2_b[:, :])
                            nc.vector.tensor_scalar(t2[:, :], t1[:, :],
                                                    ln4w_sb[:, m:m + 1],
                                                    ln4b_sb[:, m:m + 1],
                                                    ALU.mult, ALU.add)
                            ot = phD.tile([P, TC], F32, tag="ot", name="ot")
                            nc.vector.tensor_add(ot[:, :], t2[:, :],
                                                 ln_in[m][:, sl])
                            nc.sync.dma_start(
                                out=out_ext[m * P:(m + 1) * P,
                                            t8 * TC:(t8 + 1) * TC],
                                in_=ot[:, :])

                for t8 in range(NTC):
                    h4h_chunk(t8)
                    if t8 == 0:
                        ln3_pair(2, pheps, "st_s", "st_q", 1)
                    if t8 == 1:
                        dense_consume(7, pheps, "st_s", "st_q", 1)
                        ln2_pair(2)
                    if t8 == 2:
                        ln3_pair(3, pheps, "st_s", "st_q", 1)
                    if t8 == 3:
                        ln2_pair(3)
                    if t8 >= 1:
                        fourhh_partial(t8 - 1)
                    if t8 >= 2:
                        mlp_consume(t8 - 2)
                    if t8 == 5:
                        ln4_apply(0)
                    if t8 == 7:
                        ln4_apply(1)
                fourhh_partial(NTC - 1)
                mlp_consume(6)
                ln4_apply(2)
                mlp_consume(7)
                ln4_apply(3)
            phD_cm.__exit__(None, None, None)
            whp_cm.__exit__(None, None, None)

    nc.compile()
    return nc


def _stats_t8(nc, pool, pspool, rows, t8, ar_in, ones_bf,
              stag="st_s", qtag="st_q", sbufs=1, slot=None):
    """Sum & sumsq over the 256 local features of token-chunk t8 (bf16)."""
    if slot is None:
        slot = t8
    ps_s = pspool.tile([1, TC], F32, tag=stag, bufs=sbufs, name="ps_s")
    ps_q = pspool.tile([1, TC], F32, tag=qtag, bufs=sbufs, name="ps_q")
    sl = slice(t8 * TC, (t8 + 1) * TC)
    osl = slice(slot * TC, (slot + 1) * TC)
    for m in range(2):
        nc.tensor.matmul(ps_s[:, :], ones_bf[:, 0:1], rows[m][:, sl],
                         start=(m == 0), stop=(m == 1))
    for m in range(2):
        sq = pool.tile([P, TC], BF16, tag="sq", bufs=2, name="sq")
        nc.vector.tensor_mul(sq[:, :], rows[m][:, sl], rows[m][:, sl])
        nc.tensor.matmul(ps_q[:, :], ones_bf[:, 0:1], sq[:, :],
                         start=(m == 0), stop=(m == 1))
    tmp_s = pool.tile([1, TC], F32, tag="tmp_s", bufs=1, name="tmp_s")
    tmp_q = pool.tile([1, TC], F32, tag="tmp_q", bufs=1, name="tmp_q")
    nc.vector.tensor_copy(tmp_s[:, :], ps_s[:, :])
    nc.vector.tensor_copy(tmp_q[:, :], ps_q[:, :])
    nc.sync.dma_start(out=ar_in[0:1, osl], in_=tmp_s[:, :])
    nc.sync.dma_start(out=ar_in[1:2, osl], in_=tmp_q[:, :])


def _ln_rows_batch(nc, pool, ar_out, name, nrows=8):
    """Batched LN row math on [nrows,TC] tiles, one reciprocal total."""
    s8 = pool.tile([nrows, TC], F32, tag="lnrb_s8", bufs=1, name=f"{name}_s8")
    q8 = pool.tile([nrows, TC], F32, tag="lnrb_q8", bufs=1, name=f"{name}_q8")
    nc.sync.dma_start(out=s8[:, :], in_=ar_out[0:1, :])
    nc.sync.dma_start(out=q8[:, :], in_=ar_out[1:2, :])
    mu = pool.tile([nrows, TC], F32, tag="lnrb_mu", bufs=1, name=f"{name}_mu")
    a8 = pool.tile([nrows, TC], F32, tag="lnrb_a8", bufs=1, name=f"{name}_a8")
    b28 = pool.tile([nrows, TC], F32, tag="lnrb_b28", bufs=1,
                    name=f"{name}_b28")
    nc.vector.tensor_scalar_mul(mu[:, :], s8[:, :], 1.0 / H)
    nc.vector.tensor_scalar_mul(q8[:, :], q8[:, :], 1.0 / H)
    nc.vector.tensor_mul(b28[:, :], mu[:, :], mu[:, :])
    nc.vector.tensor_sub(q8[:, :], q8[:, :], b28[:, :])
    nc.scalar.activation(q8[:, :], q8[:, :], AF.Sqrt, bias=EPS)
    nc.vector.reciprocal(a8[:, :], q8[:, :])
    nc.vector.tensor_mul(b28[:, :], mu[:, :], a8[:, :])
    nc.vector.tensor_scalar_mul(b28[:, :], b28[:, :], -1.0)
    return a8, b28


def _ln_bcast(nc, pool, ab, t8):
    """Extract row t8 from the batched (a8,b28) and broadcast to [P,TC]."""
    a8, b28 = ab
    a_row = pool.tile([1, TC], F32, tag="a_row", name="a_row")
    b2_row = pool.tile([1, TC], F32, tag="b2_row", name="b2_row")
    nc.sync.dma_start(out=a_row[:, :], in_=a8[t8:t8 + 1, :])
    nc.sync.dma_start(out=b2_row[:, :], in_=b28[t8:t8 + 1, :])
    a_b = pool.tile([P, TC], F32, tag="a_b", name="a_b")
    b2_b = pool.tile([P, TC], F32, tag="b2_b", name="b2_b")
    nc.gpsimd.partition_broadcast(a_b[:, :], a_row[:, :])
    nc.gpsimd.partition_broadcast(b2_b[:, :], b2_row[:, :])
    return a_b, b2_b


# ----------------------------------------------------------------------
_cache = {}


def _get_program(mask_np, zero_bv, zero_bias):
    key = (mask_np.tobytes(), zero_bv, zero_bias)
    kh = hash(key)
    if kh not in _cache:
        _cache[kh] = build_program(_causal_block_status(mask_np), zero_bv,
                                   zero_bias)
    return _cache[kh]


def kernel(hidden_states, mask, ln1_w, ln1_b, w_qkv, b_qkv, w_dense, b_dense,
           ln3_w, ln3_b, ln2_w, ln2_b, w_h4h, b_h4h, w_4hh, b_4hh,
           ln4_w, ln4_b):
    hidden_states = np.asarray(hidden_states, np.float32)
    mask2d = np.asarray(mask, np.float32).reshape(S, S)
    w_qkv = np.asarray(w_qkv, np.float32)
    b_qkv = np.asarray(b_qkv, np.float32)
    w_dense = np.asarray(w_dense, np.float32)
    w_h4h = np.asarray(w_h4h, np.float32)
    w_4hh = np.asarray(w_4hh, np.float32)

    zero_bv = bool(np.all(b_qkv[2 * H:] == 0.0))
    zero_bias = bool(np.all(b_qkv[:2 * H] == 0.0)
                     and np.all(np.asarray(b_dense) == 0.0)
                     and np.all(np.asarray(b_4hh) == 0.0))
    prog = _get_program(mask2d, zero_bv, zero_bias)

    hT = np.ascontiguousarray(hidden_states.reshape(T, H).T)
    maskT_bf = np.ascontiguousarray(mask2d.T).astype(bf16)

    in_maps = []
    for c in range(NC):
        qs = slice(c * DC, (c + 1) * DC)
        wq_c = np.concatenate([w_qkv[:, c * DC:(c + 1) * DC],
                               w_qkv[:, H + c * DC:H + (c + 1) * DC],
                               w_qkv[:, 2 * H + c * DC:2 * H + (c + 1) * DC]],
                              axis=1)
        b_qk_c = np.concatenate([b_qkv[c * DC:(c + 1) * DC],
                                 b_qkv[H + c * DC:H + (c + 1) * DC]])
        b_v_c = b_qkv[2 * H + c * DC:2 * H + (c + 1) * DC]
        im = {
            "h_ln1": np.ascontiguousarray(
                hT[:, c * TC:(c + 1) * TC]).astype(bf16),
            "h_res": np.ascontiguousarray(hT[qs, :]),
            "ln1_w": np.asarray(ln1_w, np.float32).reshape(H, 1),
            "ln1_b": np.asarray(ln1_b, np.float32).reshape(H, 1),
            "ln2_w": np.asarray(ln2_w, np.float32)[qs].reshape(DC, 1),
            "ln2_b": np.asarray(ln2_b, np.float32)[qs].reshape(DC, 1),
            "ln3_w": np.asarray(ln3_w, np.float32)[qs].reshape(DC, 1),
            "ln3_b": np.asarray(ln3_b, np.float32)[qs].reshape(DC, 1),
            "ln4_w": np.asarray(ln4_w, np.float32)[qs].reshape(DC, 1),
            "ln4_b": np.asarray(ln4_b, np.float32)[qs].reshape(DC, 1),
            "w_qkv": np.ascontiguousarray(wq_c).astype(bf16),
            "b_qk": np.ascontiguousarray(b_qk_c).reshape(2 * DC, 1),
            "b_v": np.ascontiguousarray(b_v_c).reshape(1, DC),
            "w_dense": np.ascontiguousarray(w_dense[qs, :]).astype(bf16),
            "b_dense": np.asarray(b_dense, np.float32)[qs].reshape(DC, 1),
            "w_h4h": np.ascontiguousarray(
                w_h4h[:, c * FC:(c + 1) * FC]).astype(bf16),
            "b_h4h": np.asarray(b_h4h, np.float32)[
                c * FC:(c + 1) * FC].reshape(FC, 1),
            "w_4hh": np.ascontiguousarray(
                w_4hh[c * FC:(c + 1) * FC, :]).astype(bf16),
            "b_4hh": np.asarray(b_4hh, np.float32)[qs].reshape(DC, 1),
            "maskT": maskT_bf,
        }
        in_maps.append(im)

    res = run_bass_kernel_spmd(prog, in_maps, core_ids=list(range(NC)))
    outT = np.concatenate([res.results[c]["out"] for c in range(NC)], axis=0)
    return np.ascontiguousarray(outT.T).reshape(B, S, H).astype(np.float32)


# revision 25
# speedup vs baseline: 1.0250x; 1.0059x over previous
"""Trainium2 8-core tensor-parallel transformer layer — v10.

On top of v9 (MLP 4hh row-parallel + per-chunk ReduceScatter):
- Dense (attention output) projection also row-parallel + per-chunk
  ReduceScatter: each core contracts its own 4 heads' ctx (straight from
  SBUF, no ctx AllGather / DRAM bounce) into a [H, TC] partial, RS'd
  down to the core's 256 resident features.
- Phase D (LN3 -> AR2 -> LN2 -> x2 AllGather) for token-half 0 is
  emitted *inside* the attention loop (split into LN3-part and
  LN2-part) so the x2 AG completes while attention for batch 3 is
  still on the tensor engine; half 1 is emitted right after the first
  h4h chunk of the MLP.
- Softmax mask-multiplies moved to GpSimd so early-emitted phase-D
  vector work cannot stall the attention pipeline.
- LN4 stat AllReduces quartered; LN4 applies interleaved into the MLP
  loop to shrink the tail.
"""

import os
import sys

sys.path.insert(0, "/opt/trn_rl_repo")
os.environ.setdefault("MYCRO_LOCAL_CACHE", "1")
os.environ.setdefault("JAX_PLATFORMS", "cpu,axon")

import numpy as np
import ml_dtypes

import concourse.bass as bass
import concourse.mybir as mybir
import concourse.tile as tile
from concourse import bacc
from concourse.bass_utils import run_bass_kernel_spmd

F32 = mybir.dt.float32
BF16 = mybir.dt.bfloat16
AF = mybir.ActivationFunctionType
ALU = mybir.AluOpType

P = 128
B, S, H, NH = 4, 1024, 2048, 32
HD = H // NH
T = B * S
NC = 8
HPC = NH // NC                 # 4 heads/core
DC = H // NC                   # 256
FC = 4 * H // NC               # 1024
F4 = 4 * H                     # 8192
TC = 512
NTC = T // TC                  # 8
NFC = H // P                   # 16
EPS = 1e-5
RG = [list(range(NC))]

bf16 = ml_dtypes.bfloat16


def _causal_block_status(mask2d):
    mt = mask2d.T
    status = {}
    patterns = {}   # fingerprint -> unique slot
    slot_of = {}    # (kt, qc) -> (unique slot, src block)
    for kt in range(S // P):
        for qc in range(S // TC):
            blk = mt[kt * P:(kt + 1) * P, qc * TC:(qc + 1) * TC]
            if np.all(blk == 0):
                status[(kt, qc)] = "skip"
            elif np.all(blk == 1):
                status[(kt, qc)] = "full"
            else:
                status[(kt, qc)] = "masked"
                fp = blk.astype(np.float32).tobytes()
                if fp not in patterns:
                    patterns[fp] = (len(patterns), (kt, qc))
                slot_of[(kt, qc)] = patterns[fp][0]
    uniq = [src for _, src in sorted(patterns.values())]
    return status, slot_of, uniq


def build_program(blockinfo, zero_bv=True, zero_bias=True):
    block_status, mask_slot, mask_uniq = blockinfo
    nc = bacc.Bacc("TRN2", target_bir_lowering=False, debug=False,
                   num_devices=NC)

    def register_const_ap(dtype, value):
        t = nc.alloc_sbuf_tensor(f"const-{dtype.name}-{value}", [128, 1], dtype)
        nc.gpsimd.memset(t.ap(), value)
        nc.const_aps.aps[(dtype, value)] = t.ap()

    register_const_ap(F32, EPS)
    register_const_ap(F32, float(1.0 / np.sqrt(HD)))
    nc.all_engine_barrier()

    # ---------------- DRAM I/O ----------------
    h_ln1 = nc.dram_tensor("h_ln1", [H, TC], BF16, kind="ExternalInput")
    h_res = nc.dram_tensor("h_res", [DC, T], F32, kind="ExternalInput")
    ln1_w = nc.dram_tensor("ln1_w", [H, 1], F32, kind="ExternalInput")
    ln1_b = nc.dram_tensor("ln1_b", [H, 1], F32, kind="ExternalInput")
    ln2_w = nc.dram_tensor("ln2_w", [DC, 1], F32, kind="ExternalInput")
    ln2_b = nc.dram_tensor("ln2_b", [DC, 1], F32, kind="ExternalInput")
    ln3_w = nc.dram_tensor("ln3_w", [DC, 1], F32, kind="ExternalInput")
    ln3_b = nc.dram_tensor("ln3_b", [DC, 1], F32, kind="ExternalInput")
    ln4_w = nc.dram_tensor("ln4_w", [DC, 1], F32, kind="ExternalInput")
    ln4_b = nc.dram_tensor("ln4_b", [DC, 1], F32, kind="ExternalInput")
    w_qkv = nc.dram_tensor("w_qkv", [H, 3 * DC], BF16, kind="ExternalInput")
    b_qk = nc.dram_tensor("b_qk", [2 * DC, 1], F32, kind="ExternalInput")
    b_v = nc.dram_tensor("b_v", [1, DC], F32, kind="ExternalInput")
    w_dense = nc.dram_tensor("w_dense", [DC, H], BF16, kind="ExternalInput")
    b_dense = nc.dram_tensor("b_dense", [DC, 1], F32, kind="ExternalInput")
    w_h4h = nc.dram_tensor("w_h4h", [H, FC], BF16, kind="ExternalInput")
    b_h4h = nc.dram_tensor("b_h4h", [FC, 1], F32, kind="ExternalInput")
    w_4hh = nc.dram_tensor("w_4hh", [FC, H], BF16, kind="ExternalInput")
    b_4hh = nc.dram_tensor("b_4hh", [DC, 1], F32, kind="ExternalInput")
    maskT = nc.dram_tensor("maskT", [S, S], BF16, kind="ExternalInput")
    out_ext = nc.dram_tensor("out", [DC, T], F32, kind="ExternalOutput")


    with tile.TileContext(nc) as tc:
        with tc.tile_pool(name="const", bufs=1) as const, \
             tc.tile_pool(name="resid", bufs=1) as resid, \
             tc.tile_pool(name="dram", bufs=1, space="DRAM") as dram:
            phbw_cm = tc.tile_pool(name="ph_b_w", bufs=1, side="right")
            phbw = phbw_cm.__enter__()
            wq_all = phbw.tile([P, NFC * 3 * DC], BF16, name="wq_all")

            # ---------- constants ----------
            ones_f = const.tile([P, 1], F32)
            nc.vector.memset(ones_f[:, :], 1.0)
            ones_bf = const.tile([P, 1], BF16)
            nc.vector.memset(ones_bf[:, :], 1.0)
            ones_rows_bf = const.tile([P, P], BF16)
            nc.vector.memset(ones_rows_bf[:, :], 1.0)

            ln1w_sb = const.tile([P, NFC], F32)
            ln1b_sb = const.tile([P, NFC], F32)

            cpack = const.tile([P, 28], F32)
            _cofs = [0]

            def load_cols(t, ncols=2):
                base = _cofs[0]
                _cofs[0] += ncols
                for m in range(ncols):
                    nc.sync.dma_start(out=cpack[:, base + m:base + m + 1],
                                      in_=t[m * P:(m + 1) * P, 0:1])
                return cpack[:, base:base + ncols]

            # ---------- residents ----------
            ln_in = [resid.tile([P, T], BF16, name=f"ln_in{m}")
                     for m in range(2)]
            attn_sb = [resid.tile([P, T], BF16, tag="colsA", bufs=2,
                                  name=f"attn_sb{m}") for m in range(2)]

            # ---------- DRAM bounces ----------
            ag_x1_in = [dram.tile([P, 4 * TC], BF16, name=f"agx1i{h}")
                        for h in range(4)]
            ag_x1_out = [dram.tile([NC * P, 4 * TC], BF16,
                                   addr_space="Shared", name=f"agx1o{h}")
                         for h in range(4)]
            # dense partials: per-t8 ReduceScatter bounces
            rs_d_in = [dram.tile([H, TC], BF16, name=f"rsdi{k}")
                       for k in range(NTC)]
            rs_d_out = [dram.tile([DC, TC], BF16, name=f"rsdo{k}")
                        for k in range(NTC)]
            # x2: 4 quarters, free = t8r*1024 + m*512
            ag_x2_in = [dram.tile([P, 2 * 1024], BF16, name=f"agx2i{h}")
                        for h in range(4)]
            ag_x2_out = [dram.tile([NC * P, 2 * 1024], BF16,
                                   addr_space="Shared", name=f"agx2o{h}")
                         for h in range(4)]
            # mlp 4hh partials: per-t8 ReduceScatter bounces
            rs_mlp_in = [dram.tile([H, TC], BF16, name=f"rsmi{k}")
                         for k in range(NTC)]
            rs_mlp_out = [dram.tile([DC, TC], BF16, name=f"rsmo{k}")
                          for k in range(NTC)]
            rs_m7_in = [dram.tile([H // 2, TC], BF16, name=f"rsm7i{i}")
                        for i in range(2)]
            rs_m7_out = [dram.tile([P, TC], BF16, name=f"rsm7o{i}")
                         for i in range(2)]
            ar3q_in = [dram.tile([2, 2 * TC], F32, name=f"ar3i{i}")
                       for i in range(4)]
            ar3q_out = [dram.tile([2, 2 * TC], F32, addr_space="Shared",
                                  name=f"ar3o{i}") for i in range(4)]
            ar2q_in = [dram.tile([2, 2 * TC], F32, name=f"ar2i{i}")
                       for i in range(4)]
            ar2q_out = [dram.tile([2, 2 * TC], F32, addr_space="Shared",
                                  name=f"ar2o{i}") for i in range(4)]
            ar4q_in = [dram.tile([2, 2 * TC], F32, name=f"ar4i{i}")
                       for i in range(4)]
            ar4q_out = [dram.tile([2, 2 * TC], F32, addr_space="Shared",
                                  name=f"ar4o{i}") for i in range(4)]

            warm_in = dram.tile([1, 64], BF16, name="warm_in")
            warm_out = dram.tile([NC, 64], BF16, addr_space="Shared",
                                 name="warm_out")
            warm_sb = const.tile([1, 64], BF16)
            nc.vector.memset(warm_sb[:, :], 0.0)
            nc.sync.dma_start(out=warm_in[:, :], in_=warm_sb[:, :])
            nc.gpsimd.collective_compute(
                "AllGather", ALU.bypass, replica_groups=RG,
                ins=[warm_in[:, :].opt()], outs=[warm_out[:, :].opt()])

            # =========================================================
            # Phase A: LN1 -> x1 (bf16) -> 2 half AllGathers
            # =========================================================
            with tc.tile_pool(name="ph_a", bufs=1) as pha, \
                 tc.tile_pool(name="ph_a_ps", bufs=2, space="PSUM") as phaps:
                h1 = [pha.tile([P, TC], BF16, name=f"h1_{fc}")
                      for fc in range(NFC)]
                for fc in range(NFC):
                    nc.sync.dma_start(out=h1[fc][:, :],
                                      in_=h_ln1[fc * P:(fc + 1) * P, :])
                for fc in range(NFC):
                    nc.sync.dma_start(out=ln1w_sb[:, fc:fc + 1],
                                      in_=ln1_w[fc * P:(fc + 1) * P, 0:1])
                    nc.sync.dma_start(out=ln1b_sb[:, fc:fc + 1],
                                      in_=ln1_b[fc * P:(fc + 1) * P, 0:1])
                for fc in range(NFC):
                    nc.sync.dma_start(
                        out=wq_all[:, fc * 3 * DC:(fc + 1) * 3 * DC],
                        in_=w_qkv[fc * P:(fc + 1) * P, :])
                ln2w_sb = load_cols(ln2_w)
                ln2b_sb = load_cols(ln2_b)
                ln3w_sb = load_cols(ln3_w)
                ln3b_sb = load_cols(ln3_b)
                ln4w_sb = load_cols(ln4_w)
                ln4b_sb = load_cols(ln4_b)
                bdense_sb = load_cols(b_dense)
                b4hh_sb = load_cols(b_4hh)
                bqk_sb = load_cols(b_qk, 4)
                bh4h_sb = load_cols(b_h4h, 8)
                if not zero_bv:
                    bv_row = const.tile([1, DC], F32)
                    nc.sync.dma_start(out=bv_row[:, :], in_=b_v[0:1, :])
                    bv_b = const.tile([P, DC], F32)
                    nc.gpsimd.partition_broadcast(bv_b[:, :], bv_row[:, :])
                if mask_uniq:
                    mask_sb = const.tile([P, len(mask_uniq) * TC], BF16)
                    for i, (kt, qc) in enumerate(mask_uniq):
                        nc.sync.dma_start(
                            out=mask_sb[:, i * TC:(i + 1) * TC],
                            in_=maskT[kt * P:(kt + 1) * P,
                                      qc * TC:(qc + 1) * TC])
                ps_s = phaps.tile([1, TC], F32, name="ps_s")
                ps_q = phaps.tile([1, TC], F32, name="ps_q")
                for fc in range(NFC):
                    nc.tensor.matmul(ps_s[:, :], ones_bf[:, 0:1],
                                     h1[fc][:, :],
                                     start=(fc == 0), stop=(fc == NFC - 1))
                    sq = pha.tile([P, TC], BF16, tag="sq", bufs=3, name="sq")
                    nc.vector.tensor_mul(sq[:, :], h1[fc][:, :],
                                         h1[fc][:, :])
                    nc.tensor.matmul(ps_q[:, :], ones_bf[:, 0:1], sq[:, :],
                                     start=(fc == 0), stop=(fc == NFC - 1))
                mu = pha.tile([1, TC], F32)
                m2 = pha.tile([1, TC], F32)
                var = pha.tile([1, TC], F32)
                sd = pha.tile([1, TC], F32)
                a_row = pha.tile([1, TC], F32)
                b2_row = pha.tile([1, TC], F32)
                nc.vector.tensor_scalar_mul(mu[:, :], ps_s[:, :], 1.0 / H)
                nc.vector.tensor_scalar_mul(m2[:, :], ps_q[:, :], 1.0 / H)
                nc.vector.tensor_mul(var[:, :], mu[:, :], mu[:, :])
                nc.vector.tensor_sub(var[:, :], m2[:, :], var[:, :])
                nc.scalar.activation(sd[:, :], var[:, :], AF.Sqrt, bias=EPS)
                nc.vector.reciprocal(a_row[:, :], sd[:, :])
                nc.vector.tensor_mul(b2_row[:, :], mu[:, :], a_row[:, :])
                nc.vector.tensor_scalar_mul(b2_row[:, :], b2_row[:, :], -1.0)
                a_b = pha.tile([P, TC], F32)
                b2_b = pha.tile([P, TC], F32)
                nc.gpsimd.partition_broadcast(a_b[:, :], a_row[:, :])
                nc.gpsimd.partition_broadcast(b2_b[:, :], b2_row[:, :])
                x1h = [pha.tile([P, 4 * TC], BF16, name=f"x1h{h}")
                       for h in range(4)]
                for fc in range(NFC):
                    t1 = pha.tile([P, TC], F32, tag="t1", bufs=3, name="t1")
                    nc.vector.tensor_mul(t1[:, :], h1[fc][:, :], a_b[:, :])
                    nc.vector.tensor_add(t1[:, :], t1[:, :], b2_b[:, :])
                    hh, fr = fc // 4, fc % 4
                    nc.vector.tensor_scalar(
                        x1h[hh][:, fr * TC:(fr + 1) * TC], t1[:, :],
                        ln1w_sb[:, fc:fc + 1], ln1b_sb[:, fc:fc + 1],
                        ALU.mult, ALU.add)
                    if fr == 3:
                        nc.sync.dma_start(out=ag_x1_in[hh][:, :],
                                          in_=x1h[hh][:, :])
                        nc.gpsimd.collective_compute(
                            "AllGather", ALU.bypass, replica_groups=RG,
                            ins=[ag_x1_in[hh][:, :].opt()],
                            outs=[ag_x1_out[hh][:, :].opt()])

            # =========================================================
            # Phase B: QKV (consumes x1 halves as they arrive)
            # =========================================================
            attn_res_cm = tc.tile_pool(name="attn_res", bufs=1)
            attn_res = attn_res_cm.__enter__()
            qT2 = attn_res.tile([P, 2 * T], BF16)
            kT2 = attn_res.tile([P, 2 * T], BF16)
            v_sb = attn_res.tile([P, (T // P) * DC], BF16)
            with tc.tile_pool(name="ph_b", bufs=2) as phb, \
                 tc.tile_pool(name="ph_b_ps", bufs=3, space="PSUM") as phbps:
                for t8 in range(NTC):
                    x1c = [phb.tile([P, 4 * TC], BF16, tag=f"x1c{q}",
                                    name=f"x1c{q}") for q in range(4)]
                    for q in range(4):
                        nc.sync.dma_start(
                            out=x1c[q][:, :],
                            in_=ag_x1_out[q][t8 * P:(t8 + 1) * P, :])
                    qk_ps = [phbps.tile([P, 2 * TC], F32, tag=f"qkp{i}",
                                        bufs=1, name=f"qk_ps{i}")
                             for i in range(2)]
                    v_ps = [phbps.tile([P, DC], F32, tag=f"vps{i}", bufs=1,
                                       name=f"v_ps{i}") for i in range(4)]
                    for qt in range(4):
                        for m in range(4):
                            for f in range(4):
                                fc = qt * 4 + f
                                nc.tensor.matmul(
                                    qk_ps[m // 2][:, (m % 2) * TC:
                                                  (m % 2 + 1) * TC],
                                    wq_all[:, fc * 3 * DC + m * P:
                                           fc * 3 * DC + (m + 1) * P],
                                    x1c[qt][:, f * TC:(f + 1) * TC],
                                    start=(fc == 0), stop=(fc == NFC - 1))
                        for tt in range(TC // P):
                            for f in range(4):
                                fc = qt * 4 + f
                                nc.tensor.matmul(
                                    v_ps[tt][:, :],
                                    x1c[qt][:, f * TC + tt * P:
                                            f * TC + (tt + 1) * P],
                                    wq_all[:, fc * 3 * DC + 2 * DC:
                                           fc * 3 * DC + 3 * DC],
                                    start=(fc == 0), stop=(fc == NFC - 1))
                    for m in range(4):
                        dst = qT2 if m < 2 else kT2
                        pair = m % 2
                        off = pair * T + t8 * TC
                        src_ap = qk_ps[m // 2][:, pair * TC:(pair + 1) * TC]
                        if zero_bias:
                            nc.scalar.activation(dst[:, off:off + TC],
                                                 src_ap, AF.Copy)
                        else:
                            nc.scalar.activation(dst[:, off:off + TC],
                                                 src_ap, AF.Identity,
                                                 bias=bqk_sb[:, m:m + 1])
                    voff = t8 * 4 * DC
                    for tt in range(TC // P):
                        if zero_bv:
                            nc.scalar.activation(
                                v_sb[:, voff + tt * DC:voff + (tt + 1) * DC],
                                v_ps[tt][:, :], AF.Copy)
                        else:
                            nc.vector.tensor_add(
                                v_sb[:, voff + tt * DC:voff + (tt + 1) * DC],
                                v_ps[tt][:, :], bv_b[:, :])

            # =========================================================
            # Phase C: attention + row-parallel dense partials + early
            # phase-D (LN3/AR2/LN2/x2-AG) for token-half 0.
            # PSUM: s(3) + ctx(2) + den(1) + dn(2) = 8 banks.
            # =========================================================
            phbw_cm.__exit__(None, None, None)
            ctx_cm = tc.tile_pool(name="ctx_pool", bufs=1)
            ctx_pool = ctx_cm.__enter__()
            ctxF = {}
            phdw_cm = tc.tile_pool(name="ph_d_w", bufs=1)
            phdw = phdw_cm.__enter__()
            wd_all = phdw.tile([P, 2 * H], BF16, name="wd_all")
            for p2 in range(2):
                nc.sync.dma_start(out=wd_all[:, p2 * H:(p2 + 1) * H],
                                  in_=w_dense[p2 * P:(p2 + 1) * P, :])
            # MLP h4h weights: load early (SBUF region free after QKV)
            whp_cm = tc.tile_pool(name="ph_wh", bufs=1, side="right")
            whp = whp_cm.__enter__()
            wh_all = whp.tile([P, NFC * FC], BF16, name="wh_all")
            # long-lived pool for dense evicts + phase-D/LN4 row work
            phD_cm = tc.tile_pool(name="ph_D", bufs=1, side="right")
            phD = phD_cm.__enter__()

            def dense_partial(t8, pspool):
                b_, qc_ = t8 // 2, t8 % 2
                for jj in range(NFC):
                    ps = pspool.tile([P, TC], F32, tag="dn", bufs=2,
                                     name="ps_dn")
                    for p2 in range(2):
                        nc.tensor.matmul(
                            ps[:, :],
                            wd_all[:, p2 * H + jj * P:
                                   p2 * H + (jj + 1) * P],
                            ctxF[b_][:, qc_ * 2 * TC + p2 * TC:
                                     qc_ * 2 * TC + (p2 + 1) * TC],
                            start=(p2 == 0), stop=(p2 == 1))
                    pt = phD.tile([P, TC], BF16, tag="dpt", bufs=3,
                                  name="dpt")
                    nc.scalar.activation(pt[:, :], ps[:, :], AF.Copy)
                    nc.sync.dma_start(
                        out=rs_d_in[t8][jj * P:(jj + 1) * P, :],
                        in_=pt[:, :])
                nc.gpsimd.collective_compute(
                    "ReduceScatter", ALU.add, replica_groups=RG,
                    ins=[rs_d_in[t8][:, :].opt()],
                    outs=[rs_d_out[t8][:, :].opt()])

            def dense_consume(t8, pspool, stag, qtag, sbufs):
                sl = slice(t8 * TC, (t8 + 1) * TC)
                for m in range(2):
                    if zero_bias:
                        nc.sync.dma_start(
                            out=attn_sb[m][:, sl],
                            in_=rs_d_out[t8][m * P:(m + 1) * P, :])
                    else:
                        tmp = phD.tile([P, TC], BF16, tag="rsb", bufs=2,
                                       name="rsb")
                        nc.sync.dma_start(
                            out=tmp[:, :],
                            in_=rs_d_out[t8][m * P:(m + 1) * P, :])
                        nc.vector.tensor_scalar(
                            attn_sb[m][:, sl], tmp[:, :],
                            bdense_sb[:, m:m + 1], 0.0, ALU.add, ALU.add)
                _stats_t8(nc, phD, pspool, attn_sb, t8,
                          ar3q_in[t8 // 2], ones_bf, stag=stag, qtag=qtag,
                          sbufs=sbufs, slot=t8 % 2)
                if t8 % 2 == 1:
                    p = t8 // 2
                    nc.gpsimd.collective_compute(
                        "AllReduce", ALU.add, replica_groups=RG,
                        ins=[ar3q_in[p][:, :].opt()],
                        outs=[ar3q_out[p][:, :].opt()])

            def ln3_pair(p, pspool, stag, qtag, sbufs):
                ab3 = _ln_rows_batch(nc, phD, ar3q_out[p], f"ln3p{p}",
                                     nrows=2)
                for t8 in range(2 * p, 2 * p + 2):
                    a_b, b2_b = _ln_bcast(nc, phD, ab3, t8 % 2)
                    for m in range(2):
                        hres = phD.tile([P, TC], F32, tag="hres",
                                        bufs=2, name="hres")
                        nc.sync.dma_start(
                            out=hres[:, :],
                            in_=h_res[m * P:(m + 1) * P,
                                      t8 * TC:(t8 + 1) * TC])
                        sl = slice(t8 * TC, (t8 + 1) * TC)
                        t1 = phD.tile([P, TC], F32, tag="t1", name="t1")
                        t2 = phD.tile([P, TC], F32, tag="t2", name="t2")
                        nc.vector.tensor_mul(t1[:, :], attn_sb[m][:, sl],
                                             a_b[:, :])
                        nc.vector.tensor_add(t1[:, :], t1[:, :], b2_b[:, :])
                        nc.vector.tensor_scalar(t2[:, :], t1[:, :],
                                                ln3w_sb[:, m:m + 1],
                                                ln3b_sb[:, m:m + 1],
                                                ALU.mult, ALU.add)
                        nc.vector.tensor_add(ln_in[m][:, sl], t2[:, :],
                                             hres[:, :])
                    _stats_t8(nc, phD, pspool, ln_in, t8, ar2q_in[p],
                              ones_bf, stag=stag, qtag=qtag, sbufs=sbufs,
                              slot=t8 % 2)
                nc.gpsimd.collective_compute(
                    "AllReduce", ALU.add, replica_groups=RG,
                    ins=[ar2q_in[p][:, :].opt()],
                    outs=[ar2q_out[p][:, :].opt()])

            def ln2_pair(p):
                ab2 = _ln_rows_batch(nc, phD, ar2q_out[p], f"ln2p{p}",
                                     nrows=2)
                for t8 in range(2 * p, 2 * p + 2):
                    a_b, b2_b = _ln_bcast(nc, phD, ab2, t8 % 2)
                    tq = t8 % 2
                    for m in range(2):
                        sl = slice(t8 * TC, (t8 + 1) * TC)
                        t1 = phD.tile([P, TC], F32, tag="t1", name="t1")
                        nc.vector.tensor_mul(t1[:, :], ln_in[m][:, sl],
                                             a_b[:, :])
                        nc.vector.tensor_add(t1[:, :], t1[:, :], b2_b[:, :])
                        x2q = phD.tile([P, TC], BF16, tag="x2q", bufs=2,
                                       name="x2q")
                        nc.vector.tensor_scalar(
                            x2q[:, :], t1[:, :], ln2w_sb[:, m:m + 1],
                            ln2b_sb[:, m:m + 1], ALU.mult, ALU.add)
                        nc.sync.dma_start(
                            out=ag_x2_in[p][:, tq * 1024 + m * TC:
                                            tq * 1024 + (m + 1) * TC],
                            in_=x2q[:, :])
                nc.gpsimd.collective_compute(
                    "AllGather", ALU.bypass, replica_groups=RG,
                    ins=[ag_x2_in[p][:, :].opt()],
                    outs=[ag_x2_out[p][:, :].opt()])

            with tc.tile_pool(name="ph_c", bufs=1) as phc, \
                 tc.tile_pool(name="ph_c_ps", bufs=1, space="PSUM") as phcps:
                for b in range(B):
                    ctxF[b] = ctx_pool.tile([P, 2 * S], BF16, tag="ctxF",
                                            bufs=2, name=f"ctxF{b}")
                    if b == 1:
                        for fc in range(NFC):
                            nc.sync.dma_start(
                                out=wh_all[:, fc * FC:(fc + 1) * FC],
                                in_=w_h4h[fc * P:(fc + 1) * P, :])
                    for qc in range(S // TC):
                        t8c = 2 * b + qc
                        ctx_ps = [phcps.tile([P, TC], F32, tag=f"ctx{p}",
                                             bufs=1, name=f"ctx_ps{p}")
                                  for p in range(2)]
                        den_ps = phcps.tile([P, TC], F32, tag="den",
                                            bufs=1, name="den_ps")
                        kts = [kt for kt in range(S // P)
                               if block_status[(kt, qc)] != "skip"]
                        nkt = len(kts)

                        def emit_scores(ki):
                            kt = kts[ki]
                            st = block_status[(kt, qc)]
                            es = []
                            for h in range(HPC):
                                pair, rho = h // 2, h % 2
                                ps_s = phcps.tile([P, TC], F32, tag="s",
                                                  bufs=3, name="ps_s")
                                qoff = pair * T + b * S + qc * TC
                                koff = pair * T + b * S + kts[ki] * P
                                nc.tensor.matmul(
                                    ps_s[:, :],
                                    kT2[rho * HD:(rho + 1) * HD,
                                        koff:koff + P],
                                    qT2[rho * HD:(rho + 1) * HD,
                                        qoff:qoff + TC],
                                    start=True, stop=True)
                                e = phc.tile([P, TC], BF16, tag="e", bufs=6,
                                             name="e")
                                nc.scalar.activation(e[:, :], ps_s[:, :],
                                                     AF.Exp,
                                                     scale=1.0 / np.sqrt(HD))
                                if st == "masked":
                                    i = mask_slot[(kt, qc)]
                                    nc.vector.tensor_mul(
                                        e[:, :], e[:, :],
                                        mask_sb[:, i * TC:(i + 1) * TC])
                                es.append(e)
                            return es

                        def emit_ctx(ki, es):
                            kt = kts[ki]
                            ttg = b * (S // P) + kt
                            for h in range(HPC):
                                pair, rho = h // 2, h % 2
                                nc.tensor.matmul(
                                    ctx_ps[pair][rho * HD:(rho + 1) * HD, :],
                                    v_sb[:, ttg * DC + h * HD:
                                         ttg * DC + (h + 1) * HD],
                                    es[h][:, :],
                                    start=(ki == 0), stop=(ki == nkt - 1))
                                nc.tensor.matmul(
                                    den_ps[32 * h:32 * h + 1, :],
                                    ones_bf[:, 0:1], es[h][:, :],
                                    start=(ki == 0), stop=(ki == nkt - 1),
                                    tile_position=(0, 32 * h))

                        prev = emit_scores(0)
                        for ki in range(1, nkt):
                            cur = emit_scores(ki)
                            emit_ctx(ki - 1, prev)
                            prev = cur
                        emit_ctx(nkt - 1, prev)
                        # batched reciprocal of all 4 head denominators
                        rd = phc.tile([P, TC], F32, tag="rd", bufs=1,
                                      name="rd")
                        rd_bf = phc.tile([P, TC], BF16, tag="rd_bf", bufs=1,
                                         name="rd_bf")
                        nc.vector.reciprocal(rd[:, :], den_ps[:, :])
                        nc.vector.tensor_copy(rd_bf[:, :], rd[:, :])
                        for h in range(HPC):
                            pair, rho = h // 2, h % 2
                            r32 = slice(32 * h, 32 * h + 1)
                            ps_b = phcps.tile([P, TC], F32, tag="s", bufs=3,
                                              name="ps_b")
                            nc.tensor.matmul(ps_b[:, :],
                                             ones_rows_bf[r32, :],
                                             rd_bf[r32, :], start=True,
                                             stop=True,
                                             tile_position=(32 * h, 0))
                            rd_b = phc.tile([P, TC], F32, tag="rd_b", bufs=1,
                                            name="rd_b")
                            nc.vector.tensor_copy(rd_b[:, :], ps_b[:, :])
                            off = qc * 2 * TC + pair * TC
                            hs = slice(rho * HD, (rho + 1) * HD)
                            nc.vector.tensor_mul(ctxF[b][hs, off:off + TC],
                                                 ctx_ps[pair][hs, :],
                                                 rd_b[hs, :])
                        if t8c == 5:
                            ln2_pair(0)
                        dense_partial(t8c, phcps)
                        if t8c >= 2:
                            dense_consume(t8c - 2, phcps, "s", "s", 3)
                        if t8c == 4:
                            ln3_pair(0, phcps, "s", "s", 3)
                        if t8c == 6:
                            ln3_pair(1, phcps, "s", "s", 3)
                        if t8c == 7:
                            ln2_pair(1)
                dense_consume(6, phcps, "s", "s", 3)
            phdw_cm.__exit__(None, None, None)
            ctx_cm.__exit__(None, None, None)
            attn_res_cm.__exit__(None, None, None)

            # =========================================================
            # Phase E+F: h4h+gelu -> 4hh row-parallel partials -> per-t8
            # ReduceScatter; LN4 applies interleaved.
            # PSUM: h(2) + f(2) + st(2) = 6 banks.
            # =========================================================
            with tc.tile_pool(name="ph_e_w", bufs=1) as phew, \
                 tc.tile_pool(name="ph_ef", bufs=1) as phef, \
                 tc.tile_pool(name="ph_ef_ps", bufs=1, space="PSUM") as pheps:
                w4_all = phew.tile([P, (FC // P) * H], BF16, name="w4_all")
                for j in range(FC // P):
                    nc.sync.dma_start(out=w4_all[:, j * H:(j + 1) * H],
                                      in_=w_4hh[j * P:(j + 1) * P, :])
                mlp_sb = [resid.tile([P, T], BF16, tag="colsM", bufs=2,
                                     name=f"mlp_sb{m}") for m in range(2)]

                inter_t = {}

                def h4h_chunk(t8):
                    hh, t8r = t8 // 2, t8 % 2
                    x2c_all = phef.tile([P, NFC * TC], BF16, tag="x2c",
                                        bufs=2, name="x2c_all")
                    for c8 in range(NC):
                        for m2 in range(2):
                            fc = c8 * 2 + m2
                            nc.sync.dma_start(
                                out=x2c_all[:, fc * TC:(fc + 1) * TC],
                                in_=ag_x2_out[hh][c8 * P:(c8 + 1) * P,
                                                  t8r * 1024 + m2 * TC:
                                                  t8r * 1024 + (m2 + 1) * TC])
                    inter = phef.tile([P, (FC // P) * TC], BF16, tag="inter",
                                      bufs=2, name="inter")
                    for g in range(4):
                        ps = [pheps.tile([P, TC], F32, tag=f"h{mi}", bufs=1,
                                         name=f"ps_h{mi}") for mi in range(2)]
                        for fc in range(NFC):
                            for mi in range(2):
                                m = g * 2 + mi
                                nc.tensor.matmul(
                                    ps[mi][:, :],
                                    wh_all[:, fc * FC + m * P:
                                           fc * FC + (m + 1) * P],
                                    x2c_all[:, fc * TC:(fc + 1) * TC],
                                    start=(fc == 0), stop=(fc == NFC - 1))
                        for mi in range(2):
                            m = g * 2 + mi
                            nc.scalar.activation(
                                inter[:, m * TC:(m + 1) * TC], ps[mi][:, :],
                                AF.Gelu_apprx_tanh,
                                bias=bh4h_sb[:, m:m + 1])
                    inter_t[t8] = inter

                def fourhh_partial(t8):
                    inter = inter_t.pop(t8)
                    split = (t8 == NTC - 1)
                    order = ([2 * i for i in range(NFC // 2)]
                             + [2 * i + 1 for i in range(NFC // 2)]
                             if split else range(NFC))
                    for jj in order:
                        ps = pheps.tile([P, TC], F32, tag="f", bufs=2,
                                        name="ps_f")
                        for j in range(FC // P):
                            nc.tensor.matmul(
                                ps[:, :],
                                w4_all[:, j * H + jj * P:
                                       j * H + (jj + 1) * P],
                                inter[:, j * TC:(j + 1) * TC],
                                start=(j == 0), stop=(j == FC // P - 1))
                        pt = phef.tile([P, TC], BF16, tag="pt", bufs=4,
                                       name="pt")
                        if jj % 2 == 0:
                            nc.vector.tensor_copy(pt[:, :], ps[:, :])
                        else:
                            nc.scalar.activation(pt[:, :], ps[:, :], AF.Copy)
                        if split:
                            nc.sync.dma_start(
                                out=rs_m7_in[jj % 2][(jj // 2) * P:
                                                    (jj // 2 + 1) * P, :],
                                in_=pt[:, :])
                            if jj == NFC - 2 or jj == NFC - 1:
                                h = jj % 2
                                nc.gpsimd.collective_compute(
                                    "ReduceScatter", ALU.add,
                                    replica_groups=RG,
                                    ins=[rs_m7_in[h][:, :].opt()],
                                    outs=[rs_m7_out[h][:, :].opt()])
                        else:
                            nc.sync.dma_start(
                                out=rs_mlp_in[t8][jj * P:(jj + 1) * P, :],
                                in_=pt[:, :])
                    if not split:
                        nc.gpsimd.collective_compute(
                            "ReduceScatter", ALU.add, replica_groups=RG,
                            ins=[rs_mlp_in[t8][:, :].opt()],
                            outs=[rs_mlp_out[t8][:, :].opt()])

                def mlp_consume(t8):
                    sl = slice(t8 * TC, (t8 + 1) * TC)
                    for m in range(2):
                        src_ap = (rs_m7_out[m][:, :] if t8 == NTC - 1
                                  else rs_mlp_out[t8][m * P:(m + 1) * P, :])
                        if zero_bias:
                            nc.sync.dma_start(
                                out=mlp_sb[m][:, sl], in_=src_ap)
                        else:
                            tmp = phef.tile([P, TC], BF16, tag="rsb", bufs=2,
                                            name="rsb")
                            nc.sync.dma_start(out=tmp[:, :], in_=src_ap)
                            nc.vector.tensor_scalar(
                                mlp_sb[m][:, sl], tmp[:, :],
                                b4hh_sb[:, m:m + 1], 0.0, ALU.add, ALU.add)
                    _stats_t8(nc, phD, pheps, mlp_sb, t8,
                              ar4q_in[t8 // 2], ones_bf, slot=t8 % 2)
                    if t8 % 2 == 1:
                        qq = t8 // 2
                        nc.gpsimd.collective_compute(
                            "AllReduce", ALU.add, replica_groups=RG,
                            ins=[ar4q_in[qq][:, :].opt()],
                            outs=[ar4q_out[qq][:, :].opt()])

                def ln4_apply(qq):
                    ab4 = _ln_rows_batch(nc, phD, ar4q_out[qq],
                                         f"ln4q{qq}", nrows=2)
                    for t8 in range(2 * qq, 2 * qq + 2):
                        a_b, b2_b = _ln_bcast(nc, phD, ab4, t8 % 2)
                        for m in range(2):
                            sl = slice(t8 * TC, (t8 + 1) * TC)
                            t1 = phD.tile([P, TC], F32, tag="t1", name="t1")
                            t2 = phD.tile([P, TC], F32, tag="t2", name="t2")
                            nc.vector.tensor_mul(t1[:, :], mlp_sb[m][:, sl],
                                                 a_b[:, :])
                            nc.vector.tensor_add(t1[:, :], t1[:, :],
                                                 b2_b[:, :])
                            nc.vector.tensor_scalar(t2[:, :], t1[:, :],
                                                    ln4w_sb[:, m:m + 1],
                                                    ln4b_sb[:, m:m + 1],
                                                    ALU.mult, ALU.add)
                            ot = phD.tile([P, TC], F32, tag="ot", name="ot")
                            nc.vector.tensor_add(ot[:, :], t2[:, :],
                                                 ln_in[m][:, sl])
                            nc.sync.dma_start(
                                out=out_ext[m * P:(m + 1) * P,
                                            t8 * TC:(t8 + 1) * TC],
                                in_=ot[:, :])

                for t8 in range(NTC):
                    h4h_chunk(t8)
                    if t8 == 0:
                        ln3_pair(2, pheps, "st_s", "st_q", 1)
                    if t8 == 1:
                        dense_consume(7, pheps, "st_s", "st_q", 1)
                        ln2_pair(2)
                    if t8 == 2:
                        ln3_pair(3, pheps, "st_s", "st_q", 1)
                    if t8 == 3:
                        ln2_pair(3)
                    if t8 >= 1:
                        fourhh_partial(t8 - 1)
                    if t8 >= 2:
                        mlp_consume(t8 - 2)
                    if t8 == 5:
                        ln4_apply(0)
                    if t8 == 7:
                        ln4_apply(1)
                fourhh_partial(NTC - 1)
                mlp_consume(6)
                ln4_apply(2)
                mlp_consume(7)
                ln4_apply(3)
            phD_cm.__exit__(None, None, None)
            whp_cm.__exit__(None, None, None)

    nc.compile()
    return nc


def _stats_t8(nc, pool, pspool, rows, t8, ar_in, ones_bf,
              stag="st_s", qtag="st_q", sbufs=1, slot=None):
    """Sum & sumsq over the 256 local features of token-chunk t8 (bf16)."""
    if slot is None:
        slot = t8
    ps_s = pspool.tile([1, TC], F32, tag=stag, bufs=sbufs, name="ps_s")
    ps_q = pspool.tile([1, TC], F32, tag=qtag, bufs=sbufs, name="ps_q")
    sl = slice(t8 * TC, (t8 + 1) * TC)
    osl = slice(slot * TC, (slot + 1) * TC)
    for m in range(2):
        nc.tensor.matmul(ps_s[:, :], ones_bf[:, 0:1], rows[m][:, sl],
                         start=(m == 0), stop=(m == 1))
    for m in range(2):
        sq = pool.tile([P, TC], BF16, tag="sq", bufs=2, name="sq")
        nc.vector.tensor_mul(sq[:, :], rows[m][:, sl], rows[m][:, sl])
        nc.tensor.matmul(ps_q[:, :], ones_bf[:, 0:1], sq[:, :],
                         start=(m == 0), stop=(m == 1))
    tmp_s = pool.tile([1, TC], F32, tag="tmp_s", bufs=1, name="tmp_s")
    tmp_q = pool.tile([1, TC], F32, tag="tmp_q", bufs=1, name="tmp_q")
    nc.vector.tensor_copy(tmp_s[:, :], ps_s[:, :])
    nc.vector.tensor_copy(tmp_q[:, :], ps_q[:, :])
    nc.sync.dma_start(out=ar_in[0:1, osl], in_=tmp_s[:, :])
    nc.sync.dma_start(out=ar_in[1:2, osl], in_=tmp_q[:, :])


def _ln_rows_batch(nc, pool, ar_out, name, nrows=8):
    """Batched LN row math on [nrows,TC] tiles, one reciprocal total."""
    s8 = pool.tile([nrows, TC], F32, tag="lnrb_s8", bufs=1, name=f"{name}_s8")
    q8 = pool.tile([nrows, TC], F32, tag="lnrb_q8", bufs=1, name=f"{name}_q8")
    nc.sync.dma_start(out=s8[:, :], in_=ar_out[0:1, :])
    nc.sync.dma_start(out=q8[:, :], in_=ar_out[1:2, :])
    mu = pool.tile([nrows, TC], F32, tag="lnrb_mu", bufs=1, name=f"{name}_mu")
    a8 = pool.tile([nrows, TC], F32, tag="lnrb_a8", bufs=1, name=f"{name}_a8")
    b28 = pool.tile([nrows, TC], F32, tag="lnrb_b28", bufs=1,
                    name=f"{name}_b28")
    nc.vector.tensor_scalar_mul(mu[:, :], s8[:, :], 1.0 / H)
    nc.vector.tensor_scalar_mul(q8[:, :], q8[:, :], 1.0 / H)
    nc.vector.tensor_mul(b28[:, :], mu[:, :], mu[:, :])
    nc.vector.tensor_sub(q8[:, :], q8[:, :], b28[:, :])
    nc.scalar.activation(q8[:, :], q8[:, :], AF.Sqrt, bias=EPS)
    nc.vector.reciprocal(a8[:, :], q8[:, :])
    nc.vector.tensor_mul(b28[:, :], mu[:, :], a8[:, :])
    nc.vector.tensor_scalar_mul(b28[:, :], b28[:, :], -1.0)
    return a8, b28


def _ln_bcast(nc, pool, ab, t8):
    """Extract row t8 from the batched (a8,b28) and broadcast to [P,TC]."""
    a8, b28 = ab
    a_row = pool.tile([1, TC], F32, tag="a_row", name="a_row")
    b2_row = pool.tile([1, TC], F32, tag="b2_row", name="b2_row")
    nc.sync.dma_start(out=a_row[:, :], in_=a8[t8:t8 + 1, :])
    nc.sync.dma_start(out=b2_row[:, :], in_=b28[t8:t8 + 1, :])
    a_b = pool.tile([P, TC], F32, tag="a_b", name="a_b")
    b2_b = pool.tile([P, TC], F32, tag="b2_b", name="b2_b")
    nc.gpsimd.partition_broadcast(a_b[:, :], a_row[:, :])
    nc.gpsimd.partition_broadcast(b2_b[:, :], b2_row[:, :])
    return a_b, b2_b


# ----------------------------------------------------------------------
_cache = {}


def _get_program(mask_np, zero_bv, zero_bias):
    key = (mask_np.tobytes(), zero_bv, zero_bias)
    kh = hash(key)
    if kh not in _cache:
        _cache[kh] = build_program(_causal_block_status(mask_np), zero_bv,
                                   zero_bias)
    return _cache[kh]


def kernel(hidden_states, mask, ln1_w, ln1_b, w_qkv, b_qkv, w_dense, b_dense,
           ln3_w, ln3_b, ln2_w, ln2_b, w_h4h, b_h4h, w_4hh, b_4hh,
           ln4_w, ln4_b):
    hidden_states = np.asarray(hidden_states, np.float32)
    mask2d = np.asarray(mask, np.float32).reshape(S, S)
    w_qkv = np.asarray(w_qkv, np.float32)
    b_qkv = np.asarray(b_qkv, np.float32)
    w_dense = np.asarray(w_dense, np.float32)
    w_h4h = np.asarray(w_h4h, np.float32)
    w_4hh = np.asarray(w_4hh, np.float32)

    zero_bv = bool(np.all(b_qkv[2 * H:] == 0.0))
    zero_bias = bool(np.all(b_qkv[:2 * H] == 0.0)
                     and np.all(np.asarray(b_dense) == 0.0)
                     and np.all(np.asarray(b_4hh) == 0.0))
    prog = _get_program(mask2d, zero_bv, zero_bias)

    hT = np.ascontiguousarray(hidden_states.reshape(T, H).T)
    maskT_bf = np.ascontiguousarray(mask2d.T).astype(bf16)

    in_maps = []
    for c in range(NC):
        qs = slice(c * DC, (c + 1) * DC)
        wq_c = np.concatenate([w_qkv[:, c * DC:(c + 1) * DC],
                               w_qkv[:, H + c * DC:H + (c + 1) * DC],
                               w_qkv[:, 2 * H + c * DC:2 * H + (c + 1) * DC]],
                              axis=1)
        b_qk_c = np.concatenate([b_qkv[c * DC:(c + 1) * DC],
                                 b_qkv[H + c * DC:H + (c + 1) * DC]])
        b_v_c = b_qkv[2 * H + c * DC:2 * H + (c + 1) * DC]
        im = {
            "h_ln1": np.ascontiguousarray(
                hT[:, c * TC:(c + 1) * TC]).astype(bf16),
            "h_res": np.ascontiguousarray(hT[qs, :]),
            "ln1_w": np.asarray(ln1_w, np.float32).reshape(H, 1),
            "ln1_b": np.asarray(ln1_b, np.float32).reshape(H, 1),
            "ln2_w": np.asarray(ln2_w, np.float32)[qs].reshape(DC, 1),
            "ln2_b": np.asarray(ln2_b, np.float32)[qs].reshape(DC, 1),
            "ln3_w": np.asarray(ln3_w, np.float32)[qs].reshape(DC, 1),
            "ln3_b": np.asarray(ln3_b, np.float32)[qs].reshape(DC, 1),
            "ln4_w": np.asarray(ln4_w, np.float32)[qs].reshape(DC, 1),
            "ln4_b": np.asarray(ln4_b, np.float32)[qs].reshape(DC, 1),
            "w_qkv": np.ascontiguousarray(wq_c).astype(bf16),
            "b_qk": np.ascontiguousarray(b_qk_c).reshape(2 * DC, 1),
            "b_v": np.ascontiguousarray(b_v_c).reshape(1, DC),
            "w_dense": np.ascontiguousarray(w_dense[qs, :]).astype(bf16),
            "b_dense": np.asarray(b_dense, np.float32)[qs].reshape(DC, 1),
            "w_h4h": np.ascontiguousarray(
                w_h4h[:, c * FC:(c + 1) * FC]).astype(bf16),
            "b_h4h": np.asarray(b_h4h, np.float32)[
                c * FC:(c + 1) * FC].reshape(FC, 1),
            "w_4hh": np.ascontiguousarray(
                w_4hh[c * FC:(c + 1) * FC, :]).astype(bf16),
            "b_4hh": np.asarray(b_4hh, np.float32)[qs].reshape(DC, 1),
            "maskT": maskT_bf,
        }
        in_maps.append(im)

    res = run_bass_kernel_spmd(prog, in_maps, core_ids=list(range(NC)))
    outT = np.concatenate([res.results[c]["out"] for c in range(NC)], axis=0)
    return np.ascontiguousarray(outT.T).reshape(B, S, H).astype(np.float32)


# revision 26
# speedup vs baseline: 1.0293x; 1.0042x over previous
"""Trainium2 8-core tensor-parallel transformer layer — v10.

On top of v9 (MLP 4hh row-parallel + per-chunk ReduceScatter):
- Dense (attention output) projection also row-parallel + per-chunk
  ReduceScatter: each core contracts its own 4 heads' ctx (straight from
  SBUF, no ctx AllGather / DRAM bounce) into a [H, TC] partial, RS'd
  down to the core's 256 resident features.
- Phase D (LN3 -> AR2 -> LN2 -> x2 AllGather) for token-half 0 is
  emitted *inside* the attention loop (split into LN3-part and
  LN2-part) so the x2 AG completes while attention for batch 3 is
  still on the tensor engine; half 1 is emitted right after the first
  h4h chunk of the MLP.
- Softmax mask-multiplies moved to GpSimd so early-emitted phase-D
  vector work cannot stall the attention pipeline.
- LN4 stat AllReduces quartered; LN4 applies interleaved into the MLP
  loop to shrink the tail.
"""

import os
import sys

sys.path.insert(0, "/opt/trn_rl_repo")
os.environ.setdefault("MYCRO_LOCAL_CACHE", "1")
os.environ.setdefault("JAX_PLATFORMS", "cpu,axon")

import numpy as np
import ml_dtypes

import concourse.bass as bass
import concourse.mybir as mybir
import concourse.tile as tile
from concourse import bacc
from concourse.bass_utils import run_bass_kernel_spmd

F32 = mybir.dt.float32
BF16 = mybir.dt.bfloat16
AF = mybir.ActivationFunctionType
ALU = mybir.AluOpType

P = 128
B, S, H, NH = 4, 1024, 2048, 32
HD = H // NH
T = B * S
NC = 8
HPC = NH // NC                 # 4 heads/core
DC = H // NC                   # 256
FC = 4 * H // NC               # 1024
F4 = 4 * H                     # 8192
TC = 512
NTC = T // TC                  # 8
NFC = H // P                   # 16
EPS = 1e-5
RG = [list(range(NC))]

bf16 = ml_dtypes.bfloat16


def _causal_block_status(mask2d):
    mt = mask2d.T
    status = {}
    patterns = {}   # fingerprint -> unique slot
    slot_of = {}    # (kt, qc) -> (unique slot, src block)
    for kt in range(S // P):
        for qc in range(S // TC):
            blk = mt[kt * P:(kt + 1) * P, qc * TC:(qc + 1) * TC]
            if np.all(blk == 0):
                status[(kt, qc)] = "skip"
            elif np.all(blk == 1):
                status[(kt, qc)] = "full"
            else:
                status[(kt, qc)] = "masked"
                fp = blk.astype(np.float32).tobytes()
                if fp not in patterns:
                    patterns[fp] = (len(patterns), (kt, qc))
                slot_of[(kt, qc)] = patterns[fp][0]
    uniq = [src for _, src in sorted(patterns.values())]
    return status, slot_of, uniq


def build_program(blockinfo, zero_bv=True, zero_bias=True):
    block_status, mask_slot, mask_uniq = blockinfo
    nc = bacc.Bacc("TRN2", target_bir_lowering=False, debug=False,
                   num_devices=NC)

    def register_const_ap(dtype, value):
        t = nc.alloc_sbuf_tensor(f"const-{dtype.name}-{value}", [128, 1], dtype)
        nc.gpsimd.memset(t.ap(), value)
        nc.const_aps.aps[(dtype, value)] = t.ap()

    register_const_ap(F32, EPS)
    register_const_ap(F32, float(1.0 / np.sqrt(HD)))
    nc.all_engine_barrier()

    # ---------------- DRAM I/O ----------------
    h_ln1 = nc.dram_tensor("h_ln1", [H, TC], BF16, kind="ExternalInput")
    h_res = nc.dram_tensor("h_res", [DC, T], F32, kind="ExternalInput")
    ln1_w = nc.dram_tensor("ln1_w", [H, 1], F32, kind="ExternalInput")
    ln1_b = nc.dram_tensor("ln1_b", [H, 1], F32, kind="ExternalInput")
    ln2_w = nc.dram_tensor("ln2_w", [DC, 1], F32, kind="ExternalInput")
    ln2_b = nc.dram_tensor("ln2_b", [DC, 1], F32, kind="ExternalInput")
    ln3_w = nc.dram_tensor("ln3_w", [DC, 1], F32, kind="ExternalInput")
    ln3_b = nc.dram_tensor("ln3_b", [DC, 1], F32, kind="ExternalInput")
    ln4_w = nc.dram_tensor("ln4_w", [DC, 1], F32, kind="ExternalInput")
    ln4_b = nc.dram_tensor("ln4_b", [DC, 1], F32, kind="ExternalInput")
    w_qkv = nc.dram_tensor("w_qkv", [H, 3 * DC], BF16, kind="ExternalInput")
    b_qk = nc.dram_tensor("b_qk", [2 * DC, 1], F32, kind="ExternalInput")
    b_v = nc.dram_tensor("b_v", [1, DC], F32, kind="ExternalInput")
    w_dense = nc.dram_tensor("w_dense", [DC, H], BF16, kind="ExternalInput")
    b_dense = nc.dram_tensor("b_dense", [DC, 1], F32, kind="ExternalInput")
    w_h4h = nc.dram_tensor("w_h4h", [H, FC], BF16, kind="ExternalInput")
    b_h4h = nc.dram_tensor("b_h4h", [FC, 1], F32, kind="ExternalInput")
    w_4hh = nc.dram_tensor("w_4hh", [FC, H], BF16, kind="ExternalInput")
    b_4hh = nc.dram_tensor("b_4hh", [DC, 1], F32, kind="ExternalInput")
    maskT = nc.dram_tensor("maskT", [S, S], BF16, kind="ExternalInput")
    out_ext = nc.dram_tensor("out", [DC, T], F32, kind="ExternalOutput")


    with tile.TileContext(nc) as tc:
        with tc.tile_pool(name="const", bufs=1) as const, \
             tc.tile_pool(name="resid", bufs=1) as resid, \
             tc.tile_pool(name="dram", bufs=1, space="DRAM") as dram:
            phbw_cm = tc.tile_pool(name="ph_b_w", bufs=1, side="right")
            phbw = phbw_cm.__enter__()
            wq_all = phbw.tile([P, NFC * 3 * DC], BF16, name="wq_all")

            # ---------- constants ----------
            ones_f = const.tile([P, 1], F32)
            nc.vector.memset(ones_f[:, :], 1.0)
            ones_bf = const.tile([P, 1], BF16)
            nc.vector.memset(ones_bf[:, :], 1.0)
            ones_rows_bf = const.tile([P, P], BF16)
            nc.vector.memset(ones_rows_bf[:, :], 1.0)

            ln1w_sb = const.tile([P, NFC], F32)
            ln1b_sb = const.tile([P, NFC], F32)

            cpack = const.tile([P, 28], F32)
            _cofs = [0]

            def load_cols(t, ncols=2):
                base = _cofs[0]
                _cofs[0] += ncols
                for m in range(ncols):
                    nc.sync.dma_start(out=cpack[:, base + m:base + m + 1],
                                      in_=t[m * P:(m + 1) * P, 0:1])
                return cpack[:, base:base + ncols]

            # ---------- residents ----------
            ln_in = [resid.tile([P, T], BF16, name=f"ln_in{m}")
                     for m in range(2)]
            attn_sb = [resid.tile([P, T], BF16, tag="colsA", bufs=2,
                                  name=f"attn_sb{m}") for m in range(2)]

            # ---------- DRAM bounces ----------
            ag_x1_in = [dram.tile([P, 4 * TC], BF16, name=f"agx1i{h}")
                        for h in range(4)]
            ag_x1_out = [dram.tile([NC * P, 4 * TC], BF16,
                                   addr_space="Shared", name=f"agx1o{h}")
                         for h in range(4)]
            # dense partials: per-t8 ReduceScatter bounces
            rs_d_in = [dram.tile([H, TC], BF16, name=f"rsdi{k}")
                       for k in range(NTC)]
            rs_d_out = [dram.tile([DC, TC], BF16, name=f"rsdo{k}")
                        for k in range(NTC)]
            # x2: 4 quarters, free = t8r*1024 + m*512
            ag_x2_in = [dram.tile([P, 2 * 1024], BF16, name=f"agx2i{h}")
                        for h in range(4)]
            ag_x2_out = [dram.tile([NC * P, 2 * 1024], BF16,
                                   addr_space="Shared", name=f"agx2o{h}")
                         for h in range(4)]
            # mlp 4hh partials: per-t8 ReduceScatter bounces
            rs_mlp_in = [dram.tile([H, TC], BF16, name=f"rsmi{k}")
                         for k in range(NTC)]
            rs_mlp_out = [dram.tile([DC, TC], BF16, name=f"rsmo{k}")
                          for k in range(NTC)]
            rs_m7_in = [dram.tile([H // 2, TC], BF16, name=f"rsm7i{i}")
                        for i in range(2)]
            rs_m7_out = [dram.tile([P, TC], BF16, name=f"rsm7o{i}")
                         for i in range(2)]
            ar3q_in = [dram.tile([2, 2 * TC], F32, name=f"ar3i{i}")
                       for i in range(4)]
            ar3q_out = [dram.tile([2, 2 * TC], F32, addr_space="Shared",
                                  name=f"ar3o{i}") for i in range(4)]
            ar2q_in = [dram.tile([2, 2 * TC], F32, name=f"ar2i{i}")
                       for i in range(4)]
            ar2q_out = [dram.tile([2, 2 * TC], F32, addr_space="Shared",
                                  name=f"ar2o{i}") for i in range(4)]
            ar4q_in = [dram.tile([2, 2 * TC], F32, name=f"ar4i{i}")
                       for i in range(4)]
            ar4q_out = [dram.tile([2, 2 * TC], F32, addr_space="Shared",
                                  name=f"ar4o{i}") for i in range(4)]

            warm_in = dram.tile([1, 64], BF16, name="warm_in")
            warm_out = dram.tile([NC, 64], BF16, addr_space="Shared",
                                 name="warm_out")
            warm_sb = const.tile([1, 64], BF16)
            nc.vector.memset(warm_sb[:, :], 0.0)
            nc.sync.dma_start(out=warm_in[:, :], in_=warm_sb[:, :])
            nc.gpsimd.collective_compute(
                "AllGather", ALU.bypass, replica_groups=RG,
                ins=[warm_in[:, :].opt()], outs=[warm_out[:, :].opt()])

            # =========================================================
            # Phase A: LN1 -> x1 (bf16) -> 2 half AllGathers
            # =========================================================
            with tc.tile_pool(name="ph_a", bufs=1) as pha, \
                 tc.tile_pool(name="ph_a_ps", bufs=2, space="PSUM") as phaps:
                h1 = [pha.tile([P, TC], BF16, name=f"h1_{fc}")
                      for fc in range(NFC)]
                for fc in range(NFC):
                    nc.sync.dma_start(out=h1[fc][:, :],
                                      in_=h_ln1[fc * P:(fc + 1) * P, :])
                for fc in range(NFC):
                    nc.sync.dma_start(out=ln1w_sb[:, fc:fc + 1],
                                      in_=ln1_w[fc * P:(fc + 1) * P, 0:1])
                    nc.sync.dma_start(out=ln1b_sb[:, fc:fc + 1],
                                      in_=ln1_b[fc * P:(fc + 1) * P, 0:1])
                for fc in range(NFC):
                    nc.sync.dma_start(
                        out=wq_all[:, fc * 3 * DC:(fc + 1) * 3 * DC],
                        in_=w_qkv[fc * P:(fc + 1) * P, :])
                ln2w_sb = load_cols(ln2_w)
                ln2b_sb = load_cols(ln2_b)
                ln3w_sb = load_cols(ln3_w)
                ln3b_sb = load_cols(ln3_b)
                ln4w_sb = load_cols(ln4_w)
                ln4b_sb = load_cols(ln4_b)
                bdense_sb = load_cols(b_dense)
                b4hh_sb = load_cols(b_4hh)
                bqk_sb = load_cols(b_qk, 4)
                bh4h_sb = load_cols(b_h4h, 8)
                if not zero_bv:
                    bv_row = const.tile([1, DC], F32)
                    nc.sync.dma_start(out=bv_row[:, :], in_=b_v[0:1, :])
                    bv_b = const.tile([P, DC], F32)
                    nc.gpsimd.partition_broadcast(bv_b[:, :], bv_row[:, :])
                if mask_uniq:
                    mask_sb = const.tile([P, len(mask_uniq) * TC], BF16)
                    for i, (kt, qc) in enumerate(mask_uniq):
                        nc.sync.dma_start(
                            out=mask_sb[:, i * TC:(i + 1) * TC],
                            in_=maskT[kt * P:(kt + 1) * P,
                                      qc * TC:(qc + 1) * TC])
                ps_s = phaps.tile([1, TC], F32, name="ps_s")
                ps_q = phaps.tile([1, TC], F32, name="ps_q")
                for fc in range(NFC):
                    nc.tensor.matmul(ps_s[:, :], ones_bf[:, 0:1],
                                     h1[fc][:, :],
                                     start=(fc == 0), stop=(fc == NFC - 1))
                    sq = pha.tile([P, TC], BF16, tag="sq", bufs=3, name="sq")
                    nc.vector.tensor_mul(sq[:, :], h1[fc][:, :],
                                         h1[fc][:, :])
                    nc.tensor.matmul(ps_q[:, :], ones_bf[:, 0:1], sq[:, :],
                                     start=(fc == 0), stop=(fc == NFC - 1))
                mu = pha.tile([1, TC], F32)
                m2 = pha.tile([1, TC], F32)
                var = pha.tile([1, TC], F32)
                sd = pha.tile([1, TC], F32)
                a_row = pha.tile([1, TC], F32)
                b2_row = pha.tile([1, TC], F32)
                nc.vector.tensor_scalar_mul(mu[:, :], ps_s[:, :], 1.0 / H)
                nc.vector.tensor_scalar_mul(m2[:, :], ps_q[:, :], 1.0 / H)
                nc.vector.tensor_mul(var[:, :], mu[:, :], mu[:, :])
                nc.vector.tensor_sub(var[:, :], m2[:, :], var[:, :])
                nc.scalar.activation(sd[:, :], var[:, :], AF.Sqrt, bias=EPS)
                nc.vector.reciprocal(a_row[:, :], sd[:, :])
                nc.vector.tensor_mul(b2_row[:, :], mu[:, :], a_row[:, :])
                nc.vector.tensor_scalar_mul(b2_row[:, :], b2_row[:, :], -1.0)
                a_b = pha.tile([P, TC], F32)
                b2_b = pha.tile([P, TC], F32)
                nc.gpsimd.partition_broadcast(a_b[:, :], a_row[:, :])
                nc.gpsimd.partition_broadcast(b2_b[:, :], b2_row[:, :])
                x1h = [pha.tile([P, 4 * TC], BF16, name=f"x1h{h}")
                       for h in range(4)]
                for fc in range(NFC):
                    t1 = pha.tile([P, TC], F32, tag="t1", bufs=3, name="t1")
                    nc.vector.tensor_mul(t1[:, :], h1[fc][:, :], a_b[:, :])
                    nc.vector.tensor_add(t1[:, :], t1[:, :], b2_b[:, :])
                    hh, fr = fc // 4, fc % 4
                    nc.vector.tensor_scalar(
                        x1h[hh][:, fr * TC:(fr + 1) * TC], t1[:, :],
                        ln1w_sb[:, fc:fc + 1], ln1b_sb[:, fc:fc + 1],
                        ALU.mult, ALU.add)
                    if fr == 3:
                        nc.sync.dma_start(out=ag_x1_in[hh][:, :],
                                          in_=x1h[hh][:, :])
                        nc.gpsimd.collective_compute(
                            "AllGather", ALU.bypass, replica_groups=RG,
                            ins=[ag_x1_in[hh][:, :].opt()],
                            outs=[ag_x1_out[hh][:, :].opt()])

            # =========================================================
            # Phase B: QKV (consumes x1 halves as they arrive)
            # =========================================================
            attn_res_cm = tc.tile_pool(name="attn_res", bufs=1)
            attn_res = attn_res_cm.__enter__()
            qT2 = attn_res.tile([P, 2 * T], BF16)
            kT2 = attn_res.tile([P, 2 * T], BF16)
            v_sb = attn_res.tile([P, (T // P) * DC], BF16)
            with tc.tile_pool(name="ph_b", bufs=2) as phb, \
                 tc.tile_pool(name="ph_b_ps", bufs=3, space="PSUM") as phbps:
                for t8 in range(NTC):
                    x1c = [phb.tile([P, 4 * TC], BF16, tag=f"x1c{q}",
                                    name=f"x1c{q}") for q in range(4)]
                    for q in range(4):
                        nc.sync.dma_start(
                            out=x1c[q][:, :],
                            in_=ag_x1_out[q][t8 * P:(t8 + 1) * P, :])
                    qk_ps = [phbps.tile([P, 2 * TC], F32, tag=f"qkp{i}",
                                        bufs=1, name=f"qk_ps{i}")
                             for i in range(2)]
                    v_ps = [phbps.tile([P, DC], F32, tag=f"vps{i}", bufs=1,
                                       name=f"v_ps{i}") for i in range(4)]
                    for qt in range(4):
                        for m in range(4):
                            for f in range(4):
                                fc = qt * 4 + f
                                nc.tensor.matmul(
                                    qk_ps[m // 2][:, (m % 2) * TC:
                                                  (m % 2 + 1) * TC],
                                    wq_all[:, fc * 3 * DC + m * P:
                                           fc * 3 * DC + (m + 1) * P],
                                    x1c[qt][:, f * TC:(f + 1) * TC],
                                    start=(fc == 0), stop=(fc == NFC - 1))
                        for tt in range(TC // P):
                            for f in range(4):
                                fc = qt * 4 + f
                                nc.tensor.matmul(
                                    v_ps[tt][:, :],
                                    x1c[qt][:, f * TC + tt * P:
                                            f * TC + (tt + 1) * P],
                                    wq_all[:, fc * 3 * DC + 2 * DC:
                                           fc * 3 * DC + 3 * DC],
                                    start=(fc == 0), stop=(fc == NFC - 1))
                    for m in range(4):
                        dst = qT2 if m < 2 else kT2
                        pair = m % 2
                        off = pair * T + t8 * TC
                        src_ap = qk_ps[m // 2][:, pair * TC:(pair + 1) * TC]
                        if zero_bias:
                            nc.scalar.activation(dst[:, off:off + TC],
                                                 src_ap, AF.Copy)
                        else:
                            nc.scalar.activation(dst[:, off:off + TC],
                                                 src_ap, AF.Identity,
                                                 bias=bqk_sb[:, m:m + 1])
                    voff = t8 * 4 * DC
                    for tt in range(TC // P):
                        if zero_bv:
                            nc.scalar.activation(
                                v_sb[:, voff + tt * DC:voff + (tt + 1) * DC],
                                v_ps[tt][:, :], AF.Copy)
                        else:
                            nc.vector.tensor_add(
                                v_sb[:, voff + tt * DC:voff + (tt + 1) * DC],
                                v_ps[tt][:, :], bv_b[:, :])

            # =========================================================
            # Phase C: attention + row-parallel dense partials + early
            # phase-D (LN3/AR2/LN2/x2-AG) for token-half 0.
            # PSUM: s(3) + ctx(2) + den(1) + dn(2) = 8 banks.
            # =========================================================
            phbw_cm.__exit__(None, None, None)
            ctx_cm = tc.tile_pool(name="ctx_pool", bufs=1)
            ctx_pool = ctx_cm.__enter__()
            ctxF = {}
            phdw_cm = tc.tile_pool(name="ph_d_w", bufs=1)
            phdw = phdw_cm.__enter__()
            wd_all = phdw.tile([P, 2 * H], BF16, name="wd_all")
            for p2 in range(2):
                nc.sync.dma_start(out=wd_all[:, p2 * H:(p2 + 1) * H],
                                  in_=w_dense[p2 * P:(p2 + 1) * P, :])
            # MLP h4h weights: load early (SBUF region free after QKV)
            whp_cm = tc.tile_pool(name="ph_wh", bufs=1, side="right")
            whp = whp_cm.__enter__()
            wh_all = whp.tile([P, NFC * FC], BF16, name="wh_all")
            # long-lived pool for dense evicts + phase-D/LN4 row work
            phD_cm = tc.tile_pool(name="ph_D", bufs=1, side="right")
            phD = phD_cm.__enter__()

            def dense_partial(t8, pspool):
                b_, qc_ = t8 // 2, t8 % 2
                for jj in range(NFC):
                    ps = pspool.tile([P, TC], F32, tag="dn", bufs=2,
                                     name="ps_dn")
                    for p2 in range(2):
                        nc.tensor.matmul(
                            ps[:, :],
                            wd_all[:, p2 * H + jj * P:
                                   p2 * H + (jj + 1) * P],
                            ctxF[b_][:, qc_ * 2 * TC + p2 * TC:
                                     qc_ * 2 * TC + (p2 + 1) * TC],
                            start=(p2 == 0), stop=(p2 == 1))
                    pt = phD.tile([P, TC], BF16, tag="dpt", bufs=3,
                                  name="dpt")
                    nc.scalar.activation(pt[:, :], ps[:, :], AF.Copy)
                    nc.sync.dma_start(
                        out=rs_d_in[t8][jj * P:(jj + 1) * P, :],
                        in_=pt[:, :])
                nc.gpsimd.collective_compute(
                    "ReduceScatter", ALU.add, replica_groups=RG,
                    ins=[rs_d_in[t8][:, :].opt()],
                    outs=[rs_d_out[t8][:, :].opt()])

            def dense_consume(t8, pspool, stag, qtag, sbufs):
                sl = slice(t8 * TC, (t8 + 1) * TC)
                for m in range(2):
                    if zero_bias:
                        nc.sync.dma_start(
                            out=attn_sb[m][:, sl],
                            in_=rs_d_out[t8][m * P:(m + 1) * P, :])
                    else:
                        tmp = phD.tile([P, TC], BF16, tag="rsb", bufs=2,
                                       name="rsb")
                        nc.sync.dma_start(
                            out=tmp[:, :],
                            in_=rs_d_out[t8][m * P:(m + 1) * P, :])
                        nc.vector.tensor_scalar(
                            attn_sb[m][:, sl], tmp[:, :],
                            bdense_sb[:, m:m + 1], 0.0, ALU.add, ALU.add)
                _stats_t8(nc, phD, pspool, attn_sb, t8,
                          ar3q_in[t8 // 2], ones_bf, stag=stag, qtag=qtag,
                          sbufs=sbufs, slot=t8 % 2)
                if t8 % 2 == 1:
                    p = t8 // 2
                    nc.gpsimd.collective_compute(
                        "AllReduce", ALU.add, replica_groups=RG,
                        ins=[ar3q_in[p][:, :].opt()],
                        outs=[ar3q_out[p][:, :].opt()])

            def ln3_pair(p, pspool, stag, qtag, sbufs):
                ab3 = _ln_rows_batch(nc, phD, ar3q_out[p], f"ln3p{p}",
                                     nrows=2)
                for t8 in range(2 * p, 2 * p + 2):
                    a_b, b2_b = _ln_bcast(nc, phD, ab3, t8 % 2)
                    for m in range(2):
                        hres = phD.tile([P, TC], F32, tag="hres",
                                        bufs=2, name="hres")
                        nc.sync.dma_start(
                            out=hres[:, :],
                            in_=h_res[m * P:(m + 1) * P,
                                      t8 * TC:(t8 + 1) * TC])
                        sl = slice(t8 * TC, (t8 + 1) * TC)
                        t1 = phD.tile([P, TC], F32, tag="t1", name="t1")
                        t2 = phD.tile([P, TC], F32, tag="t2", name="t2")
                        nc.vector.tensor_mul(t1[:, :], attn_sb[m][:, sl],
                                             a_b[:, :])
                        nc.vector.tensor_add(t1[:, :], t1[:, :], b2_b[:, :])
                        nc.vector.tensor_scalar(t2[:, :], t1[:, :],
                                                ln3w_sb[:, m:m + 1],
                                                ln3b_sb[:, m:m + 1],
                                                ALU.mult, ALU.add)
                        nc.vector.tensor_add(ln_in[m][:, sl], t2[:, :],
                                             hres[:, :])
                    _stats_t8(nc, phD, pspool, ln_in, t8, ar2q_in[p],
                              ones_bf, stag=stag, qtag=qtag, sbufs=sbufs,
                              slot=t8 % 2)
                nc.gpsimd.collective_compute(
                    "AllReduce", ALU.add, replica_groups=RG,
                    ins=[ar2q_in[p][:, :].opt()],
                    outs=[ar2q_out[p][:, :].opt()])

            def ln2_pair(p):
                ab2 = _ln_rows_batch(nc, phD, ar2q_out[p], f"ln2p{p}",
                                     nrows=2)
                for t8 in range(2 * p, 2 * p + 2):
                    a_b, b2_b = _ln_bcast(nc, phD, ab2, t8 % 2)
                    tq = t8 % 2
                    for m in range(2):
                        sl = slice(t8 * TC, (t8 + 1) * TC)
                        t1 = phD.tile([P, TC], F32, tag="t1", name="t1")
                        nc.vector.tensor_mul(t1[:, :], ln_in[m][:, sl],
                                             a_b[:, :])
                        nc.vector.tensor_add(t1[:, :], t1[:, :], b2_b[:, :])
                        x2q = phD.tile([P, TC], BF16, tag="x2q", bufs=2,
                                       name="x2q")
                        nc.vector.tensor_scalar(
                            x2q[:, :], t1[:, :], ln2w_sb[:, m:m + 1],
                            ln2b_sb[:, m:m + 1], ALU.mult, ALU.add)
                        nc.sync.dma_start(
                            out=ag_x2_in[p][:, tq * 1024 + m * TC:
                                            tq * 1024 + (m + 1) * TC],
                            in_=x2q[:, :])
                nc.gpsimd.collective_compute(
                    "AllGather", ALU.bypass, replica_groups=RG,
                    ins=[ag_x2_in[p][:, :].opt()],
                    outs=[ag_x2_out[p][:, :].opt()])

            with tc.tile_pool(name="ph_c", bufs=1) as phc, \
                 tc.tile_pool(name="ph_c_ps", bufs=1, space="PSUM") as phcps:
                for b in range(B):
                    ctxF[b] = ctx_pool.tile([P, 2 * S], BF16, tag="ctxF",
                                            bufs=2, name=f"ctxF{b}")
                    if b == 1:
                        for fc in range(NFC):
                            nc.sync.dma_start(
                                out=wh_all[:, fc * FC:(fc + 1) * FC],
                                in_=w_h4h[fc * P:(fc + 1) * P, :])
                    for qc in range(S // TC):
                        t8c = 2 * b + qc
                        ctx_ps = [phcps.tile([P, TC], F32, tag=f"ctx{p}",
                                             bufs=1, name=f"ctx_ps{p}")
                                  for p in range(2)]
                        den_ps = phcps.tile([P, TC], F32, tag="den",
                                            bufs=1, name="den_ps")
                        kts = [kt for kt in range(S // P)
                               if block_status[(kt, qc)] != "skip"]
                        nkt = len(kts)

                        def emit_scores(ki):
                            kt = kts[ki]
                            st = block_status[(kt, qc)]
                            es = []
                            for h in range(HPC):
                                pair, rho = h // 2, h % 2
                                ps_s = phcps.tile([P, TC], F32, tag="s",
                                                  bufs=3, name="ps_s")
                                qoff = pair * T + b * S + qc * TC
                                koff = pair * T + b * S + kts[ki] * P
                                nc.tensor.matmul(
                                    ps_s[:, :],
                                    kT2[rho * HD:(rho + 1) * HD,
                                        koff:koff + P],
                                    qT2[rho * HD:(rho + 1) * HD,
                                        qoff:qoff + TC],
                                    start=True, stop=True)
                                e = phc.tile([P, TC], BF16, tag="e", bufs=6,
                                             name="e")
                                nc.scalar.activation(e[:, :], ps_s[:, :],
                                                     AF.Exp,
                                                     scale=1.0 / np.sqrt(HD))
                                if st == "masked":
                                    i = mask_slot[(kt, qc)]
                                    nc.vector.tensor_mul(
                                        e[:, :], e[:, :],
                                        mask_sb[:, i * TC:(i + 1) * TC])
                                es.append(e)
                            return es

                        def emit_ctx(ki, es):
                            kt = kts[ki]
                            ttg = b * (S // P) + kt
                            for h in range(HPC):
                                pair, rho = h // 2, h % 2
                                nc.tensor.matmul(
                                    ctx_ps[pair][rho * HD:(rho + 1) * HD, :],
                                    v_sb[:, ttg * DC + h * HD:
                                         ttg * DC + (h + 1) * HD],
                                    es[h][:, :],
                                    start=(ki == 0), stop=(ki == nkt - 1))
                                nc.tensor.matmul(
                                    den_ps[32 * h:32 * h + 1, :],
                                    ones_bf[:, 0:1], es[h][:, :],
                                    start=(ki == 0), stop=(ki == nkt - 1),
                                    tile_position=(0, 32 * h))

                        prev = emit_scores(0)
                        for ki in range(1, nkt):
                            cur = emit_scores(ki)
                            emit_ctx(ki - 1, prev)
                            prev = cur
                        emit_ctx(nkt - 1, prev)
                        # batched reciprocal of all 4 head denominators
                        rd = phc.tile([P, TC], F32, tag="rd", bufs=1,
                                      name="rd")
                        rd_bf = phc.tile([P, TC], BF16, tag="rd_bf", bufs=1,
                                         name="rd_bf")
                        nc.vector.reciprocal(rd[:, :], den_ps[:, :])
                        nc.vector.tensor_copy(rd_bf[:, :], rd[:, :])
                        for h in range(HPC):
                            pair, rho = h // 2, h % 2
                            r32 = slice(32 * h, 32 * h + 1)
                            ps_b = phcps.tile([P, TC], F32, tag="s", bufs=3,
                                              name="ps_b")
                            nc.tensor.matmul(ps_b[:, :],
                                             ones_rows_bf[r32, :],
                                             rd_bf[r32, :], start=True,
                                             stop=True,
                                             tile_position=(32 * h, 0))
                            rd_b = phc.tile([P, TC], F32, tag="rd_b", bufs=1,
                                            name="rd_b")
                            nc.vector.tensor_copy(rd_b[:, :], ps_b[:, :])
                            off = qc * 2 * TC + pair * TC
                            hs = slice(rho * HD, (rho + 1) * HD)
                            nc.vector.tensor_mul(ctxF[b][hs, off:off + TC],
                                                 ctx_ps[pair][hs, :],
                                                 rd_b[hs, :])
                        if t8c == 5:
                            ln2_pair(0)
                        dense_partial(t8c, phcps)
                        if t8c >= 2:
                            dense_consume(t8c - 2, phcps, "s", "s", 3)
                        if t8c == 4:
                            ln3_pair(0, phcps, "s", "s", 3)
                        if t8c == 6:
                            ln3_pair(1, phcps, "s", "s", 3)
                        if t8c == 7:
                            ln2_pair(1)
                dense_consume(6, phcps, "s", "s", 3)
            phdw_cm.__exit__(None, None, None)
            ctx_cm.__exit__(None, None, None)
            attn_res_cm.__exit__(None, None, None)

            # =========================================================
            # Phase E+F: h4h+gelu -> 4hh row-parallel partials -> per-t8
            # ReduceScatter; LN4 applies interleaved.
            # PSUM: h(2) + f(2) + st(2) = 6 banks.
            # =========================================================
            with tc.tile_pool(name="ph_e_w", bufs=1) as phew, \
                 tc.tile_pool(name="ph_ef", bufs=1) as phef, \
                 tc.tile_pool(name="ph_ef_ps", bufs=1, space="PSUM") as pheps:
                w4_all = phew.tile([P, (FC // P) * H], BF16, name="w4_all")
                for j in range(FC // P):
                    nc.sync.dma_start(out=w4_all[:, j * H:(j + 1) * H],
                                      in_=w_4hh[j * P:(j + 1) * P, :])
                mlp_sb = [resid.tile([P, T], BF16, tag="colsM", bufs=2,
                                     name=f"mlp_sb{m}") for m in range(2)]

                inter_t = {}

                def h4h_chunk(t8):
                    hh, t8r = t8 // 2, t8 % 2
                    x2c_all = phef.tile([P, NFC * TC], BF16, tag="x2c",
                                        bufs=2, name="x2c_all")
                    for c8 in range(NC):
                        for m2 in range(2):
                            fc = c8 * 2 + m2
                            nc.sync.dma_start(
                                out=x2c_all[:, fc * TC:(fc + 1) * TC],
                                in_=ag_x2_out[hh][c8 * P:(c8 + 1) * P,
                                                  t8r * 1024 + m2 * TC:
                                                  t8r * 1024 + (m2 + 1) * TC])
                    inter = phef.tile([P, (FC // P) * TC], BF16, tag="inter",
                                      bufs=2, name="inter")
                    for g in range(4):
                        ps = [pheps.tile([P, TC], F32, tag=f"h{mi}", bufs=1,
                                         name=f"ps_h{mi}") for mi in range(2)]
                        for fc in range(NFC):
                            for mi in range(2):
                                m = g * 2 + mi
                                nc.tensor.matmul(
                                    ps[mi][:, :],
                                    wh_all[:, fc * FC + m * P:
                                           fc * FC + (m + 1) * P],
                                    x2c_all[:, fc * TC:(fc + 1) * TC],
                                    start=(fc == 0), stop=(fc == NFC - 1))
                        for mi in range(2):
                            m = g * 2 + mi
                            nc.scalar.activation(
                                inter[:, m * TC:(m + 1) * TC], ps[mi][:, :],
                                AF.Gelu_apprx_tanh,
                                bias=bh4h_sb[:, m:m + 1])
                    inter_t[t8] = inter

                def fourhh_partial(t8):
                    inter = inter_t.pop(t8)
                    split = (t8 == NTC - 1)
                    order = ([2 * i for i in range(NFC // 2)]
                             + [2 * i + 1 for i in range(NFC // 2)]
                             if split else range(NFC))
                    for jj in order:
                        ps = pheps.tile([P, TC], F32, tag="f", bufs=3,
                                        name="ps_f")
                        for j in range(FC // P):
                            nc.tensor.matmul(
                                ps[:, :],
                                w4_all[:, j * H + jj * P:
                                       j * H + (jj + 1) * P],
                                inter[:, j * TC:(j + 1) * TC],
                                start=(j == 0), stop=(j == FC // P - 1))
                        pt = phef.tile([P, TC], BF16, tag="pt", bufs=4,
                                       name="pt")
                        nc.vector.tensor_copy(pt[:, :], ps[:, :])
                        if split:
                            nc.sync.dma_start(
                                out=rs_m7_in[jj % 2][(jj // 2) * P:
                                                    (jj // 2 + 1) * P, :],
                                in_=pt[:, :])
                            if jj == NFC - 2 or jj == NFC - 1:
                                h = jj % 2
                                nc.gpsimd.collective_compute(
                                    "ReduceScatter", ALU.add,
                                    replica_groups=RG,
                                    ins=[rs_m7_in[h][:, :].opt()],
                                    outs=[rs_m7_out[h][:, :].opt()])
                        else:
                            nc.sync.dma_start(
                                out=rs_mlp_in[t8][jj * P:(jj + 1) * P, :],
                                in_=pt[:, :])
                    if not split:
                        nc.gpsimd.collective_compute(
                            "ReduceScatter", ALU.add, replica_groups=RG,
                            ins=[rs_mlp_in[t8][:, :].opt()],
                            outs=[rs_mlp_out[t8][:, :].opt()])

                def mlp_consume(t8):
                    sl = slice(t8 * TC, (t8 + 1) * TC)
                    for m in range(2):
                        src_ap = (rs_m7_out[m][:, :] if t8 == NTC - 1
                                  else rs_mlp_out[t8][m * P:(m + 1) * P, :])
                        if zero_bias:
                            nc.sync.dma_start(
                                out=mlp_sb[m][:, sl], in_=src_ap)
                        else:
                            tmp = phef.tile([P, TC], BF16, tag="rsb", bufs=2,
                                            name="rsb")
                            nc.sync.dma_start(out=tmp[:, :], in_=src_ap)
                            nc.vector.tensor_scalar(
                                mlp_sb[m][:, sl], tmp[:, :],
                                b4hh_sb[:, m:m + 1], 0.0, ALU.add, ALU.add)
                    _stats_t8(nc, phD, pheps, mlp_sb, t8,
                              ar4q_in[t8 // 2], ones_bf, slot=t8 % 2)
                    if t8 % 2 == 1:
                        qq = t8 // 2
                        nc.gpsimd.collective_compute(
                            "AllReduce", ALU.add, replica_groups=RG,
                            ins=[ar4q_in[qq][:, :].opt()],
                            outs=[ar4q_out[qq][:, :].opt()])

                def ln4_apply(qq):
                    ab4 = _ln_rows_batch(nc, phD, ar4q_out[qq],
                                         f"ln4q{qq}", nrows=2)
                    for t8 in range(2 * qq, 2 * qq + 2):
                        a_b, b2_b = _ln_bcast(nc, phD, ab4, t8 % 2)
                        for m in range(2):
                            sl = slice(t8 * TC, (t8 + 1) * TC)
                            t1 = phD.tile([P, TC], F32, tag="t1", name="t1")
                            t2 = phD.tile([P, TC], F32, tag="t2", name="t2")
                            nc.vector.tensor_mul(t1[:, :], mlp_sb[m][:, sl],
                                                 a_b[:, :])
                            nc.vector.tensor_add(t1[:, :], t1[:, :],
                                                 b2_b[:, :])
                            nc.vector.tensor_scalar(t2[:, :], t1[:, :],
                                                    ln4w_sb[:, m:m + 1],
                                                    ln4b_sb[:, m:m + 1],
                                                    ALU.mult, ALU.add)
                            ot = phD.tile([P, TC], F32, tag="ot", name="ot")
                            nc.vector.tensor_add(ot[:, :], t2[:, :],
                                                 ln_in[m][:, sl])
                            nc.sync.dma_start(
                                out=out_ext[m * P:(m + 1) * P,
                                            t8 * TC:(t8 + 1) * TC],
                                in_=ot[:, :])

                for t8 in range(NTC):
                    h4h_chunk(t8)
                    if t8 == 0:
                        ln3_pair(2, pheps, "st_s", "st_q", 1)
                    if t8 == 1:
                        dense_consume(7, pheps, "st_s", "st_q", 1)
                        ln2_pair(2)
                    if t8 == 2:
                        ln3_pair(3, pheps, "st_s", "st_q", 1)
                    if t8 == 3:
                        ln2_pair(3)
                    if t8 >= 1:
                        fourhh_partial(t8 - 1)
                    if t8 >= 2:
                        mlp_consume(t8 - 2)
                    if t8 == 5:
                        ln4_apply(0)
                    if t8 == 7:
                        ln4_apply(1)
                fourhh_partial(NTC - 1)
                mlp_consume(6)
                ln4_apply(2)
                mlp_consume(7)
                ln4_apply(3)
            phD_cm.__exit__(None, None, None)
            whp_cm.__exit__(None, None, None)

    nc.compile()
    return nc


def _stats_t8(nc, pool, pspool, rows, t8, ar_in, ones_bf,
              stag="st_s", qtag="st_q", sbufs=1, slot=None):
    """Sum & sumsq over the 256 local features of token-chunk t8 (bf16)."""
    if slot is None:
        slot = t8
    ps_s = pspool.tile([1, TC], F32, tag=stag, bufs=sbufs, name="ps_s")
    ps_q = pspool.tile([1, TC], F32, tag=qtag, bufs=sbufs, name="ps_q")
    sl = slice(t8 * TC, (t8 + 1) * TC)
    osl = slice(slot * TC, (slot + 1) * TC)
    for m in range(2):
        nc.tensor.matmul(ps_s[:, :], ones_bf[:, 0:1], rows[m][:, sl],
                         start=(m == 0), stop=(m == 1))
    for m in range(2):
        sq = pool.tile([P, TC], BF16, tag="sq", bufs=2, name="sq")
        nc.vector.tensor_mul(sq[:, :], rows[m][:, sl], rows[m][:, sl])
        nc.tensor.matmul(ps_q[:, :], ones_bf[:, 0:1], sq[:, :],
                         start=(m == 0), stop=(m == 1))
    tmp_s = pool.tile([1, TC], F32, tag="tmp_s", bufs=1, name="tmp_s")
    tmp_q = pool.tile([1, TC], F32, tag="tmp_q", bufs=1, name="tmp_q")
    nc.vector.tensor_copy(tmp_s[:, :], ps_s[:, :])
    nc.vector.tensor_copy(tmp_q[:, :], ps_q[:, :])
    nc.sync.dma_start(out=ar_in[0:1, osl], in_=tmp_s[:, :])
    nc.sync.dma_start(out=ar_in[1:2, osl], in_=tmp_q[:, :])


def _ln_rows_batch(nc, pool, ar_out, name, nrows=8):
    """Batched LN row math on [nrows,TC] tiles, one reciprocal total."""
    s8 = pool.tile([nrows, TC], F32, tag="lnrb_s8", bufs=1, name=f"{name}_s8")
    q8 = pool.tile([nrows, TC], F32, tag="lnrb_q8", bufs=1, name=f"{name}_q8")
    nc.sync.dma_start(out=s8[:, :], in_=ar_out[0:1, :])
    nc.sync.dma_start(out=q8[:, :], in_=ar_out[1:2, :])
    mu = pool.tile([nrows, TC], F32, tag="lnrb_mu", bufs=1, name=f"{name}_mu")
    a8 = pool.tile([nrows, TC], F32, tag="lnrb_a8", bufs=1, name=f"{name}_a8")
    b28 = pool.tile([nrows, TC], F32, tag="lnrb_b28", bufs=1,
                    name=f"{name}_b28")
    nc.vector.tensor_scalar_mul(mu[:, :], s8[:, :], 1.0 / H)
    nc.vector.tensor_scalar_mul(q8[:, :], q8[:, :], 1.0 / H)
    nc.vector.tensor_mul(b28[:, :], mu[:, :], mu[:, :])
    nc.vector.tensor_sub(q8[:, :], q8[:, :], b28[:, :])
    nc.scalar.activation(q8[:, :], q8[:, :], AF.Sqrt, bias=EPS)
    nc.vector.reciprocal(a8[:, :], q8[:, :])
    nc.vector.tensor_mul(b28[:, :], mu[:, :], a8[:, :])
    nc.vector.tensor_scalar_mul(b28[:, :], b28[:, :], -1.0)
    return a8, b28


def _ln_bcast(nc, pool, ab, t8):
    """Extract row t8 from the batched (a8,b28) and broadcast to [P,TC]."""
    a8, b28 = ab
    a_row = pool.tile([1, TC], F32, tag="a_row", name="a_row")
    b2_row = pool.tile([1, TC], F32, tag="b2_row", name="b2_row")
    nc.sync.dma_start(out=a_row[:, :], in_=a8[t8:t8 + 1, :])
    nc.sync.dma_start(out=b2_row[:, :], in_=b28[t8:t8 + 1, :])
    a_b = pool.tile([P, TC], F32, tag="a_b", name="a_b")
    b2_b = pool.tile([P, TC], F32, tag="b2_b", name="b2_b")
    nc.gpsimd.partition_broadcast(a_b[:, :], a_row[:, :])
    nc.gpsimd.partition_broadcast(b2_b[:, :], b2_row[:, :])
    return a_b, b2_b


# ----------------------------------------------------------------------
_cache = {}


def _get_program(mask_np, zero_bv, zero_bias):
    key = (mask_np.tobytes(), zero_bv, zero_bias)
    kh = hash(key)
    if kh not in _cache:
        _cache[kh] = build_program(_causal_block_status(mask_np), zero_bv,
                                   zero_bias)
    return _cache[kh]


def kernel(hidden_states, mask, ln1_w, ln1_b, w_qkv, b_qkv, w_dense, b_dense,
           ln3_w, ln3_b, ln2_w, ln2_b, w_h4h, b_h4h, w_4hh, b_4hh,
           ln4_w, ln4_b):
    hidden_states = np.asarray(hidden_states, np.float32)
    mask2d = np.asarray(mask, np.float32).reshape(S, S)
    w_qkv = np.asarray(w_qkv, np.float32)
    b_qkv = np.asarray(b_qkv, np.float32)
    w_dense = np.asarray(w_dense, np.float32)
    w_h4h = np.asarray(w_h4h, np.float32)
    w_4hh = np.asarray(w_4hh, np.float32)

    zero_bv = bool(np.all(b_qkv[2 * H:] == 0.0))
    zero_bias = bool(np.all(b_qkv[:2 * H] == 0.0)
                     and np.all(np.asarray(b_dense) == 0.0)
                     and np.all(np.asarray(b_4hh) == 0.0))
    prog = _get_program(mask2d, zero_bv, zero_bias)

    hT = np.ascontiguousarray(hidden_states.reshape(T, H).T)
    maskT_bf = np.ascontiguousarray(mask2d.T).astype(bf16)

    in_maps = []
    for c in range(NC):
        qs = slice(c * DC, (c + 1) * DC)
        wq_c = np.concatenate([w_qkv[:, c * DC:(c + 1) * DC],
                               w_qkv[:, H + c * DC:H + (c + 1) * DC],
                               w_qkv[:, 2 * H + c * DC:2 * H + (c + 1) * DC]],
                              axis=1)
        b_qk_c = np.concatenate([b_qkv[c * DC:(c + 1) * DC],
                                 b_qkv[H + c * DC:H + (c + 1) * DC]])
        b_v_c = b_qkv[2 * H + c * DC:2 * H + (c + 1) * DC]
        im = {
            "h_ln1": np.ascontiguousarray(
                hT[:, c * TC:(c + 1) * TC]).astype(bf16),
            "h_res": np.ascontiguousarray(hT[qs, :]),
            "ln1_w": np.asarray(ln1_w, np.float32).reshape(H, 1),
            "ln1_b": np.asarray(ln1_b, np.float32).reshape(H, 1),
            "ln2_w": np.asarray(ln2_w, np.float32)[qs].reshape(DC, 1),
            "ln2_b": np.asarray(ln2_b, np.float32)[qs].reshape(DC, 1),
            "ln3_w": np.asarray(ln3_w, np.float32)[qs].reshape(DC, 1),
            "ln3_b": np.asarray(ln3_b, np.float32)[qs].reshape(DC, 1),
            "ln4_w": np.asarray(ln4_w, np.float32)[qs].reshape(DC, 1),
            "ln4_b": np.asarray(ln4_b, np.float32)[qs].reshape(DC, 1),
            "w_qkv": np.ascontiguousarray(wq_c).astype(bf16),
            "b_qk": np.ascontiguousarray(b_qk_c).reshape(2 * DC, 1),
            "b_v": np.ascontiguousarray(b_v_c).reshape(1, DC),
            "w_dense": np.ascontiguousarray(w_dense[qs, :]).astype(bf16),
            "b_dense": np.asarray(b_dense, np.float32)[qs].reshape(DC, 1),
            "w_h4h": np.ascontiguousarray(
                w_h4h[:, c * FC:(c + 1) * FC]).astype(bf16),
            "b_h4h": np.asarray(b_h4h, np.float32)[
                c * FC:(c + 1) * FC].reshape(FC, 1),
            "w_4hh": np.ascontiguousarray(
                w_4hh[c * FC:(c + 1) * FC, :]).astype(bf16),
            "b_4hh": np.asarray(b_4hh, np.float32)[qs].reshape(DC, 1),
            "maskT": maskT_bf,
        }
        in_maps.append(im)

    res = run_bass_kernel_spmd(prog, in_maps, core_ids=list(range(NC)))
    outT = np.concatenate([res.results[c]["out"] for c in range(NC)], axis=0)
    return np.ascontiguousarray(outT.T).reshape(B, S, H).astype(np.float32)
